# revision 16
# baseline (speedup 1.0000x reference)
"""BHGNN Trainium2 kernel (8 NeuronCores, graph-level data parallel).

Per core: 64 graphs x 128 nodes. Pipeline:
  - x^T loaded (fp16 cast); per-graph PE matmuls give al, XW; att_r matvec
    + PE broadcast replicates ar across all partitions.
  - GPSIMD ap_gather fetches ar[col] per edge; DVE packs col into the low
    7 mantissa bits of z+1 so one 16-channel Batcher sort yields sparsemax
    (tau = max_k (cum_k-1)/k) AND exact duplicate merging by bit equality.
  - GPSIMD local_scatter builds dense per-graph attention A' (incl. self
    loop); PE transposes A' and computes agg^T; h = relu(0.5*agg + b).
  - Pooling via strided DVE reduces -> 6 hypernodes/graph; outer softmax
    attention, two tiny GCNs, readout, MLP heads in f32/fp32r.
"""
import os
from contextlib import ExitStack

import numpy as np

import concourse.bass as bass
import concourse.mybir as mybir
import concourse.bacc as bacc
import concourse.tile as tile
from concourse.bass_utils import run_bass_kernel_spmd

FP16 = mybir.dt.float16
F32 = mybir.dt.float32
F32R = mybir.dt.float32r
I32 = mybir.dt.int32
I16 = mybir.dt.int16
AF = mybir.ActivationFunctionType
OP = mybir.AluOpType
X = mybir.AxisListType.X

P = 128          # partitions = nodes per graph
G = 64           # graphs per core
NN = P * G       # nodes per core (8192)
K = 16           # edges per node
NH = 128
H = 6
NCORES = 8
NHYP = G * H     # hypernodes per core (384)

WNAMES = [
    ("watt2", [P, 2]), ("winner", [P, P]), ("binner", [P, 1]),
    ("wattm", [P, 2]), ("wattx", [P, 2]),
    ("wout1a", [P, P]), ("wout1b", [P, P]), ("bout1", [P, 1]),
    ("wout2", [P, P]), ("bout2", [P, 1]),
    ("f1m00", [P, P]), ("f1m10", [P, P]), ("f1m01", [P, P]), ("f1m11", [P, P]),
    ("b1m0", [P, 1]), ("b1m1", [P, 1]),
    ("f2m0", [P, P]), ("f2m1", [P, P]), ("b2m", [P, 1]),
    ("f1v00", [P, P]), ("f1v10", [P, P]), ("f1v01", [P, P]), ("f1v11", [P, P]),
    ("b1v0", [P, 1]), ("b1v1", [P, 1]),
    ("f2v0", [P, P]), ("f2v1", [P, P]), ("b2v", [P, 1]),
    ("clsmW", [P, 10]), ("clsmb", [10, 1]),
    ("clsvW", [P, 10]), ("clsvb", [10, 1]),
]

# Batcher odd-even merge sort network for 16 channels. Each layer:
# (channel-dim factors, lo index, hi index); slicing the factored channel
# dims gives matching lo/hi comparator sets.
SORT_LAYERS = [
    ((8, 2), (slice(None), slice(0, 1)), (slice(None), slice(1, 2))),
    ((4, 4), (slice(None), slice(0, 2)), (slice(None), slice(2, 4))),
    ((4, 4), (slice(None), slice(1, 2)), (slice(None), slice(2, 3))),
    ((2, 8), (slice(None), slice(0, 4)), (slice(None), slice(4, 8))),
    ((2, 8), (slice(None), slice(2, 4)), (slice(None), slice(4, 6))),
    ((2, 4, 2), (slice(None), slice(0, 3), slice(1, 2)),
     (slice(None), slice(1, 4), slice(0, 1))),
    ((1, 16), (slice(None), slice(0, 8)), (slice(None), slice(8, 16))),
    ((1, 16), (slice(None), slice(4, 8)), (slice(None), slice(8, 12))),
    ((4, 4), (slice(0, 3), slice(2, 4)), (slice(1, 4), slice(0, 2))),
    ((8, 2), (slice(0, 7), slice(1, 2)), (slice(1, 8), slice(0, 1))),
]


def _chslice(ap, factors, idx):
    names = "abcd"[: len(factors)]
    pat = f"p ({' '.join(names)} g) -> p {' '.join(names)} g"
    v = ap.rearrange(pat, **{n: f for n, f in zip(names, factors)})
    return v[(slice(None),) + idx + (slice(None),)]


def build_nc():
    nc = bacc.Bacc("TRN2", target_bir_lowering=False, debug=False,
                   num_devices=NCORES)
    xT_d = nc.declare_dram_parameter("xT", [P, NN], F32, isOutput=False)
    col_d = nc.declare_dram_parameter("colr", [P, G * K], I32, isOutput=False)
    wd = {}
    for nm, shp in WNAMES:
        wd[nm] = nc.declare_dram_parameter(nm, shp, F32, isOutput=False)
    out_d = nc.declare_dram_parameter("out", [G, 148], F32, isOutput=True)

    with tile.TileContext(nc) as tc, ExitStack() as ctx:
        pp = ctx.enter_context(tc.tile_pool(name="persist", bufs=1))
        wk = ctx.enter_context(tc.tile_pool(name="work", bufs=1))
        pmm = ctx.enter_context(
            tc.tile_pool(name="psum", bufs=4, space="PSUM"))

        ps_ctr = [0]

        def ps(shape, dtype=F32):
            ps_ctr[0] += 1
            return pmm.tile(shape, dtype, tag="ps",
                            name=f"pst{ps_ctr[0]}")

        # ---------------- weights / constants ----------------
        w = {}
        for nm, shp in WNAMES:
            t = pp.tile(shp, F32, tag=f"w_{nm}")
            nc.sync.dma_start(t[:], wd[nm].ap())
            w[nm] = t
        watt2_h = pp.tile([P, 2], FP16, tag="watt2h")
        nc.gpsimd.dma_start(watt2_h[:], wd["watt2"].ap())
        winner_h = pp.tile([P, P], FP16, tag="winnerh")
        nc.gpsimd.dma_start(winner_h[:], wd["winner"].ap())

        iota_row = pp.tile([P, P], I32, tag="iota_row")
        nc.gpsimd.iota(iota_row[:], pattern=[[1, P]], channel_multiplier=0)
        iota_part = pp.tile([P, 1], I32, tag="iota_part")
        nc.gpsimd.iota(iota_part[:], pattern=[[0, 1]], channel_multiplier=1)
        g128rep = pp.tile([P, K * G], I32, tag="g128rep")   # (j,g): 128*(g%8)
        nc.gpsimd.iota(g128rep[:], pattern=[[0, K], [0, 8], [P, 8]],
                       channel_multiplier=0)
        selfrep = pp.tile([P, K * G], I32, tag="selfrep")   # 128*(g%8)+p
        nc.gpsimd.iota(selfrep[:], pattern=[[0, K], [0, 8], [P, 8]],
                       channel_multiplier=1)
        self16 = pp.tile([P, G], I16, tag="self16")         # 128*(g%8)+p
        nc.gpsimd.iota(self16[:], pattern=[[0, 8], [P, 8]],
                       channel_multiplier=1)

        iota_rowf = pp.tile([P, P], F32, tag="iota_rowf")
        nc.vector.tensor_copy(iota_rowf[:], iota_row[:])
        iota_partf = pp.tile([P, 1], F32, tag="iota_partf")
        nc.vector.tensor_copy(iota_partf[:], iota_part[:])
        ident32 = pp.tile([P, P], F32, tag="ident32")
        nc.vector.tensor_scalar(ident32[:], iota_rowf[:], iota_partf[:, 0:1],
                                None, op0=OP.is_equal)
        ident16 = pp.tile([P, P], FP16, tag="ident16")
        nc.vector.tensor_copy(ident16[:], ident32[:])
        ones_col = pp.tile([1, P], F32, tag="ones_col")
        nc.vector.memset(ones_col[:], 1.0)
        ones16 = pp.tile([1, P], FP16, tag="ones16")
        nc.vector.memset(ones16[:], 1.0)
        neg1_16 = pp.tile([P, K * G], I16, tag="neg1")
        nc.vector.memset(neg1_16[:], -1)
        cntinv = pp.tile([P, H * G], F32, tag="cntinv")     # (g, r)
        nc.vector.memset(cntinv[:], 1.0 / 21.0)
        nc.vector.memset(
            cntinv.rearrange("p (g r) -> p g r", g=G)[:, :, 0:2],
            1.0 / 22.0)
        # mask16[p, r] = ((p & 15) == r), used to de-interleave ap_gather out
        iota_r16 = pp.tile([P, 16], I32, tag="iota_r16")
        nc.gpsimd.iota(iota_r16[:], pattern=[[1, 16]], channel_multiplier=0)
        pmod = pp.tile([P, 1], I32, tag="pmod")
        nc.vector.tensor_scalar(pmod[:], iota_part[:], 15, None,
                                op0=OP.bitwise_and)
        r16f = pp.tile([P, 16], F32, tag="r16f")
        nc.vector.tensor_copy(r16f[:], iota_r16[:])
        pmodf = pp.tile([P, 1], F32, tag="pmodf")
        nc.vector.tensor_copy(pmodf[:], pmod[:])
        mask16 = pp.tile([P, 16], F32, tag="mask16")
        nc.vector.tensor_scalar(mask16[:], r16f[:], pmodf[:, 0:1], None,
                                op0=OP.is_equal)

        # ---------------- x^T (fp16 cast) and col ----------------
        xT = pp.tile([P, NN], FP16, tag="xT")
        for c in range(8):
            sl = slice(c * 1024, (c + 1) * 1024)
            nc.gpsimd.dma_start(xT[:, sl], xT_d.ap()[:, sl])
        colr = pp.tile([P, G * K], I32, tag="colr")
        nc.sync.dma_start(colr[:], col_d.ap())

        # ---------------- stage A: al, XW, ar ----------------
        al_sb = wk.tile([P, G], F32, tag="al")
        xw16 = wk.tile([P, G * P], FP16, tag="xw16")
        for b in range(G // 4):
            ps_xw = ps([P, 512])
            ps_al = ps([P, 4])
            for i in range(4):
                g = b * 4 + i
                lhs = xT[:, g * P:(g + 1) * P]
                nc.tensor.matmul(ps_al[:, i:i + 1], lhs, watt2_h[:, 0:1])
                nc.tensor.matmul(ps_xw[:, i * P:(i + 1) * P], lhs,
                                 winner_h[:])
            nc.vector.tensor_copy(al_sb[:, b * 4:(b + 1) * 4], ps_al[:])
            nc.scalar.copy(xw16[:, b * 512:(b + 1) * 512], ps_xw[:])

        # ---------------- stage C: gather ar[col], de-interleave ----------
        # Per half of 32 graphs: ar values for those graphs live at flat
        # index g*128+p - hf*4096, i.e. col & 4095.
        colm = wk.tile([P, G * K], I32, tag="q1")
        nc.vector.tensor_scalar(colm[:], colr[:], 4095, None,
                                op0=OP.bitwise_and)
        idx16 = wk.tile([P, G * K], I16, tag="idx16")
        nc.vector.tensor_copy(idx16[:], colm[:])
        zch = wk.tile([P, K * G], F32, tag="q2")            # (j, g)
        mask_b = mask16.rearrange("p (a b r) -> p a b r", a=1, b=1) \
            .broadcast_to([P, 32, K, 16])
        for hf in range(2):
            ar_flat = wk.tile([1, NN // 2], FP16, tag="bigA")
            for c in range(8):
                cc = hf * 8 + c
                ps_ar = ps([1, 512])
                nc.tensor.matmul(ps_ar[:], watt2_h[:, 1:2],
                                 xT[:, cc * 512:(cc + 1) * 512])
                nc.vector.tensor_copy(ar_flat[:, c * 512:(c + 1) * 512],
                                      ps_ar[:])
            ar_all = wk.tile([P, NN // 2], F32, tag="big32")
            for c in range(8):
                ps_b = ps([P, 512])
                nc.tensor.matmul(
                    ps_b[:], ones16[:],
                    ar_flat[:, c * 512:(c + 1) * 512])
                nc.scalar.copy(ar_all[:, c * 512:(c + 1) * 512], ps_b[:])
            gath = wk.tile([P, NN], F32, tag="bigA")
            nc.gpsimd.ap_gather(gath[:], ar_all[:],
                                idx16[:, hf * 512:(hf + 1) * 512],
                                channels=P, num_elems=NN // 2, d=1,
                                num_idxs=NN)
            gsel = wk.tile([P, NN], F32, tag="big32")
            gv = gath.rearrange("p (g j r) -> p g j r", g=32, j=K)
            nc.vector.tensor_tensor(
                gsel.rearrange("p (g j r) -> p g j r", g=32, j=K),
                gv, mask_b, op=OP.mult)
            nc.vector.tensor_reduce(
                zch.rearrange("p (j g) -> p g j", j=K)[:, hf * 32:
                                                       (hf + 1) * 32],
                gsel.rearrange("p (g j r) -> p g j r", g=32, j=K),
                axis=X, op=OP.add)

        # ---------------- stage D: z1, pack, sort, sparsemax, dedup -------
        al1 = wk.tile([P, G], F32, tag="al1")
        nc.vector.tensor_scalar_add(al1[:], al_sb[:], 1.0)
        al1rep = wk.tile([P, K * G], F32, tag="q3")
        nc.vector.tensor_copy(al1rep[:, 0:G], al1[:])
        for d in (1, 2, 4, 8):
            nc.vector.tensor_copy(al1rep[:, d * G:2 * d * G],
                                  al1rep[:, 0:d * G])
        zp = wk.tile([P, K * G], F32, tag="zp")
        nc.vector.tensor_add(zp[:], zch[:], al1rep[:])
        nc.vector.tensor_scalar_max(zp[:], zp[:], 1.0)
        col7 = wk.tile([P, K * G], I32, tag="q4")
        nc.vector.tensor_copy(col7[:],
                              colr.rearrange("p (g j) -> p j g", g=G))
        nc.vector.tensor_scalar(col7[:], col7[:], 127, None,
                                op0=OP.bitwise_and)
        zpi = zp[:].bitcast(I32)
        nc.vector.tensor_scalar(zpi, zpi, -128, None, op0=OP.bitwise_and)
        nc.vector.tensor_tensor(zpi, zpi, col7[:], op=OP.bitwise_or)

        sorttmp = wk.tile([P, 8 * G], F32, tag="sorttmp")
        for factors, lo_i, hi_i in SORT_LAYERS:
            lo = _chslice(zp, factors, lo_i)
            hi = _chslice(zp, factors, hi_i)
            ext = tuple(len(range(*s.indices(f)))
                        for s, f in zip(lo_i, factors))
            npair = int(np.prod(ext))
            tmp = _chslice(sorttmp[:, 0:npair * G], ext,
                           tuple(slice(None) for _ in ext))
            nc.vector.tensor_tensor(tmp, lo, hi, op=OP.min)
            nc.vector.tensor_tensor(lo, lo, hi, op=OP.max)
            nc.vector.tensor_copy(hi, tmp)

        cum = wk.tile([P, K * G], F32, tag="q2")
        nc.vector.tensor_copy(cum[:, 0:G], zp[:, 0:G])
        for k in range(1, K):
            nc.vector.tensor_add(cum[:, k * G:(k + 1) * G],
                                 cum[:, (k - 1) * G:k * G],
                                 zp[:, k * G:(k + 1) * G])
        tau = wk.tile([P, G], F32, tag="tau")
        tmp64 = wk.tile([P, G], F32, tag="tmp64")
        nc.vector.tensor_scalar(tau[:], cum[:, 0:G], -1.0, 1.0,
                                op0=OP.add, op1=OP.mult)
        for k in range(2, K + 1):
            nc.vector.tensor_scalar(tmp64[:], cum[:, (k - 1) * G:k * G],
                                    -1.0, 1.0 / k, op0=OP.add, op1=OP.mult)
            nc.vector.tensor_max(tau[:], tau[:], tmp64[:])
        taurep = wk.tile([P, K * G], F32, tag="q1")
        nc.vector.tensor_copy(taurep[:, 0:G], tau[:])
        for d in (1, 2, 4, 8):
            nc.vector.tensor_copy(taurep[:, d * G:2 * d * G],
                                  taurep[:, 0:d * G])
        attr = wk.tile([P, K * G], F32, tag="attr")
        nc.vector.tensor_sub(attr[:], zp[:], taurep[:])
        nc.vector.tensor_scalar_max(attr[:], attr[:], 0.0)

        # eq_k = (zp_k == zp_{k-1}), stored with one zero pad channel at
        # the end so dead_k = eqt[k+1] is a plain shifted view.
        eqt = wk.tile([P, (K + 1) * G], F32, tag="eqt")
        nc.vector.memset(eqt[:, 0:G], 0.0)
        nc.vector.memset(eqt[:, K * G:], 0.0)
        nc.vector.tensor_tensor(eqt[:, G:K * G], zp[:, G:],
                                zp[:, 0:(K - 1) * G], op=OP.is_equal)
        for k in range(1, K):
            nc.vector.tensor_mul(tmp64[:], eqt[:, k * G:(k + 1) * G],
                                 attr[:, (k - 1) * G:k * G])
            nc.vector.tensor_add(attr[:, k * G:(k + 1) * G],
                                 attr[:, k * G:(k + 1) * G], tmp64[:])
        deadt = eqt[:, G:]                             # [P, K*G] view

        idxg = wk.tile([P, K * G], I32, tag="q2")
        nc.vector.tensor_scalar(idxg[:], zp[:].bitcast(I32), 127, None,
                                op0=OP.bitwise_and)
        nc.vector.tensor_tensor(idxg[:], idxg[:], g128rep[:], op=OP.add)
        smf = wk.tile([P, K * G], F32, tag="q3")
        nc.vector.tensor_tensor(smf[:], idxg[:], selfrep[:], op=OP.is_equal)
        ndead = wk.tile([P, K * G], F32, tag="q4")
        nc.vector.tensor_scalar(ndead[:], deadt, -1.0, 1.0,
                                op0=OP.mult, op1=OP.add)
        nc.vector.tensor_mul(smf[:], smf[:], ndead[:])
        nc.vector.tensor_add(attr[:], attr[:], smf[:])
        absb = wk.tile([P, G], F32, tag="absb")
        t512 = wk.tile([P, 8 * G], F32, tag="sorttmp")
        nc.vector.tensor_add(t512[:], smf[:, 0:8 * G], smf[:, 8 * G:])
        nc.vector.tensor_add(t512[:, 0:4 * G], t512[:, 0:4 * G],
                             t512[:, 4 * G:8 * G])
        nc.vector.tensor_add(t512[:, 0:2 * G], t512[:, 0:2 * G],
                             t512[:, 2 * G:4 * G])
        nc.vector.tensor_add(absb[:], t512[:, 0:G], t512[:, G:2 * G])

        # ---------------- stage D2: scatter streams (g, k=18) -------------
        sidx = wk.tile([P, G * 18], I16, tag="sidx")
        sval = wk.tile([P, G * 18], FP16, tag="sval")
        sidx_v = sidx.rearrange("p (g k) -> p k g", g=G)
        sval_v = sval.rearrange("p (g k) -> p k g", g=G)
        idxg_v = idxg.rearrange("p (k g) -> p k g", k=K)
        attr_v = attr.rearrange("p (k g) -> p k g", k=K)
        deadt_v = deadt.rearrange("p (k g) -> p k g", k=K)
        neg1_v = neg1_16.rearrange("p (k g) -> p k g", k=K)
        # idx_final = dead ? -1 : idxg  ==  (idxg + 1)*(1 - dead) - 1
        idxf = wk.tile([P, K * G], F32, tag="q1")
        nc.vector.tensor_copy(idxf[:], idxg[:])
        nc.vector.scalar_tensor_tensor(idxf[:], idxf[:], 1.0, ndead[:],
                                       op0=OP.add, op1=OP.mult)
        nc.vector.tensor_scalar_add(idxf[:], idxf[:], -1.0)
        nc.vector.tensor_copy(sidx_v[:, 0:K],
                              idxf.rearrange("p (k g) -> p k g", k=K))
        nc.vector.tensor_copy(sval_v[:, 0:K], attr_v)
        nc.vector.tensor_copy(sidx_v[:, 16], self16[:])
        absbi = wk.tile([P, G], I32, tag="absbi")
        nc.vector.tensor_copy(absbi[:], absb[:])
        nc.vector.copy_predicated(sidx_v[:, 16], absbi[:], neg1_16[:, 0:G])
        nc.vector.tensor_scalar(sval_v[:, 16], absb[:], -1.0, 1.0,
                                op0=OP.mult, op1=OP.add)
        nc.vector.memset(sidx_v[:, 17], -1)
        nc.vector.memset(sval_v[:, 17], 0.0)

        # ---------------- stage E: scatter -> A, transpose, agg -----------
        A = wk.tile([P, G * P], FP16, tag="bigA")
        for wnd in range(8):
            nc.gpsimd.local_scatter(
                A[:, wnd * 1024:(wnd + 1) * 1024],
                sval[:, wnd * 144:(wnd + 1) * 144],
                sidx[:, wnd * 144:(wnd + 1) * 144],
                channels=P, num_elems=1024, num_idxs=144)
        AT = wk.tile([P, G * P], FP16, tag="big32")
        for b in range(G // 4):
            ps_t = ps([P, 512], FP16)
            for i in range(4):
                g = b * 4 + i
                nc.tensor.transpose(ps_t[:, i * P:(i + 1) * P],
                                    A[:, g * P:(g + 1) * P], ident16[:])
            nc.scalar.copy(AT[:, b * 512:(b + 1) * 512], ps_t[:])
        hT = wk.tile([P, G * P], FP16, tag="hT")
        for b in range(G // 4):
            ps_a = ps([P, 512])
            for i in range(4):
                g = b * 4 + i
                nc.tensor.matmul(ps_a[:, i * P:(i + 1) * P],
                                 xw16[:, g * P:(g + 1) * P],
                                 AT[:, g * P:(g + 1) * P])
            nc.scalar.activation(hT[:, b * 512:(b + 1) * 512], ps_a[:],
                                 AF.Relu, bias=w["binner"][:, 0:1],
                                 scale=0.5)

        # ---------------- stage F: pooling ----------------
        xhm = wk.tile([P, NHYP], F32, tag="xw16")            # (g, r) mean
        xhx = wk.tile([P, NHYP], F32, tag="q1")            # (g, r) max
        hT_v = hT.rearrange("p (g n) -> p g n", g=G)
        hT_seg = hT_v[:, :, 0:126].rearrange("p g (kk r) -> p g r kk", r=H)
        xhm_v = xhm.rearrange("p (g r) -> p g r", g=G)
        xhx_v = xhx.rearrange("p (g r) -> p g r", g=G)
        nc.vector.tensor_reduce(xhm_v, hT_seg, axis=X, op=OP.add)
        nc.vector.tensor_tensor(xhm_v[:, :, 0:2], xhm_v[:, :, 0:2],
                                hT_v[:, :, 126:128], op=OP.add)
        nc.vector.tensor_mul(xhm[:], xhm[:], cntinv[:])
        nc.vector.tensor_reduce(xhx_v, hT_seg, axis=X, op=OP.max)
        nc.vector.tensor_tensor(xhx_v[:, :, 0:2], xhx_v[:, :, 0:2],
                                hT_v[:, :, 126:128], op=OP.max)

        # ---------------- stage G: outer attention ----------------
        ps_w = ps([2, NHYP])
        nc.tensor.matmul(ps_w[:], w["wattm"][:],
                         xhm[:], start=True, stop=False)
        nc.tensor.matmul(ps_w[:], w["wattx"][:],
                         xhx[:], start=False, stop=True)
        wlr2 = wk.tile([2, NHYP], F32, tag="wlr2")
        nc.vector.tensor_copy(wlr2[:], ps_w[:])
        wlr = wk.tile([G, 12], F32, tag="wlr")
        nc.sync.dma_start(wlr[:, 0:6], wlr2[0:1, :])
        nc.sync.dma_start(wlr[:, 6:12], wlr2[1:2, :])
        whm = wk.tile([G, 36], F32, tag="whm")
        whm_v = whm.rearrange("g (r s) -> g r s", r=H)
        for r in range(H):
            nc.vector.tensor_scalar(whm_v[:, r], wlr[:, 6:12],
                                    wlr[:, r:r + 1], None, op0=OP.add)
        wt = wk.tile([G, 36], F32, tag="wt36")
        wt_v = wt.rearrange("g (r s) -> g r s", r=H)
        nc.vector.tensor_scalar_min(wt[:], whm[:], 0.0)
        nc.vector.tensor_scalar_max(whm[:], whm[:], 0.0)
        nc.vector.scalar_tensor_tensor(whm[:], wt[:], 0.2, whm[:],
                                       op0=OP.mult, op1=OP.add)
        rmax = wk.tile([G, H], F32, tag="rmax")
        nc.vector.tensor_tensor(wt_v[:, :, 0:3], whm_v[:, :, 0:3],
                                whm_v[:, :, 3:6], op=OP.max)
        nc.vector.tensor_tensor(rmax[:], wt_v[:, :, 0], wt_v[:, :, 1],
                                op=OP.max)
        nc.vector.tensor_tensor(rmax[:], rmax[:], wt_v[:, :, 2], op=OP.max)
        for r in range(H):
            nc.vector.tensor_scalar(whm_v[:, r], whm_v[:, r],
                                    rmax[:, r:r + 1], None, op0=OP.subtract)
        nc.scalar.activation(whm[:], whm[:], AF.Exp)
        rsum = wk.tile([G, H], F32, tag="rsum")
        nc.vector.tensor_tensor(wt_v[:, :, 0:3], whm_v[:, :, 0:3],
                                whm_v[:, :, 3:6], op=OP.add)
        nc.vector.tensor_tensor(rsum[:], wt_v[:, :, 0], wt_v[:, :, 1],
                                op=OP.add)
        nc.vector.tensor_tensor(rsum[:], rsum[:], wt_v[:, :, 2], op=OP.add)
        nc.vector.reciprocal(rsum[:], rsum[:])
        for r in range(H):
            nc.vector.tensor_scalar(whm_v[:, r], whm_v[:, r],
                                    rsum[:, r:r + 1], None, op0=OP.mult)
        ahflat = wk.tile([1, G * 36], F32, tag="idx16")
        nc.sync.dma_start(ahflat[:], whm[:])
        ahrep = wk.tile([P, G * 36], F32, tag="hT")
        for c in range(5):
            lo = c * 512
            n = min(512, G * 36 - lo)
            ps_b2 = ps([P, 512])
            nc.tensor.matmul(ps_b2[:, 0:n], ones_col[:],
                             ahflat[:, lo:lo + n])
            nc.scalar.copy(ahrep[:, lo:lo + n], ps_b2[:, 0:n])
        ah_v = ahrep.rearrange("p (g q) -> p g q", g=G)

        def outer_gcn(xin_m, xin_x, wa, wb, bias, name):
            p1 = ps([P, NHYP])
            if xin_x is None:
                nc.tensor.matmul(p1[:], wa[:],
                                 xin_m[:])
            else:
                nc.tensor.matmul(p1[:], wa[:],
                                 xin_m[:], start=True,
                                 stop=False)
                nc.tensor.matmul(p1[:], wb[:],
                                 xin_x[:], start=False,
                                 stop=True)
            xwT = wk.tile([P, NHYP], F32, tag="xwT")
            nc.vector.tensor_copy(xwT[:], p1[:])
            agg = wk.tile([P, NHYP], F32, tag="agg")
            agg_v = agg.rearrange("p (g r) -> p g r", g=G)
            xw_v = xwT.rearrange("p (g s) -> p g s", g=G)
            tmpa = wk.tile([P, G], F32, tag="tmpa")
            for r in range(H):
                for s in range(H):
                    if s == 0:
                        nc.vector.tensor_mul(agg_v[:, :, r], xw_v[:, :, s],
                                             ah_v[:, :, r * H + s])
                    else:
                        nc.vector.tensor_mul(tmpa[:], xw_v[:, :, s],
                                             ah_v[:, :, r * H + s])
                        nc.vector.tensor_tensor(agg_v[:, :, r],
                                                agg_v[:, :, r], tmpa[:],
                                                op=OP.add)
            nc.vector.tensor_add(agg[:], agg[:], xwT[:])
            zT = wk.tile([P, NHYP], F32, tag="zT")
            nc.scalar.activation(zT[:], agg[:], AF.Relu, bias=bias[:, 0:1],
                                 scale=0.5)
            return zT

        z1h = outer_gcn(xhm, xhx, w["wout1a"], w["wout1b"], w["bout1"], "o1")
        z2h = outer_gcn(z1h, None, w["wout2"], None, w["bout2"], "o2")

        x1m = wk.tile([P, G], F32, tag="x1m")
        x1x = wk.tile([P, G], F32, tag="x1x")
        z2_v = z2h.rearrange("p (g r) -> p g r", g=G)
        nc.vector.tensor_reduce(x1m[:], z2_v, axis=X, op=OP.add)
        nc.vector.tensor_scalar_mul(x1m[:], x1m[:], 1.0 / H)
        nc.vector.tensor_reduce(x1x[:], z2_v, axis=X, op=OP.max)

        # ---------------- MLP heads ----------------
        def head(pfx, xm, xx):
            m1 = []
            for j in range(2):
                p2 = ps([P, G])
                nc.tensor.matmul(p2[:], w[f"f1{pfx}0{j}"][:],
                                 xm[:], start=True, stop=False)
                nc.tensor.matmul(p2[:], w[f"f1{pfx}1{j}"][:],
                                 xx[:], start=False, stop=True)
                t = wk.tile([P, G], F32, tag=f"m1{pfx}{j}")
                nc.scalar.activation(t[:], p2[:], AF.Relu,
                                     bias=w[f"b1{pfx}{j}"][:, 0:1])
                m1.append(t)
            p3 = ps([P, G])
            nc.tensor.matmul(p3[:], w[f"f2{pfx}0"][:],
                             m1[0][:], start=True, stop=False)
            nc.tensor.matmul(p3[:], w[f"f2{pfx}1"][:],
                             m1[1][:], start=False, stop=True)
            mT = wk.tile([P, G], F32, tag=f"mT{pfx}")
            nc.scalar.activation(mT[:], p3[:], AF.Relu,
                                 bias=w[f"b2{pfx}"][:, 0:1])
            p4 = ps([10, G])
            nc.tensor.matmul(p4[:], w[f"cls{pfx}W"][:],
                             mT[:])
            o = wk.tile([10, G], F32, tag=f"o{pfx}")
            nc.vector.tensor_scalar(o[:], p4[:], w[f"cls{pfx}b"][:, 0:1],
                                    None, op0=OP.add)
            return mT, o

        mT, om = head("m", x1m, x1x)
        _, ov = head("v", x1m, x1x)

        # ---------------- outputs ----------------
        p5 = ps([G, P])
        nc.tensor.transpose(p5[:], mT[:], ident32[:])
        m_t = wk.tile([G, P], F32, tag="q2")
        nc.vector.tensor_copy(m_t[:], p5[:])
        nc.sync.dma_start(out_d.ap()[:, 20:148], m_t[:])
        for o_ap, cols in ((om, slice(0, 10)), (ov, slice(10, 20))):
            p6 = ps([G, 10])
            nc.tensor.transpose(p6[:], o_ap[:], ident32[0:10, 0:10])
            o_t = wk.tile([G, 10], F32, tag="o_t")
            nc.vector.tensor_copy(o_t[:], p6[:])
            nc.sync.dma_start(out_d.ap()[:, cols], o_t[:])

    nc.compile()
    return nc


def prepare_shared(inputs):
    f32 = np.float32
    att = np.asarray(inputs["att_inner"], f32)
    atto = np.asarray(inputs["att_outer"], f32)
    sh = {
        "watt2": np.ascontiguousarray(np.stack([att[:P], att[P:]], axis=1)),
        "winner": np.ascontiguousarray(np.asarray(inputs["W_inner"], f32)),
        "binner": np.asarray(inputs["b_inner"], f32).reshape(P, 1).copy(),
        "wattm": np.ascontiguousarray(
            np.stack([atto[0:128], atto[256:384]], axis=1)),
        "wattx": np.ascontiguousarray(
            np.stack([atto[128:256], atto[384:512]], axis=1)),
        "wout1a": np.ascontiguousarray(np.asarray(inputs["W_out1"], f32)[:P]),
        "wout1b": np.ascontiguousarray(np.asarray(inputs["W_out1"], f32)[P:]),
        "bout1": np.asarray(inputs["b_out1"], f32).reshape(P, 1).copy(),
        "wout2": np.ascontiguousarray(np.asarray(inputs["W_out2"], f32)),
        "bout2": np.asarray(inputs["b_out2"], f32).reshape(P, 1).copy(),
        "clsmW": np.ascontiguousarray(np.asarray(inputs["clsm_W"], f32)),
        "clsmb": np.asarray(inputs["clsm_b"], f32).reshape(10, 1).copy(),
        "clsvW": np.ascontiguousarray(np.asarray(inputs["clsv_W"], f32)),
        "clsvb": np.asarray(inputs["clsv_b"], f32).reshape(10, 1).copy(),
    }
    for pfx in ("m", "v"):
        w1 = np.asarray(inputs[f"fc1{pfx}_W"], f32)
        b1 = np.asarray(inputs[f"fc1{pfx}_b"], f32)
        w2 = np.asarray(inputs[f"fc2{pfx}_W"], f32)
        for j in range(2):
            sh[f"f1{pfx}0{j}"] = np.ascontiguousarray(
                w1[0:P, j * P:(j + 1) * P])
            sh[f"f1{pfx}1{j}"] = np.ascontiguousarray(
                w1[P:2 * P, j * P:(j + 1) * P])
            sh[f"b1{pfx}{j}"] = b1[j * P:(j + 1) * P].reshape(P, 1).copy()
        sh[f"f2{pfx}0"] = np.ascontiguousarray(w2[0:P])
        sh[f"f2{pfx}1"] = np.ascontiguousarray(w2[P:2 * P])
        sh[f"b2{pfx}"] = np.asarray(
            inputs[f"fc2{pfx}_b"], f32).reshape(P, 1).copy()
    return sh


def make_in_maps(inputs):
    x = np.asarray(inputs["x"], np.float32)
    col = np.asarray(inputs["edge_index"], np.int32)[1]
    sh = prepare_shared(inputs)
    in_maps = []
    for c in range(NCORES):
        xT = np.ascontiguousarray(x[c * NN:(c + 1) * NN].T)
        cs = col[c * NN * K:(c + 1) * NN * K].reshape(G, P, K)
        colr = np.ascontiguousarray(
            cs.transpose(1, 0, 2).reshape(P, G * K)).astype(np.int32)
        in_maps.append({"xT": xT, "colr": colr, **sh})
    return in_maps


_NC = None


def _ensure_ntff_hook():
    """Register the axon NTFF profiling hook if the image's antenv lacks
    the axon_hooks module (needed for trace=True exec-time capture)."""
    import sys, types
    try:
        from antenv.axon_hooks import get_axon_ntff_profile_hook  # noqa
        return
    except ImportError:
        pass
    try:
        import antenv
        from trn_agent_boot.trn_boot import _ntff_profile_via_ctypes
        mod = types.ModuleType("antenv.axon_hooks")
        hook = _ntff_profile_via_ctypes("/opt/axon/libaxon_pjrt.so")
        mod._hook = hook
        mod.set_axon_ntff_profile_hook = lambda h: setattr(mod, "_hook", h)
        mod.get_axon_ntff_profile_hook = lambda: mod._hook
        sys.modules["antenv.axon_hooks"] = mod
        antenv.axon_hooks = mod
    except Exception as e:  # pragma: no cover
        print(f"ntff hook setup failed: {e}")


def kernel(**inputs):
    global _NC
    if _NC is None:
        _NC = build_nc()
    in_maps = make_in_maps(inputs)
    trace = os.environ.get("BHGNN_TRACE", "") not in ("", "0")
    if trace:
        _ensure_ntff_hook()
    res = run_bass_kernel_spmd(_NC, in_maps, core_ids=list(range(NCORES)),
                               trace=trace)
    full = np.concatenate([res.results[c]["out"] for c in range(NCORES)],
                          axis=0)
    if trace and res.exec_time_ns is not None:
        print(f"HW exec time: {res.exec_time_ns} ns")
    return (np.ascontiguousarray(full[:, 0:10]),
            np.ascontiguousarray(full[:, 10:20]),
            np.ascontiguousarray(full[:, 20:148]))


# revision 18
# speedup vs baseline: 2.0158x; 2.0158x over previous
"""BHGNN Trainium2 kernel (8 NeuronCores, graph-level data parallel).

Per core: 64 graphs x 128 nodes. The sparsemax attention is computed
densely, without any per-edge gather:
  - al[n], ar[n], XW from per-graph PE matmuls on fp16 x^T.
  - Duplicate edge columns are merged by a 16-channel Batcher sort on the
    column ids; GPSIMD local_scatter builds the dense count matrix
    cnt[n, m] per graph.
  - zd1[n, m] = max(ar[m] + al[n] + 1, 1) densely (PE broadcast of ar).
  - Sparsemax threshold via tau+1 = max_m (S[n,m]-1)/N[n,m], where
    S = (cnt*zd1) @ C, N = cnt @ C and C[m',m] = [ar(m') >= ar(m)] is a
    graph-global comparison matrix -> two per-graph PE matmuls.
  - A = cnt * max(zd1 - tau1, max(1-tau1, 0)); agg = (A + I) @ XW via PE
    (identity accumulated in PSUM); h = relu(0.5*agg + b).
  - Pooling (strided DVE reduces) -> outer softmax attention, two tiny
    GCNs, readout, MLP heads in f32.
"""
import os
from contextlib import ExitStack

import numpy as np

import concourse.bass as bass
import concourse.mybir as mybir
import concourse.bacc as bacc
import concourse.tile as tile
from concourse.bass_utils import run_bass_kernel_spmd

FP16 = mybir.dt.float16
F32 = mybir.dt.float32
I32 = mybir.dt.int32
I16 = mybir.dt.int16
AF = mybir.ActivationFunctionType
OP = mybir.AluOpType
X = mybir.AxisListType.X

P = 128          # partitions = nodes per graph
G = 64           # graphs per core
NN = P * G       # nodes per core (8192)
K = 16           # edges per node
NH = 128
H = 6
NCORES = 8
NHYP = G * H     # hypernodes per core (384)

WNAMES = [
    ("watt2", [P, 2]), ("winner", [P, P]), ("binner", [P, 1]),
    ("wattm", [P, 2]), ("wattx", [P, 2]),
    ("wout1a", [P, P]), ("wout1b", [P, P]), ("bout1", [P, 1]),
    ("wout2", [P, P]), ("bout2", [P, 1]),
    ("f1m00", [P, P]), ("f1m10", [P, P]), ("f1m01", [P, P]), ("f1m11", [P, P]),
    ("b1m0", [P, 1]), ("b1m1", [P, 1]),
    ("f2m0", [P, P]), ("f2m1", [P, P]), ("b2m", [P, 1]),
    ("f1v00", [P, P]), ("f1v10", [P, P]), ("f1v01", [P, P]), ("f1v11", [P, P]),
    ("b1v0", [P, 1]), ("b1v1", [P, 1]),
    ("f2v0", [P, P]), ("f2v1", [P, P]), ("b2v", [P, 1]),
    ("clsmW", [P, 10]), ("clsmb", [10, 1]),
    ("clsvW", [P, 10]), ("clsvb", [10, 1]),
]

# Batcher odd-even merge sort network for 16 channels. Each layer:
# (channel-dim factors, lo index, hi index).
SORT_LAYERS = [
    ((8, 2), (slice(None), slice(0, 1)), (slice(None), slice(1, 2))),
    ((4, 4), (slice(None), slice(0, 2)), (slice(None), slice(2, 4))),
    ((4, 4), (slice(None), slice(1, 2)), (slice(None), slice(2, 3))),
    ((2, 8), (slice(None), slice(0, 4)), (slice(None), slice(4, 8))),
    ((2, 8), (slice(None), slice(2, 4)), (slice(None), slice(4, 6))),
    ((2, 4, 2), (slice(None), slice(0, 3), slice(1, 2)),
     (slice(None), slice(1, 4), slice(0, 1))),
    ((1, 16), (slice(None), slice(0, 8)), (slice(None), slice(8, 16))),
    ((1, 16), (slice(None), slice(4, 8)), (slice(None), slice(8, 12))),
    ((4, 4), (slice(0, 3), slice(2, 4)), (slice(1, 4), slice(0, 2))),
    ((8, 2), (slice(0, 7), slice(1, 2)), (slice(1, 8), slice(0, 1))),
]


def _chslice(ap, factors, idx):
    names = "abcd"[: len(factors)]
    pat = f"p ({' '.join(names)} g) -> p {' '.join(names)} g"
    v = ap.rearrange(pat, **{n: f for n, f in zip(names, factors)})
    return v[(slice(None),) + idx + (slice(None),)]


def build_nc():
    nc = bacc.Bacc("TRN2", target_bir_lowering=False, debug=False,
                   num_devices=NCORES)
    xT_d = nc.declare_dram_parameter("xT", [P, NN], F32, isOutput=False)
    col_d = nc.declare_dram_parameter("colr", [P, G * K], I32, isOutput=False)
    wd = {}
    for nm, shp in WNAMES:
        wd[nm] = nc.declare_dram_parameter(nm, shp, F32, isOutput=False)
    out_d = nc.declare_dram_parameter("out", [G, 148], F32, isOutput=True)

    with tile.TileContext(nc) as tc, ExitStack() as ctx:
        pp = ctx.enter_context(tc.tile_pool(name="persist", bufs=1))
        wk = ctx.enter_context(tc.tile_pool(name="work", bufs=1))
        pmm = ctx.enter_context(
            tc.tile_pool(name="psum", bufs=6, space="PSUM"))

        ps_ctr = [0]

        def ps(shape, dtype=F32):
            ps_ctr[0] += 1
            return pmm.tile(shape, dtype, tag="ps", name=f"pst{ps_ctr[0]}")

        # ---------------- weights / constants ----------------
        w = {}
        for nm, shp in WNAMES:
            t = pp.tile(shp, F32, tag=f"w_{nm}")
            nc.sync.dma_start(t[:], wd[nm].ap())
            w[nm] = t
        watt2_h = pp.tile([P, 2], FP16, tag="watt2h")
        nc.gpsimd.dma_start(watt2_h[:], wd["watt2"].ap())
        winner_h = pp.tile([P, P], FP16, tag="winnerh")
        nc.gpsimd.dma_start(winner_h[:], wd["winner"].ap())

        iota_row = pp.tile([P, P], I32, tag="iota_row")
        nc.gpsimd.iota(iota_row[:], pattern=[[1, P]], channel_multiplier=0)
        iota_part = pp.tile([P, 1], I32, tag="iota_part")
        nc.gpsimd.iota(iota_part[:], pattern=[[0, 1]], channel_multiplier=1)
        g128rep = pp.tile([P, K * G], I32, tag="g128rep")   # (k,g): 128*(g%8)
        nc.gpsimd.iota(g128rep[:], pattern=[[0, K], [0, 8], [P, 8]],
                       channel_multiplier=0)

        iota_rowf = pp.tile([P, P], F32, tag="iota_rowf")
        nc.vector.tensor_copy(iota_rowf[:], iota_row[:])
        iota_partf = pp.tile([P, 1], F32, tag="iota_partf")
        nc.vector.tensor_copy(iota_partf[:], iota_part[:])
        ident32 = pp.tile([P, P], F32, tag="ident32")
        nc.vector.tensor_scalar(ident32[:], iota_rowf[:], iota_partf[:, 0:1],
                                None, op0=OP.is_equal)
        ident16 = pp.tile([P, P], FP16, tag="ident16")
        nc.vector.tensor_copy(ident16[:], ident32[:])
        ones_col = pp.tile([1, P], F32, tag="ones_col")
        nc.vector.memset(ones_col[:], 1.0)
        ones16 = pp.tile([1, P], FP16, tag="ones16")
        nc.vector.memset(ones16[:], 1.0)
        cntinv = pp.tile([P, H * G], F32, tag="cntinv")     # (g, r)
        nc.vector.memset(cntinv[:], 1.0 / 21.0)
        nc.vector.memset(
            cntinv.rearrange("p (g r) -> p g r", g=G)[:, :, 0:2],
            1.0 / 22.0)

        # ---------------- x^T (fp16 cast) and col ----------------
        xT = pp.tile([P, NN], FP16, tag="xT")
        for c in range(8):
            sl = slice(c * 1024, (c + 1) * 1024)
            nc.gpsimd.dma_start(xT[:, sl], xT_d.ap()[:, sl])
        colr = pp.tile([P, G * K], I32, tag="colr")
        nc.sync.dma_start(colr[:], col_d.ap())

        # ---------------- stage A: al, ar, XW per graph ----------------
        al_sb = wk.tile([P, G], F32, tag="al")
        arc_sb = wk.tile([P, G], F32, tag="arc")
        xw16 = wk.tile([P, G * P], FP16, tag="xw16")
        for b in range(G // 4):
            ps_xw = ps([P, 512])
            ps_al = ps([P, 8])
            for i in range(4):
                g = b * 4 + i
                lhs = xT[:, g * P:(g + 1) * P]
                nc.tensor.matmul(ps_al[:, 2 * i:2 * i + 2], lhs, watt2_h[:])
                nc.tensor.matmul(ps_xw[:, i * P:(i + 1) * P], lhs,
                                 winner_h[:])
            alr = ps_al.rearrange("p (i t) -> p i t", i=4)
            nc.vector.tensor_copy(
                al_sb.rearrange("p (b i) -> p b i", b=G // 4)[:, b],
                alr[:, :, 0])
            nc.vector.tensor_copy(
                arc_sb.rearrange("p (b i) -> p b i", b=G // 4)[:, b],
                alr[:, :, 1])
            nc.scalar.copy(xw16[:, b * 512:(b + 1) * 512], ps_xw[:])

        # ar as fp16-rounded per-node f32 scalars (consistent with ar_all)
        arc16 = wk.tile([P, G], FP16, tag="arc16")
        nc.vector.tensor_copy(arc16[:], arc_sb[:])
        arcr = wk.tile([P, G], F32, tag="arcr")
        nc.vector.tensor_copy(arcr[:], arc16[:])
        al1h = wk.tile([P, G], FP16, tag="al1h")
        nc.vector.tensor_scalar_add(al1h[:], al_sb[:], 1.0)

        # ar replicated on all partitions: [p, (g*128+m)] fp16
        ar_flat = wk.tile([1, NN], FP16, tag="arflat")
        for c in range(NN // 512):
            ps_ar = ps([1, 512])
            nc.tensor.matmul(ps_ar[:], watt2_h[:, 1:2],
                             xT[:, c * 512:(c + 1) * 512])
            nc.vector.tensor_copy(ar_flat[:, c * 512:(c + 1) * 512],
                                  ps_ar[:])
        ar_all = wk.tile([P, NN], FP16, tag="arall")
        for c in range(NN // 512):
            ps_b = ps([P, 512])
            nc.tensor.matmul(ps_b[:], ones16[:],
                             ar_flat[:, c * 512:(c + 1) * 512])
            nc.scalar.copy(ar_all[:, c * 512:(c + 1) * 512], ps_b[:])

        # ---------------- stage C: dedup cols -> cnt matrix ----------------
        colf = wk.tile([P, K * G], F32, tag="colf")         # (k, g)
        colt = wk.tile([P, K * G], I32, tag="colt")
        nc.vector.tensor_copy(colt[:],
                              colr.rearrange("p (g j) -> p j g", g=G))
        nc.vector.tensor_scalar(colt[:], colt[:], 127, None,
                                op0=OP.bitwise_and)
        nc.vector.tensor_copy(colf[:], colt[:])
        sorttmp = wk.tile([P, 8 * G], F32, tag="sorttmp")
        for factors, lo_i, hi_i in SORT_LAYERS:
            lo = _chslice(colf, factors, lo_i)
            hi = _chslice(colf, factors, hi_i)
            ext = tuple(len(range(*s.indices(f)))
                        for s, f in zip(lo_i, factors))
            npair = int(np.prod(ext))
            tmp = _chslice(sorttmp[:, 0:npair * G], ext,
                           tuple(slice(None) for _ in ext))
            nc.vector.tensor_tensor(tmp, lo, hi, op=OP.min)
            nc.vector.tensor_tensor(lo, lo, hi, op=OP.max)
            nc.vector.tensor_copy(hi, tmp)
        # eq channels with zero pad; dead_k = eq_{k+1}
        eqt = wk.tile([P, (K + 1) * G], F32, tag="eqt")
        nc.vector.memset(eqt[:, 0:G], 0.0)
        nc.vector.memset(eqt[:, K * G:], 0.0)
        nc.vector.tensor_tensor(eqt[:, G:K * G], colf[:, G:],
                                colf[:, 0:(K - 1) * G], op=OP.is_equal)
        # run counts: c_k = 1 + eq_k * c_{k-1}; last slot of run holds total
        cntc = wk.tile([P, K * G], F32, tag="cntc")
        nc.vector.memset(cntc[:, 0:G], 1.0)
        for k in range(1, K):
            nc.vector.tensor_mul(cntc[:, k * G:(k + 1) * G],
                                 eqt[:, k * G:(k + 1) * G],
                                 cntc[:, (k - 1) * G:k * G])
            nc.vector.tensor_scalar_add(cntc[:, k * G:(k + 1) * G],
                                        cntc[:, k * G:(k + 1) * G], 1.0)
        deadt = eqt[:, G:]
        ndead = wk.tile([P, K * G], F32, tag="ndead")
        nc.vector.tensor_scalar(ndead[:], deadt, -1.0, 1.0,
                                op0=OP.mult, op1=OP.add)
        rampf = wk.tile([P, K * G], F32, tag="rampf")
        nc.vector.tensor_copy(rampf[:], g128rep[:])
        # idx = dead ? -1 : col + 128*(g%8)  == (col+ramp+1)*ndead - 1
        idxf = wk.tile([P, K * G], F32, tag="idxf")
        nc.vector.tensor_add(idxf[:], colf[:], rampf[:])
        nc.vector.scalar_tensor_tensor(idxf[:], idxf[:], 1.0, ndead[:],
                                       op0=OP.add, op1=OP.mult)
        nc.vector.tensor_scalar_add(idxf[:], idxf[:], -1.0)
        sidx = wk.tile([P, G * K], I16, tag="sidx")
        sval = wk.tile([P, G * K], FP16, tag="sval")
        nc.vector.tensor_copy(sidx.rearrange("p (g k) -> p k g", g=G),
                              idxf.rearrange("p (k g) -> p k g", k=K))
        nc.vector.tensor_copy(sval.rearrange("p (g k) -> p k g", g=G),
                              cntc.rearrange("p (k g) -> p k g", k=K))
        cntd = wk.tile([P, G * P], FP16, tag="cntd")
        for wnd in range(8):
            nc.gpsimd.local_scatter(
                cntd[:, wnd * 1024:(wnd + 1) * 1024],
                sval[:, wnd * 128:(wnd + 1) * 128],
                sidx[:, wnd * 128:(wnd + 1) * 128],
                channels=P, num_elems=1024, num_idxs=128)

        # ---------------- stage D: zd1, S/N matmuls, tau ----------------
        zd1 = wk.tile([P, NN], FP16, tag="zd1")
        zd3 = zd1.rearrange("p (g m) -> p g m", g=G)
        al1b = al1h.rearrange("p (g o) -> p g o", o=1) \
            .broadcast_to([P, G, P])
        nc.vector.tensor_tensor(zd3, ar_all.rearrange("p (g m) -> p g m",
                                                      g=G), al1b, op=OP.add)
        nc.vector.tensor_scalar_max(zd1[:], zd1[:], 1.0)

        tau = wk.tile([P, G], F32, tag="tau")
        for hf in range(2):
            S_h = wk.tile([P, NN // 2], FP16, tag="Sh")
            N_h = wk.tile([P, NN // 2], FP16, tag="Nh")
            for b in range(8):
                gb = hf * 8 + b
                czb = wk.tile([P, 512], FP16, tag="czb", bufs=2)
                nc.vector.tensor_mul(czb[:],
                                     cntd[:, gb * 512:(gb + 1) * 512],
                                     zd1[:, gb * 512:(gb + 1) * 512])
                ps_t1 = ps([P, 512], FP16)
                ps_t2 = ps([P, 512], FP16)
                for i in range(4):
                    g = gb * 4 + i
                    nc.tensor.transpose(ps_t1[:, i * P:(i + 1) * P],
                                        cntd[:, g * P:(g + 1) * P],
                                        ident16[:])
                    nc.tensor.transpose(ps_t2[:, i * P:(i + 1) * P],
                                        czb[:, i * P:(i + 1) * P],
                                        ident16[:])
                ctb = wk.tile([P, 512], FP16, tag="ctb", bufs=2)
                cztb = wk.tile([P, 512], FP16, tag="cztb", bufs=2)
                nc.scalar.copy(ctb[:], ps_t1[:])
                nc.scalar.copy(cztb[:], ps_t2[:])
                ps_s = ps([P, 512])
                ps_n = ps([P, 512])
                for i in range(4):
                    g = gb * 4 + i
                    Cg = wk.tile([P, P], FP16, tag="Cg", bufs=8)
                    nc.vector.tensor_scalar(
                        Cg[:], ar_all[:, g * P:(g + 1) * P],
                        arcr[:, g:g + 1], None, op0=OP.is_le)
                    nc.tensor.matmul(ps_s[:, i * P:(i + 1) * P],
                                     cztb[:, i * P:(i + 1) * P], Cg[:])
                    nc.tensor.matmul(ps_n[:, i * P:(i + 1) * P],
                                     ctb[:, i * P:(i + 1) * P], Cg[:])
                nc.scalar.copy(S_h[:, b * 512:(b + 1) * 512], ps_s[:])
                nc.scalar.copy(N_h[:, b * 512:(b + 1) * 512], ps_n[:])
            # tau over this half: max_m (S-1)/(N+eps)
            for q in range(2):
                rq = wk.tile([P, 2048], F32, tag="rq")
                tq = wk.tile([P, 2048], F32, tag="tq")
                sl = slice(q * 2048, (q + 1) * 2048)
                nc.vector.tensor_scalar_add(rq[:], N_h[:, sl], 1e-6)
                nc.vector.reciprocal(rq[:], rq[:])
                nc.vector.scalar_tensor_tensor(tq[:], S_h[:, sl], -1.0,
                                               rq[:], op0=OP.add,
                                               op1=OP.mult)
                nc.vector.tensor_reduce(
                    tau[:, hf * 32 + q * 16:hf * 32 + (q + 1) * 16],
                    tq.rearrange("p (g m) -> p g m", g=16), axis=X,
                    op=OP.max)

        # ---------------- stage E: A dense, transpose, agg ----------------
        ntau16 = wk.tile([P, G], FP16, tag="ntau16")
        nc.vector.tensor_scalar_mul(ntau16[:], tau[:], -1.0)
        gam16 = wk.tile([P, G], FP16, tag="gam16")
        nc.vector.tensor_scalar(gam16[:], tau[:], -1.0, 1.0,
                                op0=OP.mult, op1=OP.add)
        nc.vector.tensor_scalar_max(gam16[:], gam16[:], 0.0)
        A = wk.tile([P, G * P], FP16, tag="arall")
        A3 = A.rearrange("p (g m) -> p g m", g=G)
        ntb = ntau16.rearrange("p (g o) -> p g o", o=1) \
            .broadcast_to([P, G, P])
        gmb = gam16.rearrange("p (g o) -> p g o", o=1) \
            .broadcast_to([P, G, P])
        nc.vector.tensor_tensor(A3, zd3, ntb, op=OP.add)
        nc.vector.tensor_tensor(A3, A3, gmb, op=OP.max)
        nc.vector.tensor_mul(A[:], A[:], cntd[:])

        hT = wk.tile([P, G * P], FP16, tag="cntd")
        for b in range(G // 4):
            ps_t = ps([P, 512], FP16)
            for i in range(4):
                g = b * 4 + i
                nc.tensor.transpose(ps_t[:, i * P:(i + 1) * P],
                                    A[:, g * P:(g + 1) * P], ident16[:])
            atb = wk.tile([P, 512], FP16, tag="atb", bufs=2)
            nc.scalar.copy(atb[:], ps_t[:])
            ps_a = ps([P, 512])
            for i in range(4):
                g = b * 4 + i
                nc.tensor.matmul(ps_a[:, i * P:(i + 1) * P],
                                 xw16[:, g * P:(g + 1) * P],
                                 atb[:, i * P:(i + 1) * P],
                                 start=True, stop=False)
                nc.tensor.matmul(ps_a[:, i * P:(i + 1) * P],
                                 xw16[:, g * P:(g + 1) * P],
                                 ident16[:], start=False, stop=True)
            nc.scalar.activation(hT[:, b * 512:(b + 1) * 512], ps_a[:],
                                 AF.Relu, bias=w["binner"][:, 0:1],
                                 scale=0.5)

        # ---------------- stage F: pooling ----------------
        xhm = wk.tile([P, NHYP], F32, tag="xw16")           # (g, r) mean
        xhx = wk.tile([P, NHYP], F32, tag="colf")           # (g, r) max
        hT_v = hT.rearrange("p (g n) -> p g n", g=G)
        hT_seg = hT_v[:, :, 0:126].rearrange("p g (kk r) -> p g r kk", r=H)
        xhm_v = xhm.rearrange("p (g r) -> p g r", g=G)
        xhx_v = xhx.rearrange("p (g r) -> p g r", g=G)
        nc.vector.tensor_reduce(xhm_v, hT_seg, axis=X, op=OP.add)
        nc.vector.tensor_tensor(xhm_v[:, :, 0:2], xhm_v[:, :, 0:2],
                                hT_v[:, :, 126:128], op=OP.add)
        nc.vector.tensor_mul(xhm[:], xhm[:], cntinv[:])
        nc.vector.tensor_reduce(xhx_v, hT_seg, axis=X, op=OP.max)
        nc.vector.tensor_tensor(xhx_v[:, :, 0:2], xhx_v[:, :, 0:2],
                                hT_v[:, :, 126:128], op=OP.max)

        # ---------------- stage G: outer attention ----------------
        ps_w = ps([2, NHYP])
        nc.tensor.matmul(ps_w[:], w["wattm"][:], xhm[:], start=True,
                         stop=False)
        nc.tensor.matmul(ps_w[:], w["wattx"][:], xhx[:], start=False,
                         stop=True)
        wlr2 = wk.tile([2, NHYP], F32, tag="wlr2")
        nc.vector.tensor_copy(wlr2[:], ps_w[:])
        wlr = wk.tile([G, 12], F32, tag="wlr")
        nc.sync.dma_start(wlr[:, 0:6], wlr2[0:1, :])
        nc.sync.dma_start(wlr[:, 6:12], wlr2[1:2, :])
        whm = wk.tile([G, 36], F32, tag="whm")
        whm_v = whm.rearrange("g (r s) -> g r s", r=H)
        for r in range(H):
            nc.vector.tensor_scalar(whm_v[:, r], wlr[:, 6:12],
                                    wlr[:, r:r + 1], None, op0=OP.add)
        wt = wk.tile([G, 36], F32, tag="wt36")
        wt_v = wt.rearrange("g (r s) -> g r s", r=H)
        nc.vector.tensor_scalar_min(wt[:], whm[:], 0.0)
        nc.vector.tensor_scalar_max(whm[:], whm[:], 0.0)
        nc.vector.scalar_tensor_tensor(whm[:], wt[:], 0.2, whm[:],
                                       op0=OP.mult, op1=OP.add)
        rmax = wk.tile([G, H], F32, tag="rmax")
        nc.vector.tensor_tensor(wt_v[:, :, 0:3], whm_v[:, :, 0:3],
                                whm_v[:, :, 3:6], op=OP.max)
        nc.vector.tensor_tensor(rmax[:], wt_v[:, :, 0], wt_v[:, :, 1],
                                op=OP.max)
        nc.vector.tensor_tensor(rmax[:], rmax[:], wt_v[:, :, 2], op=OP.max)
        for r in range(H):
            nc.vector.tensor_scalar(whm_v[:, r], whm_v[:, r],
                                    rmax[:, r:r + 1], None, op0=OP.subtract)
        nc.scalar.activation(whm[:], whm[:], AF.Exp)
        rsum = wk.tile([G, H], F32, tag="rsum")
        nc.vector.tensor_tensor(wt_v[:, :, 0:3], whm_v[:, :, 0:3],
                                whm_v[:, :, 3:6], op=OP.add)
        nc.vector.tensor_tensor(rsum[:], wt_v[:, :, 0], wt_v[:, :, 1],
                                op=OP.add)
        nc.vector.tensor_tensor(rsum[:], rsum[:], wt_v[:, :, 2], op=OP.add)
        nc.vector.reciprocal(rsum[:], rsum[:])
        for r in range(H):
            nc.vector.tensor_scalar(whm_v[:, r], whm_v[:, r],
                                    rsum[:, r:r + 1], None, op0=OP.mult)
        ahflat = wk.tile([1, G * 36], F32, tag="arflat")
        nc.sync.dma_start(ahflat[:], whm[:])
        ahrep = wk.tile([P, G * 36], F32, tag="zd1")
        for c in range(5):
            lo = c * 512
            n = min(512, G * 36 - lo)
            ps_b2 = ps([P, 512])
            nc.tensor.matmul(ps_b2[:, 0:n], ones_col[:], ahflat[:, lo:lo + n])
            nc.scalar.copy(ahrep[:, lo:lo + n], ps_b2[:, 0:n])
        ah_v = ahrep.rearrange("p (g q) -> p g q", g=G)

        def outer_gcn(xin_m, xin_x, wa, wb, bias, name):
            p1 = ps([P, NHYP])
            if xin_x is None:
                nc.tensor.matmul(p1[:], wa[:], xin_m[:])
            else:
                nc.tensor.matmul(p1[:], wa[:], xin_m[:], start=True,
                                 stop=False)
                nc.tensor.matmul(p1[:], wb[:], xin_x[:], start=False,
                                 stop=True)
            xwT = wk.tile([P, NHYP], F32, tag="xwT")
            nc.vector.tensor_copy(xwT[:], p1[:])
            agg = wk.tile([P, NHYP], F32, tag="agg")
            agg_v = agg.rearrange("p (g r) -> p g r", g=G)
            xw_v = xwT.rearrange("p (g s) -> p g s", g=G)
            tmpa = wk.tile([P, G], F32, tag="tmpa")
            for r in range(H):
                for s in range(H):
                    if s == 0:
                        nc.vector.tensor_mul(agg_v[:, :, r], xw_v[:, :, s],
                                             ah_v[:, :, r * H + s])
                    else:
                        nc.vector.tensor_mul(tmpa[:], xw_v[:, :, s],
                                             ah_v[:, :, r * H + s])
                        nc.vector.tensor_tensor(agg_v[:, :, r],
                                                agg_v[:, :, r], tmpa[:],
                                                op=OP.add)
            nc.vector.tensor_add(agg[:], agg[:], xwT[:])
            zT = wk.tile([P, NHYP], F32, tag="zT")
            nc.scalar.activation(zT[:], agg[:], AF.Relu, bias=bias[:, 0:1],
                                 scale=0.5)
            return zT

        z1h = outer_gcn(xhm, xhx, w["wout1a"], w["wout1b"], w["bout1"], "o1")
        z2h = outer_gcn(z1h, None, w["wout2"], None, w["bout2"], "o2")

        x1m = wk.tile([P, G], F32, tag="x1m")
        x1x = wk.tile([P, G], F32, tag="x1x")
        z2_v = z2h.rearrange("p (g r) -> p g r", g=G)
        nc.vector.tensor_reduce(x1m[:], z2_v, axis=X, op=OP.add)
        nc.vector.tensor_scalar_mul(x1m[:], x1m[:], 1.0 / H)
        nc.vector.tensor_reduce(x1x[:], z2_v, axis=X, op=OP.max)

        # ---------------- MLP heads ----------------
        def head(pfx, xm, xx):
            m1 = []
            for j in range(2):
                p2 = ps([P, G])
                nc.tensor.matmul(p2[:], w[f"f1{pfx}0{j}"][:], xm[:],
                                 start=True, stop=False)
                nc.tensor.matmul(p2[:], w[f"f1{pfx}1{j}"][:], xx[:],
                                 start=False, stop=True)
                t = wk.tile([P, G], F32, tag=f"m1{pfx}{j}")
                nc.scalar.activation(t[:], p2[:], AF.Relu,
                                     bias=w[f"b1{pfx}{j}"][:, 0:1])
                m1.append(t)
            p3 = ps([P, G])
            nc.tensor.matmul(p3[:], w[f"f2{pfx}0"][:], m1[0][:],
                             start=True, stop=False)
            nc.tensor.matmul(p3[:], w[f"f2{pfx}1"][:], m1[1][:],
                             start=False, stop=True)
            mT = wk.tile([P, G], F32, tag=f"mT{pfx}")
            nc.scalar.activation(mT[:], p3[:], AF.Relu,
                                 bias=w[f"b2{pfx}"][:, 0:1])
            p4 = ps([10, G])
            nc.tensor.matmul(p4[:], w[f"cls{pfx}W"][:], mT[:])
            o = wk.tile([10, G], F32, tag=f"o{pfx}")
            nc.vector.tensor_scalar(o[:], p4[:], w[f"cls{pfx}b"][:, 0:1],
                                    None, op0=OP.add)
            return mT, o

        mT, om = head("m", x1m, x1x)
        _, ov = head("v", x1m, x1x)

        # ---------------- outputs ----------------
        p5 = ps([G, P])
        nc.tensor.transpose(p5[:], mT[:], ident32[:])
        m_t = wk.tile([G, P], F32, tag="m_t")
        nc.vector.tensor_copy(m_t[:], p5[:])
        nc.sync.dma_start(out_d.ap()[:, 20:148], m_t[:])
        for o_ap, cols in ((om, slice(0, 10)), (ov, slice(10, 20))):
            p6 = ps([G, 10])
            nc.tensor.transpose(p6[:], o_ap[:], ident32[0:10, 0:10])
            o_t = wk.tile([G, 10], F32, tag="o_t")
            nc.vector.tensor_copy(o_t[:], p6[:])
            nc.sync.dma_start(out_d.ap()[:, cols], o_t[:])

    nc.compile()
    return nc


def prepare_shared(inputs):
    f32 = np.float32
    att = np.asarray(inputs["att_inner"], f32)
    atto = np.asarray(inputs["att_outer"], f32)
    sh = {
        "watt2": np.ascontiguousarray(np.stack([att[:P], att[P:]], axis=1)),
        "winner": np.ascontiguousarray(np.asarray(inputs["W_inner"], f32)),
        "binner": np.asarray(inputs["b_inner"], f32).reshape(P, 1).copy(),
        "wattm": np.ascontiguousarray(
            np.stack([atto[0:128], atto[256:384]], axis=1)),
        "wattx": np.ascontiguousarray(
            np.stack([atto[128:256], atto[384:512]], axis=1)),
        "wout1a": np.ascontiguousarray(np.asarray(inputs["W_out1"], f32)[:P]),
        "wout1b": np.ascontiguousarray(np.asarray(inputs["W_out1"], f32)[P:]),
        "bout1": np.asarray(inputs["b_out1"], f32).reshape(P, 1).copy(),
        "wout2": np.ascontiguousarray(np.asarray(inputs["W_out2"], f32)),
        "bout2": np.asarray(inputs["b_out2"], f32).reshape(P, 1).copy(),
        "clsmW": np.ascontiguousarray(np.asarray(inputs["clsm_W"], f32)),
        "clsmb": np.asarray(inputs["clsm_b"], f32).reshape(10, 1).copy(),
        "clsvW": np.ascontiguousarray(np.asarray(inputs["clsv_W"], f32)),
        "clsvb": np.asarray(inputs["clsv_b"], f32).reshape(10, 1).copy(),
    }
    for pfx in ("m", "v"):
        w1 = np.asarray(inputs[f"fc1{pfx}_W"], f32)
        b1 = np.asarray(inputs[f"fc1{pfx}_b"], f32)
        w2 = np.asarray(inputs[f"fc2{pfx}_W"], f32)
        for j in range(2):
            sh[f"f1{pfx}0{j}"] = np.ascontiguousarray(
                w1[0:P, j * P:(j + 1) * P])
            sh[f"f1{pfx}1{j}"] = np.ascontiguousarray(
                w1[P:2 * P, j * P:(j + 1) * P])
            sh[f"b1{pfx}{j}"] = b1[j * P:(j + 1) * P].reshape(P, 1).copy()
        sh[f"f2{pfx}0"] = np.ascontiguousarray(w2[0:P])
        sh[f"f2{pfx}1"] = np.ascontiguousarray(w2[P:2 * P])
        sh[f"b2{pfx}"] = np.asarray(
            inputs[f"fc2{pfx}_b"], f32).reshape(P, 1).copy()
    return sh


def make_in_maps(inputs):
    x = np.asarray(inputs["x"], np.float32)
    col = np.asarray(inputs["edge_index"], np.int32)[1]
    sh = prepare_shared(inputs)
    in_maps = []
    for c in range(NCORES):
        xT = np.ascontiguousarray(x[c * NN:(c + 1) * NN].T)
        cs = col[c * NN * K:(c + 1) * NN * K].reshape(G, P, K)
        colr = np.ascontiguousarray(
            cs.transpose(1, 0, 2).reshape(P, G * K)).astype(np.int32)
        in_maps.append({"xT": xT, "colr": colr, **sh})
    return in_maps


_NC = None


def _ensure_ntff_hook():
    """Register the axon NTFF profiling hook if the image's antenv lacks
    the axon_hooks module (needed for trace=True exec-time capture)."""
    import sys, types
    try:
        from antenv.axon_hooks import get_axon_ntff_profile_hook  # noqa
        return
    except ImportError:
        pass
    try:
        import antenv
        from trn_agent_boot.trn_boot import _ntff_profile_via_ctypes
        mod = types.ModuleType("antenv.axon_hooks")
        hook = _ntff_profile_via_ctypes("/opt/axon/libaxon_pjrt.so")
        mod._hook = hook
        mod.set_axon_ntff_profile_hook = lambda h: setattr(mod, "_hook", h)
        mod.get_axon_ntff_profile_hook = lambda: mod._hook
        sys.modules["antenv.axon_hooks"] = mod
        antenv.axon_hooks = mod
    except Exception as e:  # pragma: no cover
        print(f"ntff hook setup failed: {e}")


def kernel(**inputs):
    global _NC
    if _NC is None:
        _NC = build_nc()
    in_maps = make_in_maps(inputs)
    trace = os.environ.get("BHGNN_TRACE", "") not in ("", "0")
    if trace:
        _ensure_ntff_hook()
    res = run_bass_kernel_spmd(_NC, in_maps, core_ids=list(range(NCORES)),
                               trace=trace)
    full = np.concatenate([res.results[c]["out"] for c in range(NCORES)],
                          axis=0)
    if trace and res.exec_time_ns is not None:
        print(f"HW exec time: {res.exec_time_ns} ns")
    return (np.ascontiguousarray(full[:, 0:10]),
            np.ascontiguousarray(full[:, 10:20]),
            np.ascontiguousarray(full[:, 20:148]))


# revision 20
# speedup vs baseline: 2.1259x; 1.0546x over previous
"""BHGNN Trainium2 kernel (8 NeuronCores, graph-level data parallel).

Per core: 64 graphs x 128 nodes. The sparsemax attention is computed
densely, without any per-edge gather:
  - al[n], ar[n], XW from per-graph PE matmuls on fp16 x^T.
  - Duplicate edge columns are merged by a 16-channel Batcher sort on the
    column ids; GPSIMD local_scatter builds the dense count matrix
    cnt[n, m] per graph.
  - zd1[n, m] = max(ar[m] + al[n] + 1, 1) densely (PE broadcast of ar).
  - Sparsemax threshold via tau+1 = max_m (S[n,m]-1)/N[n,m], where
    S = (cnt*zd1) @ C, N = cnt @ C and C[m',m] = [ar(m') >= ar(m)] is a
    graph-global comparison matrix -> two per-graph PE matmuls.
  - A = cnt * max(zd1 - tau1, max(1-tau1, 0)); agg = (A + I) @ XW via PE
    (identity accumulated in PSUM); h = relu(0.5*agg + b).
  - Pooling (strided DVE reduces) -> outer softmax attention, two tiny
    GCNs, readout, MLP heads in f32.
"""
import os
from contextlib import ExitStack

import numpy as np

import concourse.bass as bass
import concourse.mybir as mybir
import concourse.bacc as bacc
import concourse.tile as tile
from concourse.bass_utils import run_bass_kernel_spmd

FP16 = mybir.dt.float16
F32 = mybir.dt.float32
I32 = mybir.dt.int32
I16 = mybir.dt.int16
AF = mybir.ActivationFunctionType
OP = mybir.AluOpType
X = mybir.AxisListType.X

P = 128          # partitions = nodes per graph
G = 64           # graphs per core
NN = P * G       # nodes per core (8192)
K = 16           # edges per node
NH = 128
H = 6
NCORES = 8
NHYP = G * H     # hypernodes per core (384)

WNAMES = [
    ("watt2", [P, 2]), ("winner", [P, P]), ("binner", [P, 1]),
    ("wattm", [P, 2]), ("wattx", [P, 2]),
    ("wout1a", [P, P]), ("wout1b", [P, P]), ("bout1", [P, 1]),
    ("wout2", [P, P]), ("bout2", [P, 1]),
    ("f1m00", [P, P]), ("f1m10", [P, P]), ("f1m01", [P, P]), ("f1m11", [P, P]),
    ("b1m0", [P, 1]), ("b1m1", [P, 1]),
    ("f2m0", [P, P]), ("f2m1", [P, P]), ("b2m", [P, 1]),
    ("f1v00", [P, P]), ("f1v10", [P, P]), ("f1v01", [P, P]), ("f1v11", [P, P]),
    ("b1v0", [P, 1]), ("b1v1", [P, 1]),
    ("f2v0", [P, P]), ("f2v1", [P, P]), ("b2v", [P, 1]),
    ("clsmW", [P, 10]), ("clsmb", [10, 1]),
    ("clsvW", [P, 10]), ("clsvb", [10, 1]),
]

# Batcher odd-even merge sort network for 16 channels. Each layer:
# (channel-dim factors, lo index, hi index).
SORT_LAYERS = [
    ((8, 2), (slice(None), slice(0, 1)), (slice(None), slice(1, 2))),
    ((4, 4), (slice(None), slice(0, 2)), (slice(None), slice(2, 4))),
    ((4, 4), (slice(None), slice(1, 2)), (slice(None), slice(2, 3))),
    ((2, 8), (slice(None), slice(0, 4)), (slice(None), slice(4, 8))),
    ((2, 8), (slice(None), slice(2, 4)), (slice(None), slice(4, 6))),
    ((2, 4, 2), (slice(None), slice(0, 3), slice(1, 2)),
     (slice(None), slice(1, 4), slice(0, 1))),
    ((1, 16), (slice(None), slice(0, 8)), (slice(None), slice(8, 16))),
    ((1, 16), (slice(None), slice(4, 8)), (slice(None), slice(8, 12))),
    ((4, 4), (slice(0, 3), slice(2, 4)), (slice(1, 4), slice(0, 2))),
    ((8, 2), (slice(0, 7), slice(1, 2)), (slice(1, 8), slice(0, 1))),
]


def _chslice(ap, factors, idx):
    names = "abcd"[: len(factors)]
    pat = f"p ({' '.join(names)} g) -> p {' '.join(names)} g"
    v = ap.rearrange(pat, **{n: f for n, f in zip(names, factors)})
    return v[(slice(None),) + idx + (slice(None),)]


def build_nc():
    nc = bacc.Bacc("TRN2", target_bir_lowering=False, debug=False,
                   num_devices=NCORES)
    xT_d = nc.declare_dram_parameter("xT", [P, NN], F32, isOutput=False)
    col_d = nc.declare_dram_parameter("colr", [P, G * K], I32, isOutput=False)
    wd = {}
    for nm, shp in WNAMES:
        wd[nm] = nc.declare_dram_parameter(nm, shp, F32, isOutput=False)
    out_d = nc.declare_dram_parameter("out", [G, 148], F32, isOutput=True)

    with tile.TileContext(nc) as tc, ExitStack() as ctx:
        pp = ctx.enter_context(tc.tile_pool(name="persist", bufs=1))
        wk = ctx.enter_context(tc.tile_pool(name="work", bufs=1))
        pmm = ctx.enter_context(
            tc.tile_pool(name="psum", bufs=6, space="PSUM"))

        ps_ctr = [0]

        def ps(shape, dtype=F32):
            ps_ctr[0] += 1
            return pmm.tile(shape, dtype, tag="ps", name=f"pst{ps_ctr[0]}")

        # ---------------- weights / constants ----------------
        w = {}
        for nm, shp in WNAMES:
            t = pp.tile(shp, F32, tag=f"w_{nm}")
            nc.sync.dma_start(t[:], wd[nm].ap())
            w[nm] = t
        watt2_h = pp.tile([P, 2], FP16, tag="watt2h")
        nc.gpsimd.dma_start(watt2_h[:], wd["watt2"].ap())
        winner_h = pp.tile([P, P], FP16, tag="winnerh")
        nc.gpsimd.dma_start(winner_h[:], wd["winner"].ap())

        iota_row = pp.tile([P, P], I32, tag="iota_row")
        nc.gpsimd.iota(iota_row[:], pattern=[[1, P]], channel_multiplier=0)
        iota_part = pp.tile([P, 1], I32, tag="iota_part")
        nc.gpsimd.iota(iota_part[:], pattern=[[0, 1]], channel_multiplier=1)
        g128rep = pp.tile([P, K * G], I32, tag="g128rep")   # (k,g): 128*(g%8)
        nc.gpsimd.iota(g128rep[:], pattern=[[0, K], [0, 8], [P, 8]],
                       channel_multiplier=0)

        iota_rowf = pp.tile([P, P], F32, tag="iota_rowf")
        nc.vector.tensor_copy(iota_rowf[:], iota_row[:])
        iota_partf = pp.tile([P, 1], F32, tag="iota_partf")
        nc.vector.tensor_copy(iota_partf[:], iota_part[:])
        ident32 = pp.tile([P, P], F32, tag="ident32")
        nc.vector.tensor_scalar(ident32[:], iota_rowf[:], iota_partf[:, 0:1],
                                None, op0=OP.is_equal)
        ident16 = pp.tile([P, P], FP16, tag="ident16")
        nc.vector.tensor_copy(ident16[:], ident32[:])
        ones_col = pp.tile([1, P], F32, tag="ones_col")
        nc.vector.memset(ones_col[:], 1.0)
        ones16 = pp.tile([1, P], FP16, tag="ones16")
        nc.vector.memset(ones16[:], 1.0)
        cntinv = pp.tile([P, H * G], F32, tag="cntinv")     # (g, r)
        nc.vector.memset(cntinv[:], 1.0 / 21.0)
        nc.vector.memset(
            cntinv.rearrange("p (g r) -> p g r", g=G)[:, :, 0:2],
            1.0 / 22.0)

        # ---------------- x^T (fp16 cast) and col ----------------
        xT = pp.tile([P, NN], FP16, tag="xT")
        for c in range(8):
            sl = slice(c * 1024, (c + 1) * 1024)
            nc.gpsimd.dma_start(xT[:, sl], xT_d.ap()[:, sl])
        colr = pp.tile([P, G * K], I32, tag="colr")
        nc.sync.dma_start(colr[:], col_d.ap())

        # ---------------- stage A: al, ar, XW per graph ----------------
        al_sb = wk.tile([P, G], F32, tag="al")
        arc_sb = wk.tile([P, G], F32, tag="arc")
        xw16 = wk.tile([P, G * P], FP16, tag="xw16")
        for b in range(G // 4):
            ps_xw = ps([P, 512])
            ps_al = ps([P, 8])
            for i in range(4):
                g = b * 4 + i
                lhs = xT[:, g * P:(g + 1) * P]
                nc.tensor.matmul(ps_al[:, 2 * i:2 * i + 2], lhs, watt2_h[:])
                nc.tensor.matmul(ps_xw[:, i * P:(i + 1) * P], lhs,
                                 winner_h[:])
            alr = ps_al.rearrange("p (i t) -> p i t", i=4)
            nc.vector.tensor_copy(
                al_sb.rearrange("p (b i) -> p b i", b=G // 4)[:, b],
                alr[:, :, 0])
            nc.vector.tensor_copy(
                arc_sb.rearrange("p (b i) -> p b i", b=G // 4)[:, b],
                alr[:, :, 1])
            nc.scalar.copy(xw16[:, b * 512:(b + 1) * 512], ps_xw[:])

        # ar as fp16-rounded per-node f32 scalars (consistent with ar_all)
        arc16 = wk.tile([P, G], FP16, tag="arc16")
        nc.vector.tensor_copy(arc16[:], arc_sb[:])
        arcr = wk.tile([P, G], F32, tag="arcr")
        nc.vector.tensor_copy(arcr[:], arc16[:])
        al1h = wk.tile([P, G], FP16, tag="al1h")
        nc.vector.tensor_scalar_add(al1h[:], al_sb[:], 1.0)

        # ar replicated on all partitions: [p, (g*128+m)] fp16
        ar_flat = wk.tile([1, NN], FP16, tag="arflat")
        for c in range(NN // 512):
            ps_ar = ps([1, 512])
            nc.tensor.matmul(ps_ar[:], watt2_h[:, 1:2],
                             xT[:, c * 512:(c + 1) * 512])
            nc.vector.tensor_copy(ar_flat[:, c * 512:(c + 1) * 512],
                                  ps_ar[:])
        ar_all = wk.tile([P, NN], FP16, tag="arall")
        for c in range(NN // 512):
            ps_b = ps([P, 512])
            nc.tensor.matmul(ps_b[:], ones16[:],
                             ar_flat[:, c * 512:(c + 1) * 512])
            nc.scalar.copy(ar_all[:, c * 512:(c + 1) * 512], ps_b[:])

        # ---------------- stage C: dedup cols -> cnt matrix ----------------
        colf = wk.tile([P, K * G], F32, tag="colf")         # (k, g)
        colt = wk.tile([P, K * G], I32, tag="colt")
        nc.vector.tensor_copy(colt[:],
                              colr.rearrange("p (g j) -> p j g", g=G))
        nc.vector.tensor_scalar(colt[:], colt[:], 127, None,
                                op0=OP.bitwise_and)
        nc.vector.tensor_copy(colf[:], colt[:])
        sorttmp = wk.tile([P, 8 * G], F32, tag="sorttmp")
        for factors, lo_i, hi_i in SORT_LAYERS:
            lo = _chslice(colf, factors, lo_i)
            hi = _chslice(colf, factors, hi_i)
            ext = tuple(len(range(*s.indices(f)))
                        for s, f in zip(lo_i, factors))
            npair = int(np.prod(ext))
            tmp = _chslice(sorttmp[:, 0:npair * G], ext,
                           tuple(slice(None) for _ in ext))
            nc.vector.tensor_tensor(tmp, lo, hi, op=OP.min)
            nc.vector.tensor_tensor(lo, lo, hi, op=OP.max)
            nc.vector.tensor_copy(hi, tmp)
        # eq channels with zero pad; dead_k = eq_{k+1}
        eqt = wk.tile([P, (K + 1) * G], F32, tag="eqt")
        nc.vector.memset(eqt[:, 0:G], 0.0)
        nc.vector.memset(eqt[:, K * G:], 0.0)
        nc.vector.tensor_tensor(eqt[:, G:K * G], colf[:, G:],
                                colf[:, 0:(K - 1) * G], op=OP.is_equal)
        # run counts: c_k = 1 + eq_k * c_{k-1}; last slot of run holds total
        cntc = wk.tile([P, K * G], F32, tag="cntc")
        nc.vector.memset(cntc[:, 0:G], 1.0)
        for k in range(1, K):
            nc.vector.tensor_mul(cntc[:, k * G:(k + 1) * G],
                                 eqt[:, k * G:(k + 1) * G],
                                 cntc[:, (k - 1) * G:k * G])
            nc.vector.tensor_scalar_add(cntc[:, k * G:(k + 1) * G],
                                        cntc[:, k * G:(k + 1) * G], 1.0)
        deadt = eqt[:, G:]
        ndead = wk.tile([P, K * G], F32, tag="ndead")
        nc.vector.tensor_scalar(ndead[:], deadt, -1.0, 1.0,
                                op0=OP.mult, op1=OP.add)
        rampf = wk.tile([P, K * G], F32, tag="rampf")
        nc.vector.tensor_copy(rampf[:], g128rep[:])
        # idx = dead ? -1 : col + 128*(g%8)  == (col+ramp+1)*ndead - 1
        idxf = wk.tile([P, K * G], F32, tag="idxf")
        nc.vector.tensor_add(idxf[:], colf[:], rampf[:])
        nc.vector.scalar_tensor_tensor(idxf[:], idxf[:], 1.0, ndead[:],
                                       op0=OP.add, op1=OP.mult)
        nc.vector.tensor_scalar_add(idxf[:], idxf[:], -1.0)
        sidx = wk.tile([P, G * K], I16, tag="sidx")
        sval = wk.tile([P, G * K], FP16, tag="sval")
        nc.vector.tensor_copy(sidx.rearrange("p (g k) -> p k g", g=G),
                              idxf.rearrange("p (k g) -> p k g", k=K))
        nc.vector.tensor_copy(sval.rearrange("p (g k) -> p k g", g=G),
                              cntc.rearrange("p (k g) -> p k g", k=K))
        cntd = wk.tile([P, G * P], FP16, tag="cntd")
        for wnd in range(8):
            nc.gpsimd.local_scatter(
                cntd[:, wnd * 1024:(wnd + 1) * 1024],
                sval[:, wnd * 128:(wnd + 1) * 128],
                sidx[:, wnd * 128:(wnd + 1) * 128],
                channels=P, num_elems=1024, num_idxs=128)

        # ---------------- stage D: zd1, S/N matmuls, tau ----------------
        zd1 = wk.tile([P, NN], FP16, tag="zd1")
        zd3 = zd1.rearrange("p (g m) -> p g m", g=G)
        al1b = al1h.rearrange("p (g o) -> p g o", o=1) \
            .broadcast_to([P, G, P])
        nc.vector.tensor_tensor(zd3, ar_all.rearrange("p (g m) -> p g m",
                                                      g=G), al1b, op=OP.add)
        nc.vector.tensor_scalar_max(zd1[:], zd1[:], 1.0)

        tau = wk.tile([P, G], F32, tag="tau")
        nz = wk.tile([P, G], F32, tag="nz")
        sz = wk.tile([P, G], F32, tag="sz")
        for hf in range(2):
            S_h = wk.tile([P, NN // 2], FP16, tag="Sh")
            N_h = wk.tile([P, NN // 2], FP16, tag="Nh")
            for b in range(8):
                gb = hf * 8 + b
                czb = wk.tile([P, 512], FP16, tag="czb", bufs=2)
                nc.vector.tensor_mul(czb[:],
                                     cntd[:, gb * 512:(gb + 1) * 512],
                                     zd1[:, gb * 512:(gb + 1) * 512])
                ps_t1 = ps([P, 512], FP16)
                ps_t2 = ps([P, 512], FP16)
                for i in range(4):
                    g = gb * 4 + i
                    nc.tensor.transpose(ps_t1[:, i * P:(i + 1) * P],
                                        cntd[:, g * P:(g + 1) * P],
                                        ident16[:])
                    nc.tensor.transpose(ps_t2[:, i * P:(i + 1) * P],
                                        czb[:, i * P:(i + 1) * P],
                                        ident16[:])
                ctb = wk.tile([P, 512], FP16, tag="ctb", bufs=2)
                cztb = wk.tile([P, 512], FP16, tag="cztb", bufs=2)
                nc.scalar.copy(ctb[:], ps_t1[:])
                nc.scalar.copy(cztb[:], ps_t2[:])
                ps_s = ps([P, 512])
                ps_n = ps([P, 512])
                for i in range(4):
                    g = gb * 4 + i
                    Cg = wk.tile([P, P], FP16, tag="Cg", bufs=8)
                    nc.vector.tensor_scalar(
                        Cg[:], ar_all[:, g * P:(g + 1) * P],
                        arcr[:, g:g + 1], None, op0=OP.is_le)
                    nc.tensor.matmul(ps_s[:, i * P:(i + 1) * P],
                                     cztb[:, i * P:(i + 1) * P], Cg[:])
                    nc.tensor.matmul(ps_n[:, i * P:(i + 1) * P],
                                     ctb[:, i * P:(i + 1) * P], Cg[:])
                nc.scalar.copy(S_h[:, b * 512:(b + 1) * 512], ps_s[:])
                nc.scalar.copy(N_h[:, b * 512:(b + 1) * 512], ps_n[:])
            # support test (division-free): m in support iff N*zd1 - S > -1
            hsl = slice(hf * NN // 2, (hf + 1) * NN // 2)
            gsl = slice(hf * 32, (hf + 1) * 32)
            t1 = wk.tile([P, NN // 2], FP16, tag="rq")
            nc.vector.tensor_mul(t1[:], N_h[:], zd1[:, hsl])
            nc.vector.tensor_sub(t1[:], t1[:], S_h[:])
            nc.vector.tensor_scalar(t1[:], t1[:], -1.0, None, op0=OP.is_gt)
            nc.vector.tensor_mul(t1[:], t1[:], cntd[:, hsl])
            nc.vector.tensor_reduce(
                nz[:, gsl], t1.rearrange("p (g m) -> p g m", g=32),
                axis=X, op=OP.add)
            nc.vector.tensor_mul(t1[:], t1[:], zd1[:, hsl])
            nc.vector.tensor_reduce(
                sz[:, gsl], t1.rearrange("p (g m) -> p g m", g=32),
                axis=X, op=OP.add)

        # tau = (SZ - 1) / NZ  (tiny division)
        rnz = wk.tile([P, G], F32, tag="rnz")
        nc.vector.reciprocal(rnz[:], nz[:])
        nc.vector.scalar_tensor_tensor(tau[:], sz[:], -1.0, rnz[:],
                                       op0=OP.add, op1=OP.mult)

        # ---------------- stage E: A dense, transpose, agg ----------------
        ntau16 = wk.tile([P, G], FP16, tag="ntau16")
        nc.vector.tensor_scalar_mul(ntau16[:], tau[:], -1.0)
        gam16 = wk.tile([P, G], FP16, tag="gam16")
        nc.vector.tensor_scalar(gam16[:], tau[:], -1.0, 1.0,
                                op0=OP.mult, op1=OP.add)
        nc.vector.tensor_scalar_max(gam16[:], gam16[:], 0.0)
        A = wk.tile([P, G * P], FP16, tag="arall")
        A3 = A.rearrange("p (g m) -> p g m", g=G)
        ntb = ntau16.rearrange("p (g o) -> p g o", o=1) \
            .broadcast_to([P, G, P])
        gmb = gam16.rearrange("p (g o) -> p g o", o=1) \
            .broadcast_to([P, G, P])
        nc.vector.tensor_tensor(A3, zd3, ntb, op=OP.add)
        nc.vector.tensor_tensor(A3, A3, gmb, op=OP.max)
        nc.vector.tensor_mul(A[:], A[:], cntd[:])

        hT = wk.tile([P, G * P], FP16, tag="cntd")
        for b in range(G // 4):
            ps_t = ps([P, 512], FP16)
            for i in range(4):
                g = b * 4 + i
                nc.tensor.transpose(ps_t[:, i * P:(i + 1) * P],
                                    A[:, g * P:(g + 1) * P], ident16[:])
            atb = wk.tile([P, 512], FP16, tag="atb", bufs=2)
            nc.scalar.copy(atb[:], ps_t[:])
            ps_a = ps([P, 512])
            for i in range(4):
                g = b * 4 + i
                nc.tensor.matmul(ps_a[:, i * P:(i + 1) * P],
                                 xw16[:, g * P:(g + 1) * P],
                                 atb[:, i * P:(i + 1) * P],
                                 start=True, stop=False)
                nc.tensor.matmul(ps_a[:, i * P:(i + 1) * P],
                                 xw16[:, g * P:(g + 1) * P],
                                 ident16[:], start=False, stop=True)
            nc.scalar.activation(hT[:, b * 512:(b + 1) * 512], ps_a[:],
                                 AF.Relu, bias=w["binner"][:, 0:1],
                                 scale=0.5)

        # ---------------- stage F: pooling ----------------
        xhm = wk.tile([P, NHYP], F32, tag="xw16")           # (g, r) mean
        xhx = wk.tile([P, NHYP], F32, tag="colf")           # (g, r) max
        hT_v = hT.rearrange("p (g n) -> p g n", g=G)
        hT_seg = hT_v[:, :, 0:126].rearrange("p g (kk r) -> p g r kk", r=H)
        xhm_v = xhm.rearrange("p (g r) -> p g r", g=G)
        xhx_v = xhx.rearrange("p (g r) -> p g r", g=G)
        nc.vector.tensor_reduce(xhm_v, hT_seg, axis=X, op=OP.add)
        nc.vector.tensor_tensor(xhm_v[:, :, 0:2], xhm_v[:, :, 0:2],
                                hT_v[:, :, 126:128], op=OP.add)
        nc.vector.tensor_mul(xhm[:], xhm[:], cntinv[:])
        nc.vector.tensor_reduce(xhx_v, hT_seg, axis=X, op=OP.max)
        nc.vector.tensor_tensor(xhx_v[:, :, 0:2], xhx_v[:, :, 0:2],
                                hT_v[:, :, 126:128], op=OP.max)

        # ---------------- stage G: outer attention ----------------
        ps_w = ps([2, NHYP])
        nc.tensor.matmul(ps_w[:], w["wattm"][:], xhm[:], start=True,
                         stop=False)
        nc.tensor.matmul(ps_w[:], w["wattx"][:], xhx[:], start=False,
                         stop=True)
        wlr2 = wk.tile([2, NHYP], F32, tag="wlr2")
        nc.vector.tensor_copy(wlr2[:], ps_w[:])
        wlr = wk.tile([G, 12], F32, tag="wlr")
        nc.sync.dma_start(wlr[:, 0:6], wlr2[0:1, :])
        nc.sync.dma_start(wlr[:, 6:12], wlr2[1:2, :])
        whm = wk.tile([G, 36], F32, tag="whm")
        whm_v = whm.rearrange("g (r s) -> g r s", r=H)
        for r in range(H):
            nc.vector.tensor_scalar(whm_v[:, r], wlr[:, 6:12],
                                    wlr[:, r:r + 1], None, op0=OP.add)
        wt = wk.tile([G, 36], F32, tag="wt36")
        wt_v = wt.rearrange("g (r s) -> g r s", r=H)
        nc.vector.tensor_scalar_min(wt[:], whm[:], 0.0)
        nc.vector.tensor_scalar_max(whm[:], whm[:], 0.0)
        nc.vector.scalar_tensor_tensor(whm[:], wt[:], 0.2, whm[:],
                                       op0=OP.mult, op1=OP.add)
        rmax = wk.tile([G, H], F32, tag="rmax")
        nc.vector.tensor_tensor(wt_v[:, :, 0:3], whm_v[:, :, 0:3],
                                whm_v[:, :, 3:6], op=OP.max)
        nc.vector.tensor_tensor(rmax[:], wt_v[:, :, 0], wt_v[:, :, 1],
                                op=OP.max)
        nc.vector.tensor_tensor(rmax[:], rmax[:], wt_v[:, :, 2], op=OP.max)
        for r in range(H):
            nc.vector.tensor_scalar(whm_v[:, r], whm_v[:, r],
                                    rmax[:, r:r + 1], None, op0=OP.subtract)
        nc.scalar.activation(whm[:], whm[:], AF.Exp)
        rsum = wk.tile([G, H], F32, tag="rsum")
        nc.vector.tensor_tensor(wt_v[:, :, 0:3], whm_v[:, :, 0:3],
                                whm_v[:, :, 3:6], op=OP.add)
        nc.vector.tensor_tensor(rsum[:], wt_v[:, :, 0], wt_v[:, :, 1],
                                op=OP.add)
        nc.vector.tensor_tensor(rsum[:], rsum[:], wt_v[:, :, 2], op=OP.add)
        nc.vector.reciprocal(rsum[:], rsum[:])
        for r in range(H):
            nc.vector.tensor_scalar(whm_v[:, r], whm_v[:, r],
                                    rsum[:, r:r + 1], None, op0=OP.mult)
        ahflat = wk.tile([1, G * 36], F32, tag="arflat")
        nc.sync.dma_start(ahflat[:], whm[:])
        ahrep = wk.tile([P, G * 36], F32, tag="zd1")
        for c in range(5):
            lo = c * 512
            n = min(512, G * 36 - lo)
            ps_b2 = ps([P, 512])
            nc.tensor.matmul(ps_b2[:, 0:n], ones_col[:], ahflat[:, lo:lo + n])
            nc.scalar.copy(ahrep[:, lo:lo + n], ps_b2[:, 0:n])
        ah_v = ahrep.rearrange("p (g q) -> p g q", g=G)

        def outer_gcn(xin_m, xin_x, wa, wb, bias, name):
            p1 = ps([P, NHYP])
            if xin_x is None:
                nc.tensor.matmul(p1[:], wa[:], xin_m[:])
            else:
                nc.tensor.matmul(p1[:], wa[:], xin_m[:], start=True,
                                 stop=False)
                nc.tensor.matmul(p1[:], wb[:], xin_x[:], start=False,
                                 stop=True)
            xwT = wk.tile([P, NHYP], F32, tag="xwT")
            nc.vector.tensor_copy(xwT[:], p1[:])
            agg = wk.tile([P, NHYP], F32, tag="agg")
            agg_v = agg.rearrange("p (g r) -> p g r", g=G)
            xw_v = xwT.rearrange("p (g s) -> p g s", g=G)
            tmpa = wk.tile([P, G], F32, tag="tmpa")
            for r in range(H):
                for s in range(H):
                    if s == 0:
                        nc.vector.tensor_mul(agg_v[:, :, r], xw_v[:, :, s],
                                             ah_v[:, :, r * H + s])
                    else:
                        nc.vector.tensor_mul(tmpa[:], xw_v[:, :, s],
                                             ah_v[:, :, r * H + s])
                        nc.vector.tensor_tensor(agg_v[:, :, r],
                                                agg_v[:, :, r], tmpa[:],
                                                op=OP.add)
            nc.vector.tensor_add(agg[:], agg[:], xwT[:])
            zT = wk.tile([P, NHYP], F32, tag="zT")
            nc.scalar.activation(zT[:], agg[:], AF.Relu, bias=bias[:, 0:1],
                                 scale=0.5)
            return zT

        z1h = outer_gcn(xhm, xhx, w["wout1a"], w["wout1b"], w["bout1"], "o1")
        z2h = outer_gcn(z1h, None, w["wout2"], None, w["bout2"], "o2")

        x1m = wk.tile([P, G], F32, tag="x1m")
        x1x = wk.tile([P, G], F32, tag="x1x")
        z2_v = z2h.rearrange("p (g r) -> p g r", g=G)
        nc.vector.tensor_reduce(x1m[:], z2_v, axis=X, op=OP.add)
        nc.vector.tensor_scalar_mul(x1m[:], x1m[:], 1.0 / H)
        nc.vector.tensor_reduce(x1x[:], z2_v, axis=X, op=OP.max)

        # ---------------- MLP heads ----------------
        def head(pfx, xm, xx):
            m1 = []
            for j in range(2):
                p2 = ps([P, G])
                nc.tensor.matmul(p2[:], w[f"f1{pfx}0{j}"][:], xm[:],
                                 start=True, stop=False)
                nc.tensor.matmul(p2[:], w[f"f1{pfx}1{j}"][:], xx[:],
                                 start=False, stop=True)
                t = wk.tile([P, G], F32, tag=f"m1{pfx}{j}")
                nc.scalar.activation(t[:], p2[:], AF.Relu,
                                     bias=w[f"b1{pfx}{j}"][:, 0:1])
                m1.append(t)
            p3 = ps([P, G])
            nc.tensor.matmul(p3[:], w[f"f2{pfx}0"][:], m1[0][:],
                             start=True, stop=False)
            nc.tensor.matmul(p3[:], w[f"f2{pfx}1"][:], m1[1][:],
                             start=False, stop=True)
            mT = wk.tile([P, G], F32, tag=f"mT{pfx}")
            nc.scalar.activation(mT[:], p3[:], AF.Relu,
                                 bias=w[f"b2{pfx}"][:, 0:1])
            p4 = ps([10, G])
            nc.tensor.matmul(p4[:], w[f"cls{pfx}W"][:], mT[:])
            o = wk.tile([10, G], F32, tag=f"o{pfx}")
            nc.vector.tensor_scalar(o[:], p4[:], w[f"cls{pfx}b"][:, 0:1],
                                    None, op0=OP.add)
            return mT, o

        mT, om = head("m", x1m, x1x)
        _, ov = head("v", x1m, x1x)

        # ---------------- outputs ----------------
        p5 = ps([G, P])
        nc.tensor.transpose(p5[:], mT[:], ident32[:])
        m_t = wk.tile([G, P], F32, tag="m_t")
        nc.vector.tensor_copy(m_t[:], p5[:])
        nc.sync.dma_start(out_d.ap()[:, 20:148], m_t[:])
        for o_ap, cols in ((om, slice(0, 10)), (ov, slice(10, 20))):
            p6 = ps([G, 10])
            nc.tensor.transpose(p6[:], o_ap[:], ident32[0:10, 0:10])
            o_t = wk.tile([G, 10], F32, tag="o_t")
            nc.vector.tensor_copy(o_t[:], p6[:])
            nc.sync.dma_start(out_d.ap()[:, cols], o_t[:])

    nc.compile()
    return nc


def prepare_shared(inputs):
    f32 = np.float32
    att = np.asarray(inputs["att_inner"], f32)
    atto = np.asarray(inputs["att_outer"], f32)
    sh = {
        "watt2": np.ascontiguousarray(np.stack([att[:P], att[P:]], axis=1)),
        "winner": np.ascontiguousarray(np.asarray(inputs["W_inner"], f32)),
        "binner": np.asarray(inputs["b_inner"], f32).reshape(P, 1).copy(),
        "wattm": np.ascontiguousarray(
            np.stack([atto[0:128], atto[256:384]], axis=1)),
        "wattx": np.ascontiguousarray(
            np.stack([atto[128:256], atto[384:512]], axis=1)),
        "wout1a": np.ascontiguousarray(np.asarray(inputs["W_out1"], f32)[:P]),
        "wout1b": np.ascontiguousarray(np.asarray(inputs["W_out1"], f32)[P:]),
        "bout1": np.asarray(inputs["b_out1"], f32).reshape(P, 1).copy(),
        "wout2": np.ascontiguousarray(np.asarray(inputs["W_out2"], f32)),
        "bout2": np.asarray(inputs["b_out2"], f32).reshape(P, 1).copy(),
        "clsmW": np.ascontiguousarray(np.asarray(inputs["clsm_W"], f32)),
        "clsmb": np.asarray(inputs["clsm_b"], f32).reshape(10, 1).copy(),
        "clsvW": np.ascontiguousarray(np.asarray(inputs["clsv_W"], f32)),
        "clsvb": np.asarray(inputs["clsv_b"], f32).reshape(10, 1).copy(),
    }
    for pfx in ("m", "v"):
        w1 = np.asarray(inputs[f"fc1{pfx}_W"], f32)
        b1 = np.asarray(inputs[f"fc1{pfx}_b"], f32)
        w2 = np.asarray(inputs[f"fc2{pfx}_W"], f32)
        for j in range(2):
            sh[f"f1{pfx}0{j}"] = np.ascontiguousarray(
                w1[0:P, j * P:(j + 1) * P])
            sh[f"f1{pfx}1{j}"] = np.ascontiguousarray(
                w1[P:2 * P, j * P:(j + 1) * P])
            sh[f"b1{pfx}{j}"] = b1[j * P:(j + 1) * P].reshape(P, 1).copy()
        sh[f"f2{pfx}0"] = np.ascontiguousarray(w2[0:P])
        sh[f"f2{pfx}1"] = np.ascontiguousarray(w2[P:2 * P])
        sh[f"b2{pfx}"] = np.asarray(
            inputs[f"fc2{pfx}_b"], f32).reshape(P, 1).copy()
    return sh


def make_in_maps(inputs):
    x = np.asarray(inputs["x"], np.float32)
    col = np.asarray(inputs["edge_index"], np.int32)[1]
    sh = prepare_shared(inputs)
    in_maps = []
    for c in range(NCORES):
        xT = np.ascontiguousarray(x[c * NN:(c + 1) * NN].T)
        cs = col[c * NN * K:(c + 1) * NN * K].reshape(G, P, K)
        colr = np.ascontiguousarray(
            cs.transpose(1, 0, 2).reshape(P, G * K)).astype(np.int32)
        in_maps.append({"xT": xT, "colr": colr, **sh})
    return in_maps


_NC = None


def _ensure_ntff_hook():
    """Register the axon NTFF profiling hook if the image's antenv lacks
    the axon_hooks module (needed for trace=True exec-time capture)."""
    import sys, types
    try:
        from antenv.axon_hooks import get_axon_ntff_profile_hook  # noqa
        return
    except ImportError:
        pass
    try:
        import antenv
        from trn_agent_boot.trn_boot import _ntff_profile_via_ctypes
        mod = types.ModuleType("antenv.axon_hooks")
        hook = _ntff_profile_via_ctypes("/opt/axon/libaxon_pjrt.so")
        mod._hook = hook
        mod.set_axon_ntff_profile_hook = lambda h: setattr(mod, "_hook", h)
        mod.get_axon_ntff_profile_hook = lambda: mod._hook
        sys.modules["antenv.axon_hooks"] = mod
        antenv.axon_hooks = mod
    except Exception as e:  # pragma: no cover
        print(f"ntff hook setup failed: {e}")


def kernel(**inputs):
    global _NC
    if _NC is None:
        _NC = build_nc()
    in_maps = make_in_maps(inputs)
    trace = os.environ.get("BHGNN_TRACE", "") not in ("", "0")
    if trace:
        _ensure_ntff_hook()
    res = run_bass_kernel_spmd(_NC, in_maps, core_ids=list(range(NCORES)),
                               trace=trace)
    full = np.concatenate([res.results[c]["out"] for c in range(NCORES)],
                          axis=0)
    if trace and res.exec_time_ns is not None:
        print(f"HW exec time: {res.exec_time_ns} ns")
    return (np.ascontiguousarray(full[:, 0:10]),
            np.ascontiguousarray(full[:, 10:20]),
            np.ascontiguousarray(full[:, 20:148]))


# revision 22
# speedup vs baseline: 2.1467x; 1.0098x over previous
"""BHGNN Trainium2 kernel (8 NeuronCores, graph-level data parallel).

Per core: 64 graphs x 128 nodes. The sparsemax attention is computed
densely, without any per-edge gather:
  - al[n], ar[n], XW from per-graph PE matmuls on fp16 x^T.
  - Duplicate edge columns are merged by a 16-channel Batcher sort on the
    column ids; GPSIMD local_scatter builds the dense count matrix
    cnt[n, m] per graph.
  - zd1[n, m] = max(ar[m] + al[n] + 1, 1) densely (PE broadcast of ar).
  - Sparsemax threshold via tau+1 = max_m (S[n,m]-1)/N[n,m], where
    S = (cnt*zd1) @ C, N = cnt @ C and C[m',m] = [ar(m') >= ar(m)] is a
    graph-global comparison matrix -> two per-graph PE matmuls.
  - A = cnt * max(zd1 - tau1, max(1-tau1, 0)); agg = (A + I) @ XW via PE
    (identity accumulated in PSUM); h = relu(0.5*agg + b).
  - Pooling (strided DVE reduces) -> outer softmax attention, two tiny
    GCNs, readout, MLP heads in f32.
"""
import os
from contextlib import ExitStack

import numpy as np

import concourse.bass as bass
import concourse.mybir as mybir
import concourse.bacc as bacc
import concourse.tile as tile
from concourse.bass_utils import run_bass_kernel_spmd

FP16 = mybir.dt.float16
F32 = mybir.dt.float32
I32 = mybir.dt.int32
I16 = mybir.dt.int16
AF = mybir.ActivationFunctionType
OP = mybir.AluOpType
X = mybir.AxisListType.X

P = 128          # partitions = nodes per graph
G = 64           # graphs per core
NN = P * G       # nodes per core (8192)
K = 16           # edges per node
NH = 128
H = 6
NCORES = 8
NHYP = G * H     # hypernodes per core (384)

WNAMES = [
    ("watt2", [P, 2]), ("winner", [P, P]), ("binner", [P, 1]),
    ("wattm", [P, 2]), ("wattx", [P, 2]),
    ("wout1a", [P, P]), ("wout1b", [P, P]), ("bout1", [P, 1]),
    ("wout2", [P, P]), ("bout2", [P, 1]),
    ("f1m00", [P, P]), ("f1m10", [P, P]), ("f1m01", [P, P]), ("f1m11", [P, P]),
    ("b1m0", [P, 1]), ("b1m1", [P, 1]),
    ("f2m0", [P, P]), ("f2m1", [P, P]), ("b2m", [P, 1]),
    ("f1v00", [P, P]), ("f1v10", [P, P]), ("f1v01", [P, P]), ("f1v11", [P, P]),
    ("b1v0", [P, 1]), ("b1v1", [P, 1]),
    ("f2v0", [P, P]), ("f2v1", [P, P]), ("b2v", [P, 1]),
    ("clsmW", [P, 10]), ("clsmb", [10, 1]),
    ("clsvW", [P, 10]), ("clsvb", [10, 1]),
]

# Batcher odd-even merge sort network for 16 channels. Each layer:
# (channel-dim factors, lo index, hi index).
SORT_LAYERS = [
    ((8, 2), (slice(None), slice(0, 1)), (slice(None), slice(1, 2))),
    ((4, 4), (slice(None), slice(0, 2)), (slice(None), slice(2, 4))),
    ((4, 4), (slice(None), slice(1, 2)), (slice(None), slice(2, 3))),
    ((2, 8), (slice(None), slice(0, 4)), (slice(None), slice(4, 8))),
    ((2, 8), (slice(None), slice(2, 4)), (slice(None), slice(4, 6))),
    ((2, 4, 2), (slice(None), slice(0, 3), slice(1, 2)),
     (slice(None), slice(1, 4), slice(0, 1))),
    ((1, 16), (slice(None), slice(0, 8)), (slice(None), slice(8, 16))),
    ((1, 16), (slice(None), slice(4, 8)), (slice(None), slice(8, 12))),
    ((4, 4), (slice(0, 3), slice(2, 4)), (slice(1, 4), slice(0, 2))),
    ((8, 2), (slice(0, 7), slice(1, 2)), (slice(1, 8), slice(0, 1))),
]


def _chslice(ap, factors, idx):
    names = "abcd"[: len(factors)]
    pat = f"p ({' '.join(names)} g) -> p {' '.join(names)} g"
    v = ap.rearrange(pat, **{n: f for n, f in zip(names, factors)})
    return v[(slice(None),) + idx + (slice(None),)]


def build_nc():
    nc = bacc.Bacc("TRN2", target_bir_lowering=False, debug=False,
                   num_devices=NCORES)
    xT_d = nc.declare_dram_parameter("xT", [P, NN], F32, isOutput=False)
    col_d = nc.declare_dram_parameter("colr", [P, G * K], I32, isOutput=False)
    wd = {}
    for nm, shp in WNAMES:
        wd[nm] = nc.declare_dram_parameter(nm, shp, F32, isOutput=False)
    out_d = nc.declare_dram_parameter("out", [G, 148], F32, isOutput=True)

    with tile.TileContext(nc) as tc, ExitStack() as ctx:
        pp = ctx.enter_context(tc.tile_pool(name="persist", bufs=1))
        wk = ctx.enter_context(tc.tile_pool(name="work", bufs=1))
        pmm = ctx.enter_context(
            tc.tile_pool(name="psum", bufs=6, space="PSUM"))

        ps_ctr = [0]

        def ps(shape, dtype=F32):
            ps_ctr[0] += 1
            return pmm.tile(shape, dtype, tag="ps", name=f"pst{ps_ctr[0]}")

        # ---------------- weights / constants ----------------
        w = {}
        for nm, shp in WNAMES:
            t = pp.tile(shp, F32, tag=f"w_{nm}")
            nc.sync.dma_start(t[:], wd[nm].ap())
            w[nm] = t
        watt2_h = pp.tile([P, 2], FP16, tag="watt2h")
        nc.gpsimd.dma_start(watt2_h[:], wd["watt2"].ap())
        winner_h = pp.tile([P, P], FP16, tag="winnerh")
        nc.gpsimd.dma_start(winner_h[:], wd["winner"].ap())

        iota_row = pp.tile([P, P], I32, tag="iota_row")
        nc.gpsimd.iota(iota_row[:], pattern=[[1, P]], channel_multiplier=0)
        iota_part = pp.tile([P, 1], I32, tag="iota_part")
        nc.gpsimd.iota(iota_part[:], pattern=[[0, 1]], channel_multiplier=1)
        g128rep = pp.tile([P, K * G], I32, tag="g128rep")   # (k,g): 128*(g%8)
        nc.gpsimd.iota(g128rep[:], pattern=[[0, K], [0, 8], [P, 8]],
                       channel_multiplier=0)

        iota_rowf = pp.tile([P, P], F32, tag="iota_rowf")
        nc.vector.tensor_copy(iota_rowf[:], iota_row[:])
        iota_partf = pp.tile([P, 1], F32, tag="iota_partf")
        nc.vector.tensor_copy(iota_partf[:], iota_part[:])
        ident32 = pp.tile([P, P], F32, tag="ident32")
        nc.vector.tensor_scalar(ident32[:], iota_rowf[:], iota_partf[:, 0:1],
                                None, op0=OP.is_equal)
        ident16 = pp.tile([P, P], FP16, tag="ident16")
        nc.vector.tensor_copy(ident16[:], ident32[:])
        ones_col = pp.tile([1, P], F32, tag="ones_col")
        nc.vector.memset(ones_col[:], 1.0)
        ones16 = pp.tile([1, P], FP16, tag="ones16")
        nc.vector.memset(ones16[:], 1.0)
        cntinv = pp.tile([P, H * G], F32, tag="cntinv")     # (g, r)
        nc.vector.memset(cntinv[:], 1.0 / 21.0)
        nc.vector.memset(
            cntinv.rearrange("p (g r) -> p g r", g=G)[:, :, 0:2],
            1.0 / 22.0)

        # ---------------- x^T (fp16 cast) and col ----------------
        xT = pp.tile([P, NN], FP16, tag="xT")
        for c in range(8):
            sl = slice(c * 1024, (c + 1) * 1024)
            nc.gpsimd.dma_start(xT[:, sl], xT_d.ap()[:, sl])
        colr = pp.tile([P, G * K], I32, tag="colr")
        nc.sync.dma_start(colr[:], col_d.ap())

        # ---------------- stage A: al, ar, XW per graph ----------------
        al_sb = wk.tile([P, G], F32, tag="al")
        arc_sb = wk.tile([P, G], F32, tag="arc")
        xw16 = wk.tile([P, G * P], FP16, tag="xw16")
        for b in range(G // 4):
            ps_xw = ps([P, 512])
            ps_al = ps([P, 8])
            for i in range(4):
                g = b * 4 + i
                lhs = xT[:, g * P:(g + 1) * P]
                nc.tensor.matmul(ps_al[:, 2 * i:2 * i + 2], lhs, watt2_h[:])
                nc.tensor.matmul(ps_xw[:, i * P:(i + 1) * P], lhs,
                                 winner_h[:])
            alr = ps_al.rearrange("p (i t) -> p i t", i=4)
            nc.vector.tensor_copy(
                al_sb.rearrange("p (b i) -> p b i", b=G // 4)[:, b],
                alr[:, :, 0])
            nc.vector.tensor_copy(
                arc_sb.rearrange("p (b i) -> p b i", b=G // 4)[:, b],
                alr[:, :, 1])
            nc.scalar.copy(xw16[:, b * 512:(b + 1) * 512], ps_xw[:])

        # ar as fp16-rounded per-node f32 scalars (consistent with ar_all)
        arc16 = wk.tile([P, G], FP16, tag="arc16")
        nc.vector.tensor_copy(arc16[:], arc_sb[:])
        arcr = wk.tile([P, G], F32, tag="arcr")
        nc.vector.tensor_copy(arcr[:], arc16[:])
        al1h = wk.tile([P, G], FP16, tag="al1h")
        nc.vector.tensor_scalar_add(al1h[:], al_sb[:], 1.0)

        # ar replicated on all partitions: [p, (g*128+m)] fp16
        ar_flat = wk.tile([1, NN], FP16, tag="arflat")
        for c in range(NN // 512):
            ps_ar = ps([1, 512])
            nc.tensor.matmul(ps_ar[:], watt2_h[:, 1:2],
                             xT[:, c * 512:(c + 1) * 512])
            nc.vector.tensor_copy(ar_flat[:, c * 512:(c + 1) * 512],
                                  ps_ar[:])
        ar_all = wk.tile([P, NN], FP16, tag="arall")
        for c in range(NN // 512):
            ps_b = ps([P, 512])
            nc.tensor.matmul(ps_b[:], ones16[:],
                             ar_flat[:, c * 512:(c + 1) * 512])
            nc.scalar.copy(ar_all[:, c * 512:(c + 1) * 512], ps_b[:])

        # ---------------- stage C: dedup cols -> cnt matrix ----------------
        colf = wk.tile([P, K * G], F32, tag="colf")         # (k, g)
        colt = wk.tile([P, K * G], I32, tag="ndead")
        nc.vector.tensor_copy(colt[:],
                              colr.rearrange("p (g j) -> p j g", g=G))
        nc.vector.tensor_scalar(colt[:], colt[:], 127, None,
                                op0=OP.bitwise_and)
        nc.vector.tensor_copy(colf[:], colt[:])
        sorttmp = wk.tile([P, 8 * G], F32, tag="sorttmp")
        for factors, lo_i, hi_i in SORT_LAYERS:
            lo = _chslice(colf, factors, lo_i)
            hi = _chslice(colf, factors, hi_i)
            ext = tuple(len(range(*s.indices(f)))
                        for s, f in zip(lo_i, factors))
            npair = int(np.prod(ext))
            tmp = _chslice(sorttmp[:, 0:npair * G], ext,
                           tuple(slice(None) for _ in ext))
            nc.vector.tensor_tensor(tmp, lo, hi, op=OP.min)
            nc.vector.tensor_tensor(lo, lo, hi, op=OP.max)
            nc.vector.tensor_copy(hi, tmp)
        # eq channels with zero pad; dead_k = eq_{k+1}
        eqt = wk.tile([P, (K + 1) * G], F32, tag="eqt")
        nc.vector.memset(eqt[:, 0:G], 0.0)
        nc.vector.memset(eqt[:, K * G:], 0.0)
        nc.vector.tensor_tensor(eqt[:, G:K * G], colf[:, G:],
                                colf[:, 0:(K - 1) * G], op=OP.is_equal)
        # run counts: c_k = 1 + eq_k * c_{k-1}; last slot of run holds total
        cntc = wk.tile([P, K * G], F32, tag="cntc")
        nc.vector.memset(cntc[:, 0:G], 1.0)
        for k in range(1, K):
            nc.vector.tensor_mul(cntc[:, k * G:(k + 1) * G],
                                 eqt[:, k * G:(k + 1) * G],
                                 cntc[:, (k - 1) * G:k * G])
            nc.vector.tensor_scalar_add(cntc[:, k * G:(k + 1) * G],
                                        cntc[:, k * G:(k + 1) * G], 1.0)
        deadt = eqt[:, G:]
        ndead = wk.tile([P, K * G], F32, tag="ndead")
        nc.vector.tensor_scalar(ndead[:], deadt, -1.0, 1.0,
                                op0=OP.mult, op1=OP.add)
        rampf = wk.tile([P, K * G], F32, tag="rampf")
        nc.vector.tensor_copy(rampf[:], g128rep[:])
        # idx = dead ? -1 : col + 128*(g%8)  == (col+ramp+1)*ndead - 1
        idxf = wk.tile([P, K * G], F32, tag="idxf")
        nc.vector.tensor_add(idxf[:], colf[:], rampf[:])
        nc.vector.scalar_tensor_tensor(idxf[:], idxf[:], 1.0, ndead[:],
                                       op0=OP.add, op1=OP.mult)
        nc.vector.tensor_scalar_add(idxf[:], idxf[:], -1.0)
        sidx = wk.tile([P, G * K], I16, tag="sidx")
        sval = wk.tile([P, G * K], FP16, tag="sval")
        nc.vector.tensor_copy(sidx.rearrange("p (g k) -> p k g", g=G),
                              idxf.rearrange("p (k g) -> p k g", k=K))
        nc.vector.tensor_copy(sval.rearrange("p (g k) -> p k g", g=G),
                              cntc.rearrange("p (k g) -> p k g", k=K))
        cntd = wk.tile([P, G * P], FP16, tag="cntd")
        for wnd in range(8):
            nc.gpsimd.local_scatter(
                cntd[:, wnd * 1024:(wnd + 1) * 1024],
                sval[:, wnd * 128:(wnd + 1) * 128],
                sidx[:, wnd * 128:(wnd + 1) * 128],
                channels=P, num_elems=1024, num_idxs=128)

        # ---------------- stage D: zd1, S/N matmuls, tau ----------------
        zd1 = wk.tile([P, NN], FP16, tag="zd1")
        zd3 = zd1.rearrange("p (g m) -> p g m", g=G)
        al1b = al1h.rearrange("p (g o) -> p g o", o=1) \
            .broadcast_to([P, G, P])
        nc.vector.tensor_tensor(zd3, ar_all.rearrange("p (g m) -> p g m",
                                                      g=G), al1b, op=OP.add)
        nc.vector.tensor_scalar_max(zd1[:], zd1[:], 1.0)

        tau = wk.tile([P, G], F32, tag="tau")
        nz = wk.tile([P, G], F32, tag="nz")
        sz = wk.tile([P, G], F32, tag="sz")
        for hf in range(2):
            hsl0 = slice(hf * NN // 2, (hf + 1) * NN // 2)
            S_h = wk.tile([P, NN // 2], FP16, tag="Sh")
            N_h = wk.tile([P, NN // 2], FP16, tag="Nh")
            czh = wk.tile([P, NN // 2], FP16, tag="czh")
            nc.vector.tensor_mul(czh[:], cntd[:, hsl0], zd1[:, hsl0])
            Ch = wk.tile([P, NN // 2], FP16, tag="tq")
            for i in range(32):
                g = hf * 32 + i
                nc.vector.tensor_scalar(
                    Ch[:, i * P:(i + 1) * P], ar_all[:, g * P:(g + 1) * P],
                    arcr[:, g:g + 1], None, op0=OP.is_le)
            for b in range(8):
                gb = hf * 8 + b
                ps_t1 = ps([P, 512], FP16)
                ps_t2 = ps([P, 512], FP16)
                for i in range(4):
                    g = gb * 4 + i
                    bsl = slice((b * 4 + i) * P, (b * 4 + i + 1) * P)
                    nc.tensor.transpose(ps_t1[:, i * P:(i + 1) * P],
                                        cntd[:, g * P:(g + 1) * P],
                                        ident16[:])
                    nc.tensor.transpose(ps_t2[:, i * P:(i + 1) * P],
                                        czh[:, bsl], ident16[:])
                ctb = wk.tile([P, 512], FP16, tag="ctb", bufs=2)
                cztb = wk.tile([P, 512], FP16, tag="cztb", bufs=2)
                nc.scalar.copy(ctb[:], ps_t1[:])
                nc.scalar.copy(cztb[:], ps_t2[:])
                ps_s = ps([P, 512])
                ps_n = ps([P, 512])
                for i in range(4):
                    bsl = slice((b * 4 + i) * P, (b * 4 + i + 1) * P)
                    nc.tensor.matmul(ps_s[:, i * P:(i + 1) * P],
                                     cztb[:, i * P:(i + 1) * P], Ch[:, bsl])
                    nc.tensor.matmul(ps_n[:, i * P:(i + 1) * P],
                                     ctb[:, i * P:(i + 1) * P], Ch[:, bsl])
                nc.scalar.copy(S_h[:, b * 512:(b + 1) * 512], ps_s[:])
                nc.scalar.copy(N_h[:, b * 512:(b + 1) * 512], ps_n[:])
            # support test (division-free): m in support iff N*zd1 - S > -1
            hsl = slice(hf * NN // 2, (hf + 1) * NN // 2)
            gsl = slice(hf * 32, (hf + 1) * 32)
            t1 = wk.tile([P, NN // 2], FP16, tag="rq")
            nc.vector.tensor_mul(t1[:], N_h[:], zd1[:, hsl])
            nc.vector.tensor_sub(t1[:], t1[:], S_h[:])
            nc.vector.tensor_scalar(t1[:], t1[:], -1.0, None, op0=OP.is_gt)
            nc.vector.tensor_mul(t1[:], t1[:], cntd[:, hsl])
            nc.vector.tensor_reduce(
                nz[:, gsl], t1.rearrange("p (g m) -> p g m", g=32),
                axis=X, op=OP.add)
            nc.vector.tensor_mul(t1[:], t1[:], zd1[:, hsl])
            nc.vector.tensor_reduce(
                sz[:, gsl], t1.rearrange("p (g m) -> p g m", g=32),
                axis=X, op=OP.add)

        # tau = (SZ - 1) / NZ  (tiny division)
        rnz = wk.tile([P, G], F32, tag="rnz")
        nc.vector.reciprocal(rnz[:], nz[:])
        nc.vector.scalar_tensor_tensor(tau[:], sz[:], -1.0, rnz[:],
                                       op0=OP.add, op1=OP.mult)

        # ---------------- stage E: A dense, transpose, agg ----------------
        ntau16 = wk.tile([P, G], FP16, tag="ntau16")
        nc.vector.tensor_scalar_mul(ntau16[:], tau[:], -1.0)
        gam16 = wk.tile([P, G], FP16, tag="gam16")
        nc.vector.tensor_scalar(gam16[:], tau[:], -1.0, 1.0,
                                op0=OP.mult, op1=OP.add)
        nc.vector.tensor_scalar_max(gam16[:], gam16[:], 0.0)
        A = wk.tile([P, G * P], FP16, tag="arall")
        A3 = A.rearrange("p (g m) -> p g m", g=G)
        ntb = ntau16.rearrange("p (g o) -> p g o", o=1) \
            .broadcast_to([P, G, P])
        gmb = gam16.rearrange("p (g o) -> p g o", o=1) \
            .broadcast_to([P, G, P])
        nc.vector.tensor_tensor(A3, zd3, ntb, op=OP.add)
        nc.vector.tensor_tensor(A3, A3, gmb, op=OP.max)
        nc.vector.tensor_mul(A[:], A[:], cntd[:])

        hT = wk.tile([P, G * P], FP16, tag="cntd")
        for b in range(G // 4):
            ps_t = ps([P, 512], FP16)
            for i in range(4):
                g = b * 4 + i
                nc.tensor.transpose(ps_t[:, i * P:(i + 1) * P],
                                    A[:, g * P:(g + 1) * P], ident16[:])
            atb = wk.tile([P, 512], FP16, tag="atb", bufs=2)
            nc.scalar.copy(atb[:], ps_t[:])
            ps_a = ps([P, 512])
            for i in range(4):
                g = b * 4 + i
                nc.tensor.matmul(ps_a[:, i * P:(i + 1) * P],
                                 xw16[:, g * P:(g + 1) * P],
                                 atb[:, i * P:(i + 1) * P],
                                 start=True, stop=False)
                nc.tensor.matmul(ps_a[:, i * P:(i + 1) * P],
                                 xw16[:, g * P:(g + 1) * P],
                                 ident16[:], start=False, stop=True)
            nc.scalar.activation(hT[:, b * 512:(b + 1) * 512], ps_a[:],
                                 AF.Relu, bias=w["binner"][:, 0:1],
                                 scale=0.5)

        # ---------------- stage F: pooling ----------------
        xhm = wk.tile([P, NHYP], F32, tag="xw16")           # (g, r) mean
        xhx = wk.tile([P, NHYP], F32, tag="colf")           # (g, r) max
        # node order is sigma-permuted (host side): positions [0:44] are
        # hypernodes 0-1 (22 each), [44:128] are hypernodes 2-5 (21 each).
        hT_v = hT.rearrange("p (g n) -> p g n", g=G)
        seg_a = hT_v[:, :, 0:44].rearrange("p g (r kk) -> p g r kk", r=2)
        seg_b = hT_v[:, :, 44:128].rearrange("p g (r kk) -> p g r kk", r=4)
        xhm_v = xhm.rearrange("p (g r) -> p g r", g=G)
        xhx_v = xhx.rearrange("p (g r) -> p g r", g=G)
        nc.vector.tensor_reduce(xhm_v[:, :, 0:2], seg_a, axis=X, op=OP.add)
        nc.vector.tensor_reduce(xhm_v[:, :, 2:6], seg_b, axis=X, op=OP.add)
        nc.vector.tensor_mul(xhm[:], xhm[:], cntinv[:])
        nc.vector.tensor_reduce(xhx_v[:, :, 0:2], seg_a, axis=X, op=OP.max)
        nc.vector.tensor_reduce(xhx_v[:, :, 2:6], seg_b, axis=X, op=OP.max)

        # ---------------- stage G: outer attention ----------------
        ps_w = ps([2, NHYP])
        nc.tensor.matmul(ps_w[:], w["wattm"][:], xhm[:], start=True,
                         stop=False)
        nc.tensor.matmul(ps_w[:], w["wattx"][:], xhx[:], start=False,
                         stop=True)
        wlr2 = wk.tile([2, NHYP], F32, tag="wlr2")
        nc.vector.tensor_copy(wlr2[:], ps_w[:])
        wlr = wk.tile([G, 12], F32, tag="wlr")
        nc.sync.dma_start(wlr[:, 0:6], wlr2[0:1, :])
        nc.sync.dma_start(wlr[:, 6:12], wlr2[1:2, :])
        whm = wk.tile([G, 36], F32, tag="whm")
        whm_v = whm.rearrange("g (r s) -> g r s", r=H)
        for r in range(H):
            nc.vector.tensor_scalar(whm_v[:, r], wlr[:, 6:12],
                                    wlr[:, r:r + 1], None, op0=OP.add)
        wt = wk.tile([G, 36], F32, tag="wt36")
        wt_v = wt.rearrange("g (r s) -> g r s", r=H)
        nc.vector.tensor_scalar_min(wt[:], whm[:], 0.0)
        nc.vector.tensor_scalar_max(whm[:], whm[:], 0.0)
        nc.vector.scalar_tensor_tensor(whm[:], wt[:], 0.2, whm[:],
                                       op0=OP.mult, op1=OP.add)
        rmax = wk.tile([G, H], F32, tag="rmax")
        nc.vector.tensor_tensor(wt_v[:, :, 0:3], whm_v[:, :, 0:3],
                                whm_v[:, :, 3:6], op=OP.max)
        nc.vector.tensor_tensor(rmax[:], wt_v[:, :, 0], wt_v[:, :, 1],
                                op=OP.max)
        nc.vector.tensor_tensor(rmax[:], rmax[:], wt_v[:, :, 2], op=OP.max)
        for r in range(H):
            nc.vector.tensor_scalar(whm_v[:, r], whm_v[:, r],
                                    rmax[:, r:r + 1], None, op0=OP.subtract)
        nc.scalar.activation(whm[:], whm[:], AF.Exp)
        rsum = wk.tile([G, H], F32, tag="rsum")
        nc.vector.tensor_tensor(wt_v[:, :, 0:3], whm_v[:, :, 0:3],
                                whm_v[:, :, 3:6], op=OP.add)
        nc.vector.tensor_tensor(rsum[:], wt_v[:, :, 0], wt_v[:, :, 1],
                                op=OP.add)
        nc.vector.tensor_tensor(rsum[:], rsum[:], wt_v[:, :, 2], op=OP.add)
        nc.vector.reciprocal(rsum[:], rsum[:])
        for r in range(H):
            nc.vector.tensor_scalar(whm_v[:, r], whm_v[:, r],
                                    rsum[:, r:r + 1], None, op0=OP.mult)
        ahflat = wk.tile([1, G * 36], F32, tag="arflat")
        nc.sync.dma_start(ahflat[:], whm[:])
        ahrep = wk.tile([P, G * 36], F32, tag="zd1")
        for c in range(5):
            lo = c * 512
            n = min(512, G * 36 - lo)
            ps_b2 = ps([P, 512])
            nc.tensor.matmul(ps_b2[:, 0:n], ones_col[:], ahflat[:, lo:lo + n])
            nc.scalar.copy(ahrep[:, lo:lo + n], ps_b2[:, 0:n])
        ah_v = ahrep.rearrange("p (g q) -> p g q", g=G)

        def outer_gcn(xin_m, xin_x, wa, wb, bias, name):
            p1 = ps([P, NHYP])
            if xin_x is None:
                nc.tensor.matmul(p1[:], wa[:], xin_m[:])
            else:
                nc.tensor.matmul(p1[:], wa[:], xin_m[:], start=True,
                                 stop=False)
                nc.tensor.matmul(p1[:], wb[:], xin_x[:], start=False,
                                 stop=True)
            xwT = wk.tile([P, NHYP], F32, tag="xwT")
            nc.vector.tensor_copy(xwT[:], p1[:])
            agg = wk.tile([P, NHYP], F32, tag="agg")
            agg_v = agg.rearrange("p (g r) -> p g r", g=G)
            xw_v = xwT.rearrange("p (g s) -> p g s", g=G)
            tmpa = wk.tile([P, G], F32, tag="tmpa")
            for r in range(H):
                for s in range(H):
                    if s == 0:
                        nc.vector.tensor_mul(agg_v[:, :, r], xw_v[:, :, s],
                                             ah_v[:, :, r * H + s])
                    else:
                        nc.vector.tensor_mul(tmpa[:], xw_v[:, :, s],
                                             ah_v[:, :, r * H + s])
                        nc.vector.tensor_tensor(agg_v[:, :, r],
                                                agg_v[:, :, r], tmpa[:],
                                                op=OP.add)
            nc.vector.tensor_add(agg[:], agg[:], xwT[:])
            zT = wk.tile([P, NHYP], F32, tag="zT")
            nc.scalar.activation(zT[:], agg[:], AF.Relu, bias=bias[:, 0:1],
                                 scale=0.5)
            return zT

        z1h = outer_gcn(xhm, xhx, w["wout1a"], w["wout1b"], w["bout1"], "o1")
        z2h = outer_gcn(z1h, None, w["wout2"], None, w["bout2"], "o2")

        x1m = wk.tile([P, G], F32, tag="x1m")
        x1x = wk.tile([P, G], F32, tag="x1x")
        z2_v = z2h.rearrange("p (g r) -> p g r", g=G)
        nc.vector.tensor_reduce(x1m[:], z2_v, axis=X, op=OP.add)
        nc.vector.tensor_scalar_mul(x1m[:], x1m[:], 1.0 / H)
        nc.vector.tensor_reduce(x1x[:], z2_v, axis=X, op=OP.max)

        # ---------------- MLP heads ----------------
        def head(pfx, xm, xx):
            m1 = []
            for j in range(2):
                p2 = ps([P, G])
                nc.tensor.matmul(p2[:], w[f"f1{pfx}0{j}"][:], xm[:],
                                 start=True, stop=False)
                nc.tensor.matmul(p2[:], w[f"f1{pfx}1{j}"][:], xx[:],
                                 start=False, stop=True)
                t = wk.tile([P, G], F32, tag=f"m1{pfx}{j}")
                nc.scalar.activation(t[:], p2[:], AF.Relu,
                                     bias=w[f"b1{pfx}{j}"][:, 0:1])
                m1.append(t)
            p3 = ps([P, G])
            nc.tensor.matmul(p3[:], w[f"f2{pfx}0"][:], m1[0][:],
                             start=True, stop=False)
            nc.tensor.matmul(p3[:], w[f"f2{pfx}1"][:], m1[1][:],
                             start=False, stop=True)
            mT = wk.tile([P, G], F32, tag=f"mT{pfx}")
            nc.scalar.activation(mT[:], p3[:], AF.Relu,
                                 bias=w[f"b2{pfx}"][:, 0:1])
            p4 = ps([10, G])
            nc.tensor.matmul(p4[:], w[f"cls{pfx}W"][:], mT[:])
            o = wk.tile([10, G], F32, tag=f"o{pfx}")
            nc.vector.tensor_scalar(o[:], p4[:], w[f"cls{pfx}b"][:, 0:1],
                                    None, op0=OP.add)
            return mT, o

        mT, om = head("m", x1m, x1x)
        _, ov = head("v", x1m, x1x)

        # ---------------- outputs ----------------
        p5 = ps([G, P])
        nc.tensor.transpose(p5[:], mT[:], ident32[:])
        m_t = wk.tile([G, P], F32, tag="m_t")
        nc.vector.tensor_copy(m_t[:], p5[:])
        nc.sync.dma_start(out_d.ap()[:, 20:148], m_t[:])
        for o_ap, cols in ((om, slice(0, 10)), (ov, slice(10, 20))):
            p6 = ps([G, 10])
            nc.tensor.transpose(p6[:], o_ap[:], ident32[0:10, 0:10])
            o_t = wk.tile([G, 10], F32, tag="o_t")
            nc.vector.tensor_copy(o_t[:], p6[:])
            nc.sync.dma_start(out_d.ap()[:, cols], o_t[:])

    nc.compile()
    return nc


def prepare_shared(inputs):
    f32 = np.float32
    att = np.asarray(inputs["att_inner"], f32)
    atto = np.asarray(inputs["att_outer"], f32)
    sh = {
        "watt2": np.ascontiguousarray(np.stack([att[:P], att[P:]], axis=1)),
        "winner": np.ascontiguousarray(np.asarray(inputs["W_inner"], f32)),
        "binner": np.asarray(inputs["b_inner"], f32).reshape(P, 1).copy(),
        "wattm": np.ascontiguousarray(
            np.stack([atto[0:128], atto[256:384]], axis=1)),
        "wattx": np.ascontiguousarray(
            np.stack([atto[128:256], atto[384:512]], axis=1)),
        "wout1a": np.ascontiguousarray(np.asarray(inputs["W_out1"], f32)[:P]),
        "wout1b": np.ascontiguousarray(np.asarray(inputs["W_out1"], f32)[P:]),
        "bout1": np.asarray(inputs["b_out1"], f32).reshape(P, 1).copy(),
        "wout2": np.ascontiguousarray(np.asarray(inputs["W_out2"], f32)),
        "bout2": np.asarray(inputs["b_out2"], f32).reshape(P, 1).copy(),
        "clsmW": np.ascontiguousarray(np.asarray(inputs["clsm_W"], f32)),
        "clsmb": np.asarray(inputs["clsm_b"], f32).reshape(10, 1).copy(),
        "clsvW": np.ascontiguousarray(np.asarray(inputs["clsv_W"], f32)),
        "clsvb": np.asarray(inputs["clsv_b"], f32).reshape(10, 1).copy(),
    }
    for pfx in ("m", "v"):
        w1 = np.asarray(inputs[f"fc1{pfx}_W"], f32)
        b1 = np.asarray(inputs[f"fc1{pfx}_b"], f32)
        w2 = np.asarray(inputs[f"fc2{pfx}_W"], f32)
        for j in range(2):
            sh[f"f1{pfx}0{j}"] = np.ascontiguousarray(
                w1[0:P, j * P:(j + 1) * P])
            sh[f"f1{pfx}1{j}"] = np.ascontiguousarray(
                w1[P:2 * P, j * P:(j + 1) * P])
            sh[f"b1{pfx}{j}"] = b1[j * P:(j + 1) * P].reshape(P, 1).copy()
        sh[f"f2{pfx}0"] = np.ascontiguousarray(w2[0:P])
        sh[f"f2{pfx}1"] = np.ascontiguousarray(w2[P:2 * P])
        sh[f"b2{pfx}"] = np.asarray(
            inputs[f"fc2{pfx}_b"], f32).reshape(P, 1).copy()
    return sh


SIGMA = np.concatenate([np.arange(r, P, H) for r in range(H)])  # [128]
SIGMA_INV = np.argsort(SIGMA)


def make_in_maps(inputs):
    x = np.asarray(inputs["x"], np.float32)
    col = np.asarray(inputs["edge_index"], np.int32)[1]
    sh = prepare_shared(inputs)
    in_maps = []
    for c in range(NCORES):
        xs = x[c * NN:(c + 1) * NN].reshape(G, P, -1)[:, SIGMA, :]
        xT = np.ascontiguousarray(xs.reshape(NN, -1).T)
        cs = col[c * NN * K:(c + 1) * NN * K].reshape(G, P, K)
        cs = SIGMA_INV[cs % P][:, SIGMA, :]        # relabel + reorder rows
        colr = np.ascontiguousarray(
            cs.transpose(1, 0, 2).reshape(P, G * K)).astype(np.int32)
        in_maps.append({"xT": xT, "colr": colr, **sh})
    return in_maps


_NC = None


def _ensure_ntff_hook():
    """Register the axon NTFF profiling hook if the image's antenv lacks
    the axon_hooks module (needed for trace=True exec-time capture)."""
    import sys, types
    try:
        from antenv.axon_hooks import get_axon_ntff_profile_hook  # noqa
        return
    except ImportError:
        pass
    try:
        import antenv
        from trn_agent_boot.trn_boot import _ntff_profile_via_ctypes
        mod = types.ModuleType("antenv.axon_hooks")
        hook = _ntff_profile_via_ctypes("/opt/axon/libaxon_pjrt.so")
        mod._hook = hook
        mod.set_axon_ntff_profile_hook = lambda h: setattr(mod, "_hook", h)
        mod.get_axon_ntff_profile_hook = lambda: mod._hook
        sys.modules["antenv.axon_hooks"] = mod
        antenv.axon_hooks = mod
    except Exception as e:  # pragma: no cover
        print(f"ntff hook setup failed: {e}")


def kernel(**inputs):
    global _NC
    if _NC is None:
        _NC = build_nc()
    in_maps = make_in_maps(inputs)
    trace = os.environ.get("BHGNN_TRACE", "") not in ("", "0")
    if trace:
        _ensure_ntff_hook()
    res = run_bass_kernel_spmd(_NC, in_maps, core_ids=list(range(NCORES)),
                               trace=trace)
    full = np.concatenate([res.results[c]["out"] for c in range(NCORES)],
                          axis=0)
    if trace and res.exec_time_ns is not None:
        print(f"HW exec time: {res.exec_time_ns} ns")
    return (np.ascontiguousarray(full[:, 0:10]),
            np.ascontiguousarray(full[:, 10:20]),
            np.ascontiguousarray(full[:, 20:148]))


# revision 23
# speedup vs baseline: 2.2855x; 1.0646x over previous
"""BHGNN Trainium2 kernel (8 NeuronCores, graph-level data parallel).

Per core: 64 graphs x 128 nodes. The sparsemax attention is computed
densely, without any per-edge gather:
  - al[n], ar[n], XW from per-graph PE matmuls on fp16 x^T.
  - Duplicate edge columns are merged by a 16-channel Batcher sort on the
    column ids; GPSIMD local_scatter builds the dense count matrix
    cnt[n, m] per graph.
  - zd1[n, m] = max(ar[m] + al[n] + 1, 1) densely (PE broadcast of ar).
  - Sparsemax threshold via tau+1 = max_m (S[n,m]-1)/N[n,m], where
    S = (cnt*zd1) @ C, N = cnt @ C and C[m',m] = [ar(m') >= ar(m)] is a
    graph-global comparison matrix -> two per-graph PE matmuls.
  - A = cnt * max(zd1 - tau1, max(1-tau1, 0)); agg = (A + I) @ XW via PE
    (identity accumulated in PSUM); h = relu(0.5*agg + b).
  - Pooling (strided DVE reduces) -> outer softmax attention, two tiny
    GCNs, readout, MLP heads in f32.
"""
import os
from contextlib import ExitStack

import numpy as np

import concourse.bass as bass
import concourse.mybir as mybir
import concourse.bacc as bacc
import concourse.tile as tile
from concourse.bass_utils import run_bass_kernel_spmd

FP16 = mybir.dt.float16
F32 = mybir.dt.float32
I32 = mybir.dt.int32
I16 = mybir.dt.int16
AF = mybir.ActivationFunctionType
OP = mybir.AluOpType
X = mybir.AxisListType.X

P = 128          # partitions = nodes per graph
G = 64           # graphs per core
NN = P * G       # nodes per core (8192)
K = 16           # edges per node
NH = 128
H = 6
NCORES = 8
NHYP = G * H     # hypernodes per core (384)

WNAMES = [
    ("watt2", [P, 2]), ("winner", [P, P]), ("binner", [P, 1]),
    ("wattm", [P, 2]), ("wattx", [P, 2]),
    ("wout1a", [P, P]), ("wout1b", [P, P]), ("bout1", [P, 1]),
    ("wout2", [P, P]), ("bout2", [P, 1]),
    ("f1m00", [P, P]), ("f1m10", [P, P]), ("f1m01", [P, P]), ("f1m11", [P, P]),
    ("b1m0", [P, 1]), ("b1m1", [P, 1]),
    ("f2m0", [P, P]), ("f2m1", [P, P]), ("b2m", [P, 1]),
    ("f1v00", [P, P]), ("f1v10", [P, P]), ("f1v01", [P, P]), ("f1v11", [P, P]),
    ("b1v0", [P, 1]), ("b1v1", [P, 1]),
    ("f2v0", [P, P]), ("f2v1", [P, P]), ("b2v", [P, 1]),
    ("clsmW", [P, 10]), ("clsmb", [10, 1]),
    ("clsvW", [P, 10]), ("clsvb", [10, 1]),
]

# Batcher odd-even merge sort network for 16 channels. Each layer:
# (channel-dim factors, lo index, hi index).
SORT_LAYERS = [
    ((8, 2), (slice(None), slice(0, 1)), (slice(None), slice(1, 2))),
    ((4, 4), (slice(None), slice(0, 2)), (slice(None), slice(2, 4))),
    ((4, 4), (slice(None), slice(1, 2)), (slice(None), slice(2, 3))),
    ((2, 8), (slice(None), slice(0, 4)), (slice(None), slice(4, 8))),
    ((2, 8), (slice(None), slice(2, 4)), (slice(None), slice(4, 6))),
    ((2, 4, 2), (slice(None), slice(0, 3), slice(1, 2)),
     (slice(None), slice(1, 4), slice(0, 1))),
    ((1, 16), (slice(None), slice(0, 8)), (slice(None), slice(8, 16))),
    ((1, 16), (slice(None), slice(4, 8)), (slice(None), slice(8, 12))),
    ((4, 4), (slice(0, 3), slice(2, 4)), (slice(1, 4), slice(0, 2))),
    ((8, 2), (slice(0, 7), slice(1, 2)), (slice(1, 8), slice(0, 1))),
]


def _chslice(ap, factors, idx):
    names = "abcd"[: len(factors)]
    pat = f"p ({' '.join(names)} g) -> p {' '.join(names)} g"
    v = ap.rearrange(pat, **{n: f for n, f in zip(names, factors)})
    return v[(slice(None),) + idx + (slice(None),)]


def build_nc():
    nc = bacc.Bacc("TRN2", target_bir_lowering=False, debug=False,
                   num_devices=NCORES)
    xT_d = nc.declare_dram_parameter("xT", [P, NN], F32, isOutput=False)
    col_d = nc.declare_dram_parameter("colr", [P, G * K], I32, isOutput=False)
    wd = {}
    for nm, shp in WNAMES:
        wd[nm] = nc.declare_dram_parameter(nm, shp, F32, isOutput=False)
    out_d = nc.declare_dram_parameter("out", [G, 148], F32, isOutput=True)

    with tile.TileContext(nc) as tc, ExitStack() as ctx:
        pp = ctx.enter_context(tc.tile_pool(name="persist", bufs=1))
        wk = ctx.enter_context(tc.tile_pool(name="work", bufs=1))
        pmm = ctx.enter_context(
            tc.tile_pool(name="psum", bufs=6, space="PSUM"))

        ps_ctr = [0]

        def ps(shape, dtype=F32):
            ps_ctr[0] += 1
            return pmm.tile(shape, dtype, tag="ps", name=f"pst{ps_ctr[0]}")

        # ---------------- weights / constants ----------------
        w = {}
        for nm, shp in WNAMES:
            t = pp.tile(shp, F32, tag=f"w_{nm}")
            nc.sync.dma_start(t[:], wd[nm].ap())
            w[nm] = t
        watt2_h = pp.tile([P, 2], FP16, tag="watt2h")
        nc.gpsimd.dma_start(watt2_h[:], wd["watt2"].ap())
        winner_h = pp.tile([P, P], FP16, tag="winnerh")
        nc.gpsimd.dma_start(winner_h[:], wd["winner"].ap())

        iota_row = pp.tile([P, P], I32, tag="iota_row")
        nc.gpsimd.iota(iota_row[:], pattern=[[1, P]], channel_multiplier=0)
        iota_part = pp.tile([P, 1], I32, tag="iota_part")
        nc.gpsimd.iota(iota_part[:], pattern=[[0, 1]], channel_multiplier=1)
        g128rep = pp.tile([P, K * G], I32, tag="g128rep")   # (k,g): 128*(g%8)
        nc.gpsimd.iota(g128rep[:], pattern=[[0, K], [0, 8], [P, 8]],
                       channel_multiplier=0)

        iota_rowf = pp.tile([P, P], F32, tag="iota_rowf")
        nc.vector.tensor_copy(iota_rowf[:], iota_row[:])
        iota_partf = pp.tile([P, 1], F32, tag="iota_partf")
        nc.vector.tensor_copy(iota_partf[:], iota_part[:])
        ident32 = pp.tile([P, P], F32, tag="ident32")
        nc.vector.tensor_scalar(ident32[:], iota_rowf[:], iota_partf[:, 0:1],
                                None, op0=OP.is_equal)
        ident16 = pp.tile([P, P], FP16, tag="ident16")
        nc.vector.tensor_copy(ident16[:], ident32[:])
        ones_col = pp.tile([1, P], F32, tag="ones_col")
        nc.vector.memset(ones_col[:], 1.0)
        ones16 = pp.tile([1, P], FP16, tag="ones16")
        nc.vector.memset(ones16[:], 1.0)
        cntinv = pp.tile([P, H * G], F32, tag="cntinv")     # (g, r)
        nc.vector.memset(cntinv[:], 1.0 / 21.0)
        nc.vector.memset(
            cntinv.rearrange("p (g r) -> p g r", g=G)[:, :, 0:2],
            1.0 / 22.0)

        # ---------------- x^T (fp16 cast) and col ----------------
        xT = pp.tile([P, NN], FP16, tag="xT")
        for c in range(8):
            sl = slice(c * 1024, (c + 1) * 1024)
            nc.gpsimd.dma_start(xT[:, sl], xT_d.ap()[:, sl])
        colr = pp.tile([P, G * K], I32, tag="colr")
        nc.sync.dma_start(colr[:], col_d.ap())

        # ---------------- stage A: al, ar, XW per graph ----------------
        al_sb = wk.tile([P, G], F32, tag="al")
        arc_sb = wk.tile([P, G], F32, tag="arc")
        xw16 = wk.tile([P, G * P], FP16, tag="xw16")
        for b in range(G // 4):
            ps_xw = ps([P, 512])
            ps_al = ps([P, 8])
            for i in range(4):
                g = b * 4 + i
                lhs = xT[:, g * P:(g + 1) * P]
                nc.tensor.matmul(ps_al[:, 2 * i:2 * i + 2], lhs, watt2_h[:])
                nc.tensor.matmul(ps_xw[:, i * P:(i + 1) * P], lhs,
                                 winner_h[:])
            alr = ps_al.rearrange("p (i t) -> p i t", i=4)
            nc.vector.tensor_copy(
                al_sb.rearrange("p (b i) -> p b i", b=G // 4)[:, b],
                alr[:, :, 0])
            nc.vector.tensor_copy(
                arc_sb.rearrange("p (b i) -> p b i", b=G // 4)[:, b],
                alr[:, :, 1])
            nc.scalar.copy(xw16[:, b * 512:(b + 1) * 512], ps_xw[:])

        # ar as fp16-rounded per-node f32 scalars (consistent with ar_all)
        arc16 = wk.tile([P, G], FP16, tag="arc16")
        nc.vector.tensor_copy(arc16[:], arc_sb[:])
        arcr = wk.tile([P, G], F32, tag="arcr")
        nc.vector.tensor_copy(arcr[:], arc16[:])
        al1h = wk.tile([P, G], FP16, tag="al1h")
        nc.vector.tensor_scalar_add(al1h[:], al_sb[:], 1.0)

        # ar replicated on all partitions: [p, (g*128+m)] fp16
        ar_flat = wk.tile([1, NN], FP16, tag="arflat")
        for c in range(NN // 512):
            ps_ar = ps([1, 512])
            nc.tensor.matmul(ps_ar[:], watt2_h[:, 1:2],
                             xT[:, c * 512:(c + 1) * 512])
            nc.vector.tensor_copy(ar_flat[:, c * 512:(c + 1) * 512],
                                  ps_ar[:])
        ar_all = wk.tile([P, NN], FP16, tag="arall")
        for c in range(NN // 512):
            ps_b = ps([P, 512])
            nc.tensor.matmul(ps_b[:], ones16[:],
                             ar_flat[:, c * 512:(c + 1) * 512])
            nc.scalar.copy(ar_all[:, c * 512:(c + 1) * 512], ps_b[:])

        # ---------------- stage C: dedup cols -> cnt matrix ----------------
        colf = wk.tile([P, K * G], FP16, tag="colf")         # (k, g)
        colt = wk.tile([P, K * G], I32, tag="ndead")
        nc.vector.tensor_copy(colt[:],
                              colr.rearrange("p (g j) -> p j g", g=G))
        nc.vector.tensor_scalar(colt[:], colt[:], 127, None,
                                op0=OP.bitwise_and)
        nc.vector.tensor_copy(colf[:], colt[:])
        sorttmp = wk.tile([P, 8 * G], FP16, tag="sorttmp")
        for factors, lo_i, hi_i in SORT_LAYERS:
            lo = _chslice(colf, factors, lo_i)
            hi = _chslice(colf, factors, hi_i)
            ext = tuple(len(range(*s.indices(f)))
                        for s, f in zip(lo_i, factors))
            npair = int(np.prod(ext))
            tmp = _chslice(sorttmp[:, 0:npair * G], ext,
                           tuple(slice(None) for _ in ext))
            nc.vector.tensor_tensor(tmp, lo, hi, op=OP.min)
            nc.vector.tensor_tensor(lo, lo, hi, op=OP.max)
            nc.vector.tensor_copy(hi, tmp)
        # eq channels with zero pad; dead_k = eq_{k+1}
        eqt = wk.tile([P, (K + 1) * G], FP16, tag="eqt")
        nc.vector.memset(eqt[:, 0:G], 0.0)
        nc.vector.memset(eqt[:, K * G:], 0.0)
        nc.vector.tensor_tensor(eqt[:, G:K * G], colf[:, G:],
                                colf[:, 0:(K - 1) * G], op=OP.is_equal)
        # run counts: c_k = 1 + eq_k * c_{k-1}; last slot of run holds total
        cntc = wk.tile([P, K * G], FP16, tag="cntc")
        nc.vector.memset(cntc[:, 0:G], 1.0)
        for k in range(1, K):
            nc.vector.tensor_mul(cntc[:, k * G:(k + 1) * G],
                                 eqt[:, k * G:(k + 1) * G],
                                 cntc[:, (k - 1) * G:k * G])
            nc.vector.tensor_scalar_add(cntc[:, k * G:(k + 1) * G],
                                        cntc[:, k * G:(k + 1) * G], 1.0)
        deadt = eqt[:, G:]
        ndead = wk.tile([P, K * G], FP16, tag="ndead")
        nc.vector.tensor_scalar(ndead[:], deadt, -1.0, 1.0,
                                op0=OP.mult, op1=OP.add)
        rampf = wk.tile([P, K * G], FP16, tag="rampf")
        nc.vector.tensor_copy(rampf[:], g128rep[:])
        # idx = dead ? -1 : col + 128*(g%8)  == (col+ramp+1)*ndead - 1
        idxf = wk.tile([P, K * G], FP16, tag="idxf")
        nc.vector.tensor_add(idxf[:], colf[:], rampf[:])
        nc.vector.scalar_tensor_tensor(idxf[:], idxf[:], 1.0, ndead[:],
                                       op0=OP.add, op1=OP.mult)
        nc.vector.tensor_scalar_add(idxf[:], idxf[:], -1.0)
        sidx = wk.tile([P, G * K], I16, tag="sidx")
        sval = wk.tile([P, G * K], FP16, tag="sval")
        nc.vector.tensor_copy(sidx.rearrange("p (g k) -> p k g", g=G),
                              idxf.rearrange("p (k g) -> p k g", k=K))
        nc.vector.tensor_copy(sval.rearrange("p (g k) -> p k g", g=G),
                              cntc.rearrange("p (k g) -> p k g", k=K))
        cntd = wk.tile([P, G * P], FP16, tag="cntd")
        for wnd in range(8):
            nc.gpsimd.local_scatter(
                cntd[:, wnd * 1024:(wnd + 1) * 1024],
                sval[:, wnd * 128:(wnd + 1) * 128],
                sidx[:, wnd * 128:(wnd + 1) * 128],
                channels=P, num_elems=1024, num_idxs=128)

        # ---------------- stage D: zd1, S/N matmuls, tau ----------------
        zd1 = wk.tile([P, NN], FP16, tag="zd1")
        zd3 = zd1.rearrange("p (g m) -> p g m", g=G)
        al1b = al1h.rearrange("p (g o) -> p g o", o=1) \
            .broadcast_to([P, G, P])
        nc.vector.tensor_tensor(zd3, ar_all.rearrange("p (g m) -> p g m",
                                                      g=G), al1b, op=OP.add)
        nc.vector.tensor_scalar_max(zd1[:], zd1[:], 1.0)

        tau = wk.tile([P, G], F32, tag="tau")
        nz = wk.tile([P, G], F32, tag="nz")
        sz = wk.tile([P, G], F32, tag="sz")
        NQ = NN // 4
        for qt in range(4):
            qsl = slice(qt * NQ, (qt + 1) * NQ)
            gsl = slice(qt * 16, (qt + 1) * 16)
            S_h = wk.tile([P, NQ], FP16, tag="Sh", bufs=2)
            N_h = wk.tile([P, NQ], FP16, tag="Nh", bufs=2)
            czh = wk.tile([P, NQ], FP16, tag="czh", bufs=2)
            nc.vector.tensor_mul(czh[:], cntd[:, qsl], zd1[:, qsl])
            Ch = wk.tile([P, NQ], FP16, tag="tq", bufs=2)
            for i in range(16):
                g = qt * 16 + i
                nc.vector.tensor_scalar(
                    Ch[:, i * P:(i + 1) * P], ar_all[:, g * P:(g + 1) * P],
                    arcr[:, g:g + 1], None, op0=OP.is_le)
            for b in range(4):
                gb = qt * 4 + b
                ps_t1 = ps([P, 512], FP16)
                ps_t2 = ps([P, 512], FP16)
                for i in range(4):
                    g = gb * 4 + i
                    bsl = slice((b * 4 + i) * P, (b * 4 + i + 1) * P)
                    nc.tensor.transpose(ps_t1[:, i * P:(i + 1) * P],
                                        cntd[:, g * P:(g + 1) * P],
                                        ident16[:])
                    nc.tensor.transpose(ps_t2[:, i * P:(i + 1) * P],
                                        czh[:, bsl], ident16[:])
                ctb = wk.tile([P, 512], FP16, tag="ctb", bufs=2)
                cztb = wk.tile([P, 512], FP16, tag="cztb", bufs=2)
                nc.scalar.copy(ctb[:], ps_t1[:])
                nc.scalar.copy(cztb[:], ps_t2[:])
                ps_s = ps([P, 512])
                ps_n = ps([P, 512])
                for i in range(4):
                    bsl = slice((b * 4 + i) * P, (b * 4 + i + 1) * P)
                    nc.tensor.matmul(ps_s[:, i * P:(i + 1) * P],
                                     cztb[:, i * P:(i + 1) * P], Ch[:, bsl])
                    nc.tensor.matmul(ps_n[:, i * P:(i + 1) * P],
                                     ctb[:, i * P:(i + 1) * P], Ch[:, bsl])
                nc.scalar.copy(S_h[:, b * 512:(b + 1) * 512], ps_s[:])
                nc.scalar.copy(N_h[:, b * 512:(b + 1) * 512], ps_n[:])
            # support test (division-free): m in support iff N*zd1 - S > -1
            t1 = wk.tile([P, NQ], FP16, tag="rq", bufs=2)
            nc.vector.tensor_mul(t1[:], N_h[:], zd1[:, qsl])
            nc.vector.tensor_sub(t1[:], t1[:], S_h[:])
            nc.vector.tensor_scalar(t1[:], t1[:], -1.0, None, op0=OP.is_gt)
            nc.vector.tensor_mul(t1[:], t1[:], cntd[:, qsl])
            nc.vector.tensor_reduce(
                nz[:, gsl], t1.rearrange("p (g m) -> p g m", g=16),
                axis=X, op=OP.add)
            nc.vector.tensor_mul(t1[:], t1[:], zd1[:, qsl])
            nc.vector.tensor_reduce(
                sz[:, gsl], t1.rearrange("p (g m) -> p g m", g=16),
                axis=X, op=OP.add)
        # tau = (SZ - 1) / NZ  (tiny division)
        rnz = wk.tile([P, G], F32, tag="rnz")
        nc.vector.reciprocal(rnz[:], nz[:])
        nc.vector.scalar_tensor_tensor(tau[:], sz[:], -1.0, rnz[:],
                                       op0=OP.add, op1=OP.mult)

        # ---------------- stage E: A dense, transpose, agg ----------------
        ntau16 = wk.tile([P, G], FP16, tag="ntau16")
        nc.vector.tensor_scalar_mul(ntau16[:], tau[:], -1.0)
        gam16 = wk.tile([P, G], FP16, tag="gam16")
        nc.vector.tensor_scalar(gam16[:], tau[:], -1.0, 1.0,
                                op0=OP.mult, op1=OP.add)
        nc.vector.tensor_scalar_max(gam16[:], gam16[:], 0.0)
        A = wk.tile([P, G * P], FP16, tag="arall")
        A3 = A.rearrange("p (g m) -> p g m", g=G)
        ntb = ntau16.rearrange("p (g o) -> p g o", o=1) \
            .broadcast_to([P, G, P])
        gmb = gam16.rearrange("p (g o) -> p g o", o=1) \
            .broadcast_to([P, G, P])
        nc.vector.tensor_tensor(A3, zd3, ntb, op=OP.add)
        nc.vector.tensor_tensor(A3, A3, gmb, op=OP.max)
        nc.vector.tensor_mul(A[:], A[:], cntd[:])

        hT = wk.tile([P, G * P], FP16, tag="cntd")
        for b in range(G // 4):
            ps_t = ps([P, 512], FP16)
            for i in range(4):
                g = b * 4 + i
                nc.tensor.transpose(ps_t[:, i * P:(i + 1) * P],
                                    A[:, g * P:(g + 1) * P], ident16[:])
            atb = wk.tile([P, 512], FP16, tag="atb", bufs=2)
            nc.scalar.copy(atb[:], ps_t[:])
            ps_a = ps([P, 512])
            for i in range(4):
                g = b * 4 + i
                nc.tensor.matmul(ps_a[:, i * P:(i + 1) * P],
                                 xw16[:, g * P:(g + 1) * P],
                                 atb[:, i * P:(i + 1) * P],
                                 start=True, stop=False)
                nc.tensor.matmul(ps_a[:, i * P:(i + 1) * P],
                                 xw16[:, g * P:(g + 1) * P],
                                 ident16[:], start=False, stop=True)
            nc.scalar.activation(hT[:, b * 512:(b + 1) * 512], ps_a[:],
                                 AF.Relu, bias=w["binner"][:, 0:1],
                                 scale=0.5)

        # ---------------- stage F: pooling ----------------
        xhm = wk.tile([P, NHYP], F32, tag="xw16")           # (g, r) mean
        xhx = wk.tile([P, NHYP], F32, tag="colf")           # (g, r) max
        # node order is sigma-permuted (host side): positions [0:44] are
        # hypernodes 0-1 (22 each), [44:128] are hypernodes 2-5 (21 each).
        hT_v = hT.rearrange("p (g n) -> p g n", g=G)
        seg_a = hT_v[:, :, 0:44].rearrange("p g (r kk) -> p g r kk", r=2)
        seg_b = hT_v[:, :, 44:128].rearrange("p g (r kk) -> p g r kk", r=4)
        xhm_v = xhm.rearrange("p (g r) -> p g r", g=G)
        xhx_v = xhx.rearrange("p (g r) -> p g r", g=G)
        nc.vector.tensor_reduce(xhm_v[:, :, 0:2], seg_a, axis=X, op=OP.add)
        nc.vector.tensor_reduce(xhm_v[:, :, 2:6], seg_b, axis=X, op=OP.add)
        nc.vector.tensor_mul(xhm[:], xhm[:], cntinv[:])
        nc.vector.tensor_reduce(xhx_v[:, :, 0:2], seg_a, axis=X, op=OP.max)
        nc.vector.tensor_reduce(xhx_v[:, :, 2:6], seg_b, axis=X, op=OP.max)

        # ---------------- stage G: outer attention ----------------
        ps_w = ps([2, NHYP])
        nc.tensor.matmul(ps_w[:], w["wattm"][:], xhm[:], start=True,
                         stop=False)
        nc.tensor.matmul(ps_w[:], w["wattx"][:], xhx[:], start=False,
                         stop=True)
        wlr2 = wk.tile([2, NHYP], F32, tag="wlr2")
        nc.vector.tensor_copy(wlr2[:], ps_w[:])
        wlr = wk.tile([G, 12], F32, tag="wlr")
        nc.sync.dma_start(wlr[:, 0:6], wlr2[0:1, :])
        nc.sync.dma_start(wlr[:, 6:12], wlr2[1:2, :])
        whm = wk.tile([G, 36], F32, tag="whm")
        whm_v = whm.rearrange("g (r s) -> g r s", r=H)
        for r in range(H):
            nc.vector.tensor_scalar(whm_v[:, r], wlr[:, 6:12],
                                    wlr[:, r:r + 1], None, op0=OP.add)
        wt = wk.tile([G, 36], F32, tag="wt36")
        wt_v = wt.rearrange("g (r s) -> g r s", r=H)
        nc.vector.tensor_scalar_min(wt[:], whm[:], 0.0)
        nc.vector.tensor_scalar_max(whm[:], whm[:], 0.0)
        nc.vector.scalar_tensor_tensor(whm[:], wt[:], 0.2, whm[:],
                                       op0=OP.mult, op1=OP.add)
        rmax = wk.tile([G, H], F32, tag="rmax")
        nc.vector.tensor_tensor(wt_v[:, :, 0:3], whm_v[:, :, 0:3],
                                whm_v[:, :, 3:6], op=OP.max)
        nc.vector.tensor_tensor(rmax[:], wt_v[:, :, 0], wt_v[:, :, 1],
                                op=OP.max)
        nc.vector.tensor_tensor(rmax[:], rmax[:], wt_v[:, :, 2], op=OP.max)
        for r in range(H):
            nc.vector.tensor_scalar(whm_v[:, r], whm_v[:, r],
                                    rmax[:, r:r + 1], None, op0=OP.subtract)
        nc.scalar.activation(whm[:], whm[:], AF.Exp)
        rsum = wk.tile([G, H], F32, tag="rsum")
        nc.vector.tensor_tensor(wt_v[:, :, 0:3], whm_v[:, :, 0:3],
                                whm_v[:, :, 3:6], op=OP.add)
        nc.vector.tensor_tensor(rsum[:], wt_v[:, :, 0], wt_v[:, :, 1],
                                op=OP.add)
        nc.vector.tensor_tensor(rsum[:], rsum[:], wt_v[:, :, 2], op=OP.add)
        nc.vector.reciprocal(rsum[:], rsum[:])
        for r in range(H):
            nc.vector.tensor_scalar(whm_v[:, r], whm_v[:, r],
                                    rsum[:, r:r + 1], None, op0=OP.mult)
        ahflat = wk.tile([1, G * 36], F32, tag="arflat")
        nc.sync.dma_start(ahflat[:], whm[:])
        ahrep = wk.tile([P, G * 36], F32, tag="zd1")
        for c in range(5):
            lo = c * 512
            n = min(512, G * 36 - lo)
            ps_b2 = ps([P, 512])
            nc.tensor.matmul(ps_b2[:, 0:n], ones_col[:], ahflat[:, lo:lo + n])
            nc.scalar.copy(ahrep[:, lo:lo + n], ps_b2[:, 0:n])
        ah_v = ahrep.rearrange("p (g q) -> p g q", g=G)

        def outer_gcn(xin_m, xin_x, wa, wb, bias, name):
            p1 = ps([P, NHYP])
            if xin_x is None:
                nc.tensor.matmul(p1[:], wa[:], xin_m[:])
            else:
                nc.tensor.matmul(p1[:], wa[:], xin_m[:], start=True,
                                 stop=False)
                nc.tensor.matmul(p1[:], wb[:], xin_x[:], start=False,
                                 stop=True)
            xwT = wk.tile([P, NHYP], F32, tag="xwT")
            nc.vector.tensor_copy(xwT[:], p1[:])
            agg = wk.tile([P, NHYP], F32, tag="agg")
            agg_v = agg.rearrange("p (g r) -> p g r", g=G)
            xw_v = xwT.rearrange("p (g s) -> p g s", g=G)
            tmpa = wk.tile([P, G], F32, tag="tmpa")
            for r in range(H):
                for s in range(H):
                    if s == 0:
                        nc.vector.tensor_mul(agg_v[:, :, r], xw_v[:, :, s],
                                             ah_v[:, :, r * H + s])
                    else:
                        nc.vector.tensor_mul(tmpa[:], xw_v[:, :, s],
                                             ah_v[:, :, r * H + s])
                        nc.vector.tensor_tensor(agg_v[:, :, r],
                                                agg_v[:, :, r], tmpa[:],
                                                op=OP.add)
            nc.vector.tensor_add(agg[:], agg[:], xwT[:])
            zT = wk.tile([P, NHYP], F32, tag="zT")
            nc.scalar.activation(zT[:], agg[:], AF.Relu, bias=bias[:, 0:1],
                                 scale=0.5)
            return zT

        z1h = outer_gcn(xhm, xhx, w["wout1a"], w["wout1b"], w["bout1"], "o1")
        z2h = outer_gcn(z1h, None, w["wout2"], None, w["bout2"], "o2")

        x1m = wk.tile([P, G], F32, tag="x1m")
        x1x = wk.tile([P, G], F32, tag="x1x")
        z2_v = z2h.rearrange("p (g r) -> p g r", g=G)
        nc.vector.tensor_reduce(x1m[:], z2_v, axis=X, op=OP.add)
        nc.vector.tensor_scalar_mul(x1m[:], x1m[:], 1.0 / H)
        nc.vector.tensor_reduce(x1x[:], z2_v, axis=X, op=OP.max)

        # ---------------- MLP heads ----------------
        def head(pfx, xm, xx):
            m1 = []
            for j in range(2):
                p2 = ps([P, G])
                nc.tensor.matmul(p2[:], w[f"f1{pfx}0{j}"][:], xm[:],
                                 start=True, stop=False)
                nc.tensor.matmul(p2[:], w[f"f1{pfx}1{j}"][:], xx[:],
                                 start=False, stop=True)
                t = wk.tile([P, G], F32, tag=f"m1{pfx}{j}")
                nc.scalar.activation(t[:], p2[:], AF.Relu,
                                     bias=w[f"b1{pfx}{j}"][:, 0:1])
                m1.append(t)
            p3 = ps([P, G])
            nc.tensor.matmul(p3[:], w[f"f2{pfx}0"][:], m1[0][:],
                             start=True, stop=False)
            nc.tensor.matmul(p3[:], w[f"f2{pfx}1"][:], m1[1][:],
                             start=False, stop=True)
            mT = wk.tile([P, G], F32, tag=f"mT{pfx}")
            nc.scalar.activation(mT[:], p3[:], AF.Relu,
                                 bias=w[f"b2{pfx}"][:, 0:1])
            p4 = ps([10, G])
            nc.tensor.matmul(p4[:], w[f"cls{pfx}W"][:], mT[:])
            o = wk.tile([10, G], F32, tag=f"o{pfx}")
            nc.vector.tensor_scalar(o[:], p4[:], w[f"cls{pfx}b"][:, 0:1],
                                    None, op0=OP.add)
            return mT, o

        mT, om = head("m", x1m, x1x)
        _, ov = head("v", x1m, x1x)

        # ---------------- outputs ----------------
        p5 = ps([G, P])
        nc.tensor.transpose(p5[:], mT[:], ident32[:])
        m_t = wk.tile([G, P], F32, tag="m_t")
        nc.vector.tensor_copy(m_t[:], p5[:])
        nc.sync.dma_start(out_d.ap()[:, 20:148], m_t[:])
        for o_ap, cols in ((om, slice(0, 10)), (ov, slice(10, 20))):
            p6 = ps([G, 10])
            nc.tensor.transpose(p6[:], o_ap[:], ident32[0:10, 0:10])
            o_t = wk.tile([G, 10], F32, tag="o_t")
            nc.vector.tensor_copy(o_t[:], p6[:])
            nc.sync.dma_start(out_d.ap()[:, cols], o_t[:])

    nc.compile()
    return nc


def prepare_shared(inputs):
    f32 = np.float32
    att = np.asarray(inputs["att_inner"], f32)
    atto = np.asarray(inputs["att_outer"], f32)
    sh = {
        "watt2": np.ascontiguousarray(np.stack([att[:P], att[P:]], axis=1)),
        "winner": np.ascontiguousarray(np.asarray(inputs["W_inner"], f32)),
        "binner": np.asarray(inputs["b_inner"], f32).reshape(P, 1).copy(),
        "wattm": np.ascontiguousarray(
            np.stack([atto[0:128], atto[256:384]], axis=1)),
        "wattx": np.ascontiguousarray(
            np.stack([atto[128:256], atto[384:512]], axis=1)),
        "wout1a": np.ascontiguousarray(np.asarray(inputs["W_out1"], f32)[:P]),
        "wout1b": np.ascontiguousarray(np.asarray(inputs["W_out1"], f32)[P:]),
        "bout1": np.asarray(inputs["b_out1"], f32).reshape(P, 1).copy(),
        "wout2": np.ascontiguousarray(np.asarray(inputs["W_out2"], f32)),
        "bout2": np.asarray(inputs["b_out2"], f32).reshape(P, 1).copy(),
        "clsmW": np.ascontiguousarray(np.asarray(inputs["clsm_W"], f32)),
        "clsmb": np.asarray(inputs["clsm_b"], f32).reshape(10, 1).copy(),
        "clsvW": np.ascontiguousarray(np.asarray(inputs["clsv_W"], f32)),
        "clsvb": np.asarray(inputs["clsv_b"], f32).reshape(10, 1).copy(),
    }
    for pfx in ("m", "v"):
        w1 = np.asarray(inputs[f"fc1{pfx}_W"], f32)
        b1 = np.asarray(inputs[f"fc1{pfx}_b"], f32)
        w2 = np.asarray(inputs[f"fc2{pfx}_W"], f32)
        for j in range(2):
            sh[f"f1{pfx}0{j}"] = np.ascontiguousarray(
                w1[0:P, j * P:(j + 1) * P])
            sh[f"f1{pfx}1{j}"] = np.ascontiguousarray(
                w1[P:2 * P, j * P:(j + 1) * P])
            sh[f"b1{pfx}{j}"] = b1[j * P:(j + 1) * P].reshape(P, 1).copy()
        sh[f"f2{pfx}0"] = np.ascontiguousarray(w2[0:P])
        sh[f"f2{pfx}1"] = np.ascontiguousarray(w2[P:2 * P])
        sh[f"b2{pfx}"] = np.asarray(
            inputs[f"fc2{pfx}_b"], f32).reshape(P, 1).copy()
    return sh


SIGMA = np.concatenate([np.arange(r, P, H) for r in range(H)])  # [128]
SIGMA_INV = np.argsort(SIGMA)


def make_in_maps(inputs):
    x = np.asarray(inputs["x"], np.float32)
    col = np.asarray(inputs["edge_index"], np.int32)[1]
    sh = prepare_shared(inputs)
    in_maps = []
    for c in range(NCORES):
        xs = x[c * NN:(c + 1) * NN].reshape(G, P, -1)[:, SIGMA, :]
        xT = np.ascontiguousarray(xs.reshape(NN, -1).T)
        cs = col[c * NN * K:(c + 1) * NN * K].reshape(G, P, K)
        cs = SIGMA_INV[cs % P][:, SIGMA, :]        # relabel + reorder rows
        colr = np.ascontiguousarray(
            cs.transpose(1, 0, 2).reshape(P, G * K)).astype(np.int32)
        in_maps.append({"xT": xT, "colr": colr, **sh})
    return in_maps


_NC = None


def _ensure_ntff_hook():
    """Register the axon NTFF profiling hook if the image's antenv lacks
    the axon_hooks module (needed for trace=True exec-time capture)."""
    import sys, types
    try:
        from antenv.axon_hooks import get_axon_ntff_profile_hook  # noqa
        return
    except ImportError:
        pass
    try:
        import antenv
        from trn_agent_boot.trn_boot import _ntff_profile_via_ctypes
        mod = types.ModuleType("antenv.axon_hooks")
        hook = _ntff_profile_via_ctypes("/opt/axon/libaxon_pjrt.so")
        mod._hook = hook
        mod.set_axon_ntff_profile_hook = lambda h: setattr(mod, "_hook", h)
        mod.get_axon_ntff_profile_hook = lambda: mod._hook
        sys.modules["antenv.axon_hooks"] = mod
        antenv.axon_hooks = mod
    except Exception as e:  # pragma: no cover
        print(f"ntff hook setup failed: {e}")


def kernel(**inputs):
    global _NC
    if _NC is None:
        _NC = build_nc()
    in_maps = make_in_maps(inputs)
    trace = os.environ.get("BHGNN_TRACE", "") not in ("", "0")
    if trace:
        _ensure_ntff_hook()
    res = run_bass_kernel_spmd(_NC, in_maps, core_ids=list(range(NCORES)),
                               trace=trace)
    full = np.concatenate([res.results[c]["out"] for c in range(NCORES)],
                          axis=0)
    if trace and res.exec_time_ns is not None:
        print(f"HW exec time: {res.exec_time_ns} ns")
    return (np.ascontiguousarray(full[:, 0:10]),
            np.ascontiguousarray(full[:, 10:20]),
            np.ascontiguousarray(full[:, 20:148]))


# revision 24
# speedup vs baseline: 2.3061x; 1.0090x over previous
"""BHGNN Trainium2 kernel (8 NeuronCores, graph-level data parallel).

Per core: 64 graphs x 128 nodes. The sparsemax attention is computed
densely, without any per-edge gather:
  - al[n], ar[n], XW from per-graph PE matmuls on fp16 x^T.
  - Duplicate edge columns are merged by a 16-channel Batcher sort on the
    column ids; GPSIMD local_scatter builds the dense count matrix
    cnt[n, m] per graph.
  - zd1[n, m] = max(ar[m] + al[n] + 1, 1) densely (PE broadcast of ar).
  - Sparsemax threshold via tau+1 = max_m (S[n,m]-1)/N[n,m], where
    S = (cnt*zd1) @ C, N = cnt @ C and C[m',m] = [ar(m') >= ar(m)] is a
    graph-global comparison matrix -> two per-graph PE matmuls.
  - A = cnt * max(zd1 - tau1, max(1-tau1, 0)); agg = (A + I) @ XW via PE
    (identity accumulated in PSUM); h = relu(0.5*agg + b).
  - Pooling (strided DVE reduces) -> outer softmax attention, two tiny
    GCNs, readout, MLP heads in f32.
"""
import os
from contextlib import ExitStack

import numpy as np

import concourse.bass as bass
import concourse.mybir as mybir
import concourse.bacc as bacc
import concourse.tile as tile
from concourse.bass_utils import run_bass_kernel_spmd

FP16 = mybir.dt.float16
F32 = mybir.dt.float32
I32 = mybir.dt.int32
I16 = mybir.dt.int16
AF = mybir.ActivationFunctionType
OP = mybir.AluOpType
X = mybir.AxisListType.X

P = 128          # partitions = nodes per graph
G = 64           # graphs per core
NN = P * G       # nodes per core (8192)
K = 16           # edges per node
NH = 128
H = 6
NCORES = 8
NHYP = G * H     # hypernodes per core (384)

WNAMES = [
    ("watt2", [P, 2]), ("winner", [P, P]), ("binner", [P, 1]),
    ("wattm", [P, 2]), ("wattx", [P, 2]),
    ("wout1a", [P, P]), ("wout1b", [P, P]), ("bout1", [P, 1]),
    ("wout2", [P, P]), ("bout2", [P, 1]),
    ("f1m00", [P, P]), ("f1m10", [P, P]), ("f1m01", [P, P]), ("f1m11", [P, P]),
    ("b1m0", [P, 1]), ("b1m1", [P, 1]),
    ("f2m0", [P, P]), ("f2m1", [P, P]), ("b2m", [P, 1]),
    ("f1v00", [P, P]), ("f1v10", [P, P]), ("f1v01", [P, P]), ("f1v11", [P, P]),
    ("b1v0", [P, 1]), ("b1v1", [P, 1]),
    ("f2v0", [P, P]), ("f2v1", [P, P]), ("b2v", [P, 1]),
    ("clsmW", [P, 10]), ("clsmb", [10, 1]),
    ("clsvW", [P, 10]), ("clsvb", [10, 1]),
]

# Batcher odd-even merge sort network for 16 channels. Each layer:
# (channel-dim factors, lo index, hi index).
SORT_LAYERS = [
    ((8, 2), (slice(None), slice(0, 1)), (slice(None), slice(1, 2))),
    ((4, 4), (slice(None), slice(0, 2)), (slice(None), slice(2, 4))),
    ((4, 4), (slice(None), slice(1, 2)), (slice(None), slice(2, 3))),
    ((2, 8), (slice(None), slice(0, 4)), (slice(None), slice(4, 8))),
    ((2, 8), (slice(None), slice(2, 4)), (slice(None), slice(4, 6))),
    ((2, 4, 2), (slice(None), slice(0, 3), slice(1, 2)),
     (slice(None), slice(1, 4), slice(0, 1))),
    ((1, 16), (slice(None), slice(0, 8)), (slice(None), slice(8, 16))),
    ((1, 16), (slice(None), slice(4, 8)), (slice(None), slice(8, 12))),
    ((4, 4), (slice(0, 3), slice(2, 4)), (slice(1, 4), slice(0, 2))),
    ((8, 2), (slice(0, 7), slice(1, 2)), (slice(1, 8), slice(0, 1))),
]


def _chslice(ap, factors, idx):
    names = "abcd"[: len(factors)]
    pat = f"p ({' '.join(names)} g) -> p {' '.join(names)} g"
    v = ap.rearrange(pat, **{n: f for n, f in zip(names, factors)})
    return v[(slice(None),) + idx + (slice(None),)]


def build_nc():
    nc = bacc.Bacc("TRN2", target_bir_lowering=False, debug=False,
                   num_devices=NCORES)
    xT_d = nc.declare_dram_parameter("xT", [P, NN], F32, isOutput=False)
    col_d = nc.declare_dram_parameter("colr", [P, G * K], I32, isOutput=False)
    wd = {}
    for nm, shp in WNAMES:
        wd[nm] = nc.declare_dram_parameter(nm, shp, F32, isOutput=False)
    out_d = nc.declare_dram_parameter("out", [G, 148], F32, isOutput=True)

    with tile.TileContext(nc) as tc, ExitStack() as ctx:
        pp = ctx.enter_context(tc.tile_pool(name="persist", bufs=1))
        wk = ctx.enter_context(tc.tile_pool(name="work", bufs=1))
        pmm = ctx.enter_context(
            tc.tile_pool(name="psum", bufs=8, space="PSUM"))

        ps_ctr = [0]

        def ps(shape, dtype=F32):
            ps_ctr[0] += 1
            return pmm.tile(shape, dtype, tag="ps", name=f"pst{ps_ctr[0]}")

        # ---------------- weights / constants ----------------
        w = {}
        for nm, shp in WNAMES:
            t = pp.tile(shp, F32, tag=f"w_{nm}")
            nc.sync.dma_start(t[:], wd[nm].ap())
            w[nm] = t
        watt2_h = pp.tile([P, 2], FP16, tag="watt2h")
        nc.gpsimd.dma_start(watt2_h[:], wd["watt2"].ap())
        winner_h = pp.tile([P, P], FP16, tag="winnerh")
        nc.gpsimd.dma_start(winner_h[:], wd["winner"].ap())

        iota_row = pp.tile([P, P], I32, tag="iota_row")
        nc.gpsimd.iota(iota_row[:], pattern=[[1, P]], channel_multiplier=0)
        iota_part = pp.tile([P, 1], I32, tag="iota_part")
        nc.gpsimd.iota(iota_part[:], pattern=[[0, 1]], channel_multiplier=1)
        g128rep = pp.tile([P, K * G], I32, tag="g128rep")   # (k,g): 128*(g%8)
        nc.gpsimd.iota(g128rep[:], pattern=[[0, K], [0, 8], [P, 8]],
                       channel_multiplier=0)

        iota_rowf = pp.tile([P, P], F32, tag="iota_rowf")
        nc.vector.tensor_copy(iota_rowf[:], iota_row[:])
        iota_partf = pp.tile([P, 1], F32, tag="iota_partf")
        nc.vector.tensor_copy(iota_partf[:], iota_part[:])
        ident32 = pp.tile([P, P], F32, tag="ident32")
        nc.vector.tensor_scalar(ident32[:], iota_rowf[:], iota_partf[:, 0:1],
                                None, op0=OP.is_equal)
        ident16 = pp.tile([P, P], FP16, tag="ident16")
        nc.vector.tensor_copy(ident16[:], ident32[:])
        ones_col = pp.tile([1, P], F32, tag="ones_col")
        nc.vector.memset(ones_col[:], 1.0)
        ones16 = pp.tile([1, P], FP16, tag="ones16")
        nc.vector.memset(ones16[:], 1.0)
        cntinv = pp.tile([P, H * G], F32, tag="cntinv")     # (g, r)
        nc.vector.memset(cntinv[:], 1.0 / 21.0)
        nc.vector.memset(
            cntinv.rearrange("p (g r) -> p g r", g=G)[:, :, 0:2],
            1.0 / 22.0)

        # ---------------- x^T (fp16 cast) and col ----------------
        xT = pp.tile([P, NN], FP16, tag="xT")
        for c in range(8):
            sl = slice(c * 1024, (c + 1) * 1024)
            nc.gpsimd.dma_start(xT[:, sl], xT_d.ap()[:, sl])
        colr = pp.tile([P, G * K], I32, tag="colr")
        nc.sync.dma_start(colr[:], col_d.ap())

        # ---------------- stage A: al, ar, XW per graph ----------------
        al_sb = wk.tile([P, G], F32, tag="al")
        arc_sb = wk.tile([P, G], F32, tag="arc")
        xw16 = wk.tile([P, G * P], FP16, tag="xw16")
        for b in range(G // 4):
            ps_xw = ps([P, 512])
            ps_al = ps([P, 8])
            for i in range(4):
                g = b * 4 + i
                lhs = xT[:, g * P:(g + 1) * P]
                nc.tensor.matmul(ps_al[:, 2 * i:2 * i + 2], lhs, watt2_h[:])
                nc.tensor.matmul(ps_xw[:, i * P:(i + 1) * P], lhs,
                                 winner_h[:])
            alr = ps_al.rearrange("p (i t) -> p i t", i=4)
            nc.vector.tensor_copy(
                al_sb.rearrange("p (b i) -> p b i", b=G // 4)[:, b],
                alr[:, :, 0])
            nc.vector.tensor_copy(
                arc_sb.rearrange("p (b i) -> p b i", b=G // 4)[:, b],
                alr[:, :, 1])
            nc.scalar.copy(xw16[:, b * 512:(b + 1) * 512], ps_xw[:])

        # ar as fp16-rounded per-node f32 scalars (consistent with ar_all)
        arc16 = wk.tile([P, G], FP16, tag="arc16")
        nc.vector.tensor_copy(arc16[:], arc_sb[:])
        arcr = wk.tile([P, G], F32, tag="arcr")
        nc.vector.tensor_copy(arcr[:], arc16[:])
        al1h = wk.tile([P, G], FP16, tag="al1h")
        nc.vector.tensor_scalar_add(al1h[:], al_sb[:], 1.0)

        # ar replicated on all partitions: [p, (g*128+m)] fp16
        ar_flat = wk.tile([1, NN], FP16, tag="arflat")
        for c in range(NN // 512):
            ps_ar = ps([1, 512])
            nc.tensor.matmul(ps_ar[:], watt2_h[:, 1:2],
                             xT[:, c * 512:(c + 1) * 512])
            nc.vector.tensor_copy(ar_flat[:, c * 512:(c + 1) * 512],
                                  ps_ar[:])
        ar_all = wk.tile([P, NN], FP16, tag="arall")
        for c in range(NN // 512):
            ps_b = ps([P, 512])
            nc.tensor.matmul(ps_b[:], ones16[:],
                             ar_flat[:, c * 512:(c + 1) * 512])
            nc.scalar.copy(ar_all[:, c * 512:(c + 1) * 512], ps_b[:])

        # ---------------- stage C: dedup cols -> cnt matrix ----------------
        colf = wk.tile([P, K * G], FP16, tag="colf")         # (k, g)
        colt = wk.tile([P, K * G], I32, tag="ndead")
        nc.vector.tensor_copy(colt[:],
                              colr.rearrange("p (g j) -> p j g", g=G))
        nc.vector.tensor_scalar(colt[:], colt[:], 127, None,
                                op0=OP.bitwise_and)
        nc.vector.tensor_copy(colf[:], colt[:])
        sorttmp = wk.tile([P, 8 * G], FP16, tag="sorttmp")
        for factors, lo_i, hi_i in SORT_LAYERS:
            lo = _chslice(colf, factors, lo_i)
            hi = _chslice(colf, factors, hi_i)
            ext = tuple(len(range(*s.indices(f)))
                        for s, f in zip(lo_i, factors))
            npair = int(np.prod(ext))
            tmp = _chslice(sorttmp[:, 0:npair * G], ext,
                           tuple(slice(None) for _ in ext))
            nc.vector.tensor_tensor(tmp, lo, hi, op=OP.min)
            nc.vector.tensor_tensor(lo, lo, hi, op=OP.max)
            nc.vector.tensor_copy(hi, tmp)
        # eq channels with zero pad; dead_k = eq_{k+1}
        eqt = wk.tile([P, (K + 1) * G], FP16, tag="eqt")
        nc.vector.memset(eqt[:, 0:G], 0.0)
        nc.vector.memset(eqt[:, K * G:], 0.0)
        nc.vector.tensor_tensor(eqt[:, G:K * G], colf[:, G:],
                                colf[:, 0:(K - 1) * G], op=OP.is_equal)
        # run counts: c_k = 1 + eq_k * c_{k-1}; last slot of run holds total
        cntc = wk.tile([P, K * G], FP16, tag="cntc")
        nc.vector.memset(cntc[:, 0:G], 1.0)
        for k in range(1, K):
            nc.vector.tensor_mul(cntc[:, k * G:(k + 1) * G],
                                 eqt[:, k * G:(k + 1) * G],
                                 cntc[:, (k - 1) * G:k * G])
            nc.vector.tensor_scalar_add(cntc[:, k * G:(k + 1) * G],
                                        cntc[:, k * G:(k + 1) * G], 1.0)
        deadt = eqt[:, G:]
        ndead = wk.tile([P, K * G], FP16, tag="ndead")
        nc.vector.tensor_scalar(ndead[:], deadt, -1.0, 1.0,
                                op0=OP.mult, op1=OP.add)
        rampf = wk.tile([P, K * G], FP16, tag="rampf")
        nc.vector.tensor_copy(rampf[:], g128rep[:])
        # idx = dead ? -1 : col + 128*(g%8)  == (col+ramp+1)*ndead - 1
        idxf = wk.tile([P, K * G], FP16, tag="idxf")
        nc.vector.tensor_add(idxf[:], colf[:], rampf[:])
        nc.vector.scalar_tensor_tensor(idxf[:], idxf[:], 1.0, ndead[:],
                                       op0=OP.add, op1=OP.mult)
        nc.vector.tensor_scalar_add(idxf[:], idxf[:], -1.0)
        sidx = wk.tile([P, G * K], I16, tag="sidx")
        sval = wk.tile([P, G * K], FP16, tag="sval")
        nc.vector.tensor_copy(sidx.rearrange("p (g k) -> p k g", g=G),
                              idxf.rearrange("p (k g) -> p k g", k=K))
        nc.vector.tensor_copy(sval.rearrange("p (g k) -> p k g", g=G),
                              cntc.rearrange("p (k g) -> p k g", k=K))
        cntd = wk.tile([P, G * P], FP16, tag="cntd")
        for wnd in range(8):
            nc.gpsimd.local_scatter(
                cntd[:, wnd * 1024:(wnd + 1) * 1024],
                sval[:, wnd * 128:(wnd + 1) * 128],
                sidx[:, wnd * 128:(wnd + 1) * 128],
                channels=P, num_elems=1024, num_idxs=128)

        # ---------------- stage D: zd1, S/N matmuls, tau ----------------
        zd1 = wk.tile([P, NN], FP16, tag="zd1")
        zd3 = zd1.rearrange("p (g m) -> p g m", g=G)
        al1b = al1h.rearrange("p (g o) -> p g o", o=1) \
            .broadcast_to([P, G, P])
        nc.vector.tensor_tensor(zd3, ar_all.rearrange("p (g m) -> p g m",
                                                      g=G), al1b, op=OP.add)
        nc.vector.tensor_scalar_max(zd1[:], zd1[:], 1.0)

        tau = wk.tile([P, G], F32, tag="tau")
        nz = wk.tile([P, G], F32, tag="nz")
        sz = wk.tile([P, G], F32, tag="sz")
        NQ = NN // 4
        for qt in range(4):
            qsl = slice(qt * NQ, (qt + 1) * NQ)
            gsl = slice(qt * 16, (qt + 1) * 16)
            S_h = wk.tile([P, NQ], FP16, tag="Sh", bufs=3)
            N_h = wk.tile([P, NQ], FP16, tag="Nh", bufs=3)
            czh = wk.tile([P, NQ], FP16, tag="czh", bufs=2)
            nc.vector.tensor_mul(czh[:], cntd[:, qsl], zd1[:, qsl])
            Ch = wk.tile([P, NQ], FP16, tag="tq", bufs=2)
            for i in range(16):
                g = qt * 16 + i
                nc.vector.tensor_scalar(
                    Ch[:, i * P:(i + 1) * P], ar_all[:, g * P:(g + 1) * P],
                    arcr[:, g:g + 1], None, op0=OP.is_le)
            for b in range(4):
                gb = qt * 4 + b
                ps_t1 = ps([P, 512], FP16)
                ps_t2 = ps([P, 512], FP16)
                for i in range(4):
                    g = gb * 4 + i
                    bsl = slice((b * 4 + i) * P, (b * 4 + i + 1) * P)
                    nc.tensor.transpose(ps_t1[:, i * P:(i + 1) * P],
                                        cntd[:, g * P:(g + 1) * P],
                                        ident16[:])
                    nc.tensor.transpose(ps_t2[:, i * P:(i + 1) * P],
                                        czh[:, bsl], ident16[:])
                ctb = wk.tile([P, 512], FP16, tag="ctb", bufs=3)
                cztb = wk.tile([P, 512], FP16, tag="cztb", bufs=3)
                nc.scalar.copy(ctb[:], ps_t1[:])
                nc.scalar.copy(cztb[:], ps_t2[:])
                ps_s = ps([P, 512])
                ps_n = ps([P, 512])
                for i in range(4):
                    bsl = slice((b * 4 + i) * P, (b * 4 + i + 1) * P)
                    nc.tensor.matmul(ps_s[:, i * P:(i + 1) * P],
                                     cztb[:, i * P:(i + 1) * P], Ch[:, bsl])
                    nc.tensor.matmul(ps_n[:, i * P:(i + 1) * P],
                                     ctb[:, i * P:(i + 1) * P], Ch[:, bsl])
                nc.scalar.copy(S_h[:, b * 512:(b + 1) * 512], ps_s[:])
                nc.scalar.copy(N_h[:, b * 512:(b + 1) * 512], ps_n[:])
            # support test (division-free): m in support iff N*zd1 - S > -1
            t1 = wk.tile([P, NQ], FP16, tag="rq", bufs=2)
            nc.vector.tensor_mul(t1[:], N_h[:], zd1[:, qsl])
            nc.vector.tensor_sub(t1[:], t1[:], S_h[:])
            nc.vector.tensor_scalar(t1[:], t1[:], -1.0, None, op0=OP.is_gt)
            nc.vector.tensor_mul(t1[:], t1[:], cntd[:, qsl])
            nc.vector.tensor_reduce(
                nz[:, gsl], t1.rearrange("p (g m) -> p g m", g=16),
                axis=X, op=OP.add)
            nc.vector.tensor_mul(t1[:], t1[:], zd1[:, qsl])
            nc.vector.tensor_reduce(
                sz[:, gsl], t1.rearrange("p (g m) -> p g m", g=16),
                axis=X, op=OP.add)
        # tau = (SZ - 1) / NZ  (tiny division)
        rnz = wk.tile([P, G], F32, tag="rnz")
        nc.vector.reciprocal(rnz[:], nz[:])
        nc.vector.scalar_tensor_tensor(tau[:], sz[:], -1.0, rnz[:],
                                       op0=OP.add, op1=OP.mult)

        # ---------------- stage E: A dense, transpose, agg ----------------
        ntau16 = wk.tile([P, G], FP16, tag="ntau16")
        nc.vector.tensor_scalar_mul(ntau16[:], tau[:], -1.0)
        gam16 = wk.tile([P, G], FP16, tag="gam16")
        nc.vector.tensor_scalar(gam16[:], tau[:], -1.0, 1.0,
                                op0=OP.mult, op1=OP.add)
        nc.vector.tensor_scalar_max(gam16[:], gam16[:], 0.0)
        A = wk.tile([P, G * P], FP16, tag="arall")
        A3 = A.rearrange("p (g m) -> p g m", g=G)
        ntb = ntau16.rearrange("p (g o) -> p g o", o=1) \
            .broadcast_to([P, G, P])
        gmb = gam16.rearrange("p (g o) -> p g o", o=1) \
            .broadcast_to([P, G, P])
        nc.vector.tensor_tensor(A3, zd3, ntb, op=OP.add)
        nc.vector.tensor_tensor(A3, A3, gmb, op=OP.max)
        nc.vector.tensor_mul(A[:], A[:], cntd[:])

        hT = wk.tile([P, G * P], FP16, tag="cntd")
        for b in range(G // 4):
            ps_t = ps([P, 512], FP16)
            for i in range(4):
                g = b * 4 + i
                nc.tensor.transpose(ps_t[:, i * P:(i + 1) * P],
                                    A[:, g * P:(g + 1) * P], ident16[:])
            atb = wk.tile([P, 512], FP16, tag="atb", bufs=2)
            nc.scalar.copy(atb[:], ps_t[:])
            ps_a = ps([P, 512])
            for i in range(4):
                g = b * 4 + i
                nc.tensor.matmul(ps_a[:, i * P:(i + 1) * P],
                                 xw16[:, g * P:(g + 1) * P],
                                 atb[:, i * P:(i + 1) * P],
                                 start=True, stop=False)
                nc.tensor.matmul(ps_a[:, i * P:(i + 1) * P],
                                 xw16[:, g * P:(g + 1) * P],
                                 ident16[:], start=False, stop=True)
            nc.scalar.activation(hT[:, b * 512:(b + 1) * 512], ps_a[:],
                                 AF.Relu, bias=w["binner"][:, 0:1],
                                 scale=0.5)

        # ---------------- stage F: pooling ----------------
        xhm = wk.tile([P, NHYP], F32, tag="xw16")           # (g, r) mean
        xhx = wk.tile([P, NHYP], F32, tag="colf")           # (g, r) max
        # node order is sigma-permuted (host side): positions [0:44] are
        # hypernodes 0-1 (22 each), [44:128] are hypernodes 2-5 (21 each).
        hT_v = hT.rearrange("p (g n) -> p g n", g=G)
        seg_a = hT_v[:, :, 0:44].rearrange("p g (r kk) -> p g r kk", r=2)
        seg_b = hT_v[:, :, 44:128].rearrange("p g (r kk) -> p g r kk", r=4)
        xhm_v = xhm.rearrange("p (g r) -> p g r", g=G)
        xhx_v = xhx.rearrange("p (g r) -> p g r", g=G)
        nc.vector.tensor_reduce(xhm_v[:, :, 0:2], seg_a, axis=X, op=OP.add)
        nc.vector.tensor_reduce(xhm_v[:, :, 2:6], seg_b, axis=X, op=OP.add)
        nc.vector.tensor_mul(xhm[:], xhm[:], cntinv[:])
        nc.vector.tensor_reduce(xhx_v[:, :, 0:2], seg_a, axis=X, op=OP.max)
        nc.vector.tensor_reduce(xhx_v[:, :, 2:6], seg_b, axis=X, op=OP.max)

        # ---------------- stage G: outer attention ----------------
        ps_w = ps([2, NHYP])
        nc.tensor.matmul(ps_w[:], w["wattm"][:], xhm[:], start=True,
                         stop=False)
        nc.tensor.matmul(ps_w[:], w["wattx"][:], xhx[:], start=False,
                         stop=True)
        wlr2 = wk.tile([2, NHYP], F32, tag="wlr2")
        nc.vector.tensor_copy(wlr2[:], ps_w[:])
        wlr = wk.tile([G, 12], F32, tag="wlr")
        nc.sync.dma_start(wlr[:, 0:6], wlr2[0:1, :])
        nc.sync.dma_start(wlr[:, 6:12], wlr2[1:2, :])
        whm = wk.tile([G, 36], F32, tag="whm")
        whm_v = whm.rearrange("g (r s) -> g r s", r=H)
        for r in range(H):
            nc.vector.tensor_scalar(whm_v[:, r], wlr[:, 6:12],
                                    wlr[:, r:r + 1], None, op0=OP.add)
        wt = wk.tile([G, 36], F32, tag="wt36")
        wt_v = wt.rearrange("g (r s) -> g r s", r=H)
        nc.vector.tensor_scalar_min(wt[:], whm[:], 0.0)
        nc.vector.tensor_scalar_max(whm[:], whm[:], 0.0)
        nc.vector.scalar_tensor_tensor(whm[:], wt[:], 0.2, whm[:],
                                       op0=OP.mult, op1=OP.add)
        rmax = wk.tile([G, H], F32, tag="rmax")
        nc.vector.tensor_tensor(wt_v[:, :, 0:3], whm_v[:, :, 0:3],
                                whm_v[:, :, 3:6], op=OP.max)
        nc.vector.tensor_tensor(rmax[:], wt_v[:, :, 0], wt_v[:, :, 1],
                                op=OP.max)
        nc.vector.tensor_tensor(rmax[:], rmax[:], wt_v[:, :, 2], op=OP.max)
        for r in range(H):
            nc.vector.tensor_scalar(whm_v[:, r], whm_v[:, r],
                                    rmax[:, r:r + 1], None, op0=OP.subtract)
        nc.scalar.activation(whm[:], whm[:], AF.Exp)
        rsum = wk.tile([G, H], F32, tag="rsum")
        nc.vector.tensor_tensor(wt_v[:, :, 0:3], whm_v[:, :, 0:3],
                                whm_v[:, :, 3:6], op=OP.add)
        nc.vector.tensor_tensor(rsum[:], wt_v[:, :, 0], wt_v[:, :, 1],
                                op=OP.add)
        nc.vector.tensor_tensor(rsum[:], rsum[:], wt_v[:, :, 2], op=OP.add)
        nc.vector.reciprocal(rsum[:], rsum[:])
        for r in range(H):
            nc.vector.tensor_scalar(whm_v[:, r], whm_v[:, r],
                                    rsum[:, r:r + 1], None, op0=OP.mult)
        ahflat = wk.tile([1, G * 36], F32, tag="arflat")
        nc.sync.dma_start(ahflat[:], whm[:])
        ahrep = wk.tile([P, G * 36], F32, tag="zd1")
        for c in range(5):
            lo = c * 512
            n = min(512, G * 36 - lo)
            ps_b2 = ps([P, 512])
            nc.tensor.matmul(ps_b2[:, 0:n], ones_col[:], ahflat[:, lo:lo + n])
            nc.scalar.copy(ahrep[:, lo:lo + n], ps_b2[:, 0:n])
        ah_v = ahrep.rearrange("p (g q) -> p g q", g=G)

        def outer_gcn(xin_m, xin_x, wa, wb, bias, name):
            p1 = ps([P, NHYP])
            if xin_x is None:
                nc.tensor.matmul(p1[:], wa[:], xin_m[:])
            else:
                nc.tensor.matmul(p1[:], wa[:], xin_m[:], start=True,
                                 stop=False)
                nc.tensor.matmul(p1[:], wb[:], xin_x[:], start=False,
                                 stop=True)
            xwT = wk.tile([P, NHYP], F32, tag="xwT")
            nc.vector.tensor_copy(xwT[:], p1[:])
            agg = wk.tile([P, NHYP], F32, tag="agg")
            agg_v = agg.rearrange("p (g r) -> p g r", g=G)
            xw_v = xwT.rearrange("p (g s) -> p g s", g=G)
            tmpa = wk.tile([P, G], F32, tag="tmpa")
            for r in range(H):
                for s in range(H):
                    if s == 0:
                        nc.vector.tensor_mul(agg_v[:, :, r], xw_v[:, :, s],
                                             ah_v[:, :, r * H + s])
                    else:
                        nc.vector.tensor_mul(tmpa[:], xw_v[:, :, s],
                                             ah_v[:, :, r * H + s])
                        nc.vector.tensor_tensor(agg_v[:, :, r],
                                                agg_v[:, :, r], tmpa[:],
                                                op=OP.add)
            nc.vector.tensor_add(agg[:], agg[:], xwT[:])
            zT = wk.tile([P, NHYP], F32, tag="zT")
            nc.scalar.activation(zT[:], agg[:], AF.Relu, bias=bias[:, 0:1],
                                 scale=0.5)
            return zT

        z1h = outer_gcn(xhm, xhx, w["wout1a"], w["wout1b"], w["bout1"], "o1")
        z2h = outer_gcn(z1h, None, w["wout2"], None, w["bout2"], "o2")

        x1m = wk.tile([P, G], F32, tag="x1m")
        x1x = wk.tile([P, G], F32, tag="x1x")
        z2_v = z2h.rearrange("p (g r) -> p g r", g=G)
        nc.vector.tensor_reduce(x1m[:], z2_v, axis=X, op=OP.add)
        nc.vector.tensor_scalar_mul(x1m[:], x1m[:], 1.0 / H)
        nc.vector.tensor_reduce(x1x[:], z2_v, axis=X, op=OP.max)

        # ---------------- MLP heads ----------------
        def head(pfx, xm, xx):
            m1 = []
            for j in range(2):
                p2 = ps([P, G])
                nc.tensor.matmul(p2[:], w[f"f1{pfx}0{j}"][:], xm[:],
                                 start=True, stop=False)
                nc.tensor.matmul(p2[:], w[f"f1{pfx}1{j}"][:], xx[:],
                                 start=False, stop=True)
                t = wk.tile([P, G], F32, tag=f"m1{pfx}{j}")
                nc.scalar.activation(t[:], p2[:], AF.Relu,
                                     bias=w[f"b1{pfx}{j}"][:, 0:1])
                m1.append(t)
            p3 = ps([P, G])
            nc.tensor.matmul(p3[:], w[f"f2{pfx}0"][:], m1[0][:],
                             start=True, stop=False)
            nc.tensor.matmul(p3[:], w[f"f2{pfx}1"][:], m1[1][:],
                             start=False, stop=True)
            mT = wk.tile([P, G], F32, tag=f"mT{pfx}")
            nc.scalar.activation(mT[:], p3[:], AF.Relu,
                                 bias=w[f"b2{pfx}"][:, 0:1])
            p4 = ps([10, G])
            nc.tensor.matmul(p4[:], w[f"cls{pfx}W"][:], mT[:])
            o = wk.tile([10, G], F32, tag=f"o{pfx}")
            nc.vector.tensor_scalar(o[:], p4[:], w[f"cls{pfx}b"][:, 0:1],
                                    None, op0=OP.add)
            return mT, o

        mT, om = head("m", x1m, x1x)
        _, ov = head("v", x1m, x1x)

        # ---------------- outputs ----------------
        p5 = ps([G, P])
        nc.tensor.transpose(p5[:], mT[:], ident32[:])
        m_t = wk.tile([G, P], F32, tag="m_t")
        nc.vector.tensor_copy(m_t[:], p5[:])
        nc.sync.dma_start(out_d.ap()[:, 20:148], m_t[:])
        for o_ap, cols in ((om, slice(0, 10)), (ov, slice(10, 20))):
            p6 = ps([G, 10])
            nc.tensor.transpose(p6[:], o_ap[:], ident32[0:10, 0:10])
            o_t = wk.tile([G, 10], F32, tag="o_t")
            nc.vector.tensor_copy(o_t[:], p6[:])
            nc.sync.dma_start(out_d.ap()[:, cols], o_t[:])

    nc.compile()
    return nc


def prepare_shared(inputs):
    f32 = np.float32
    att = np.asarray(inputs["att_inner"], f32)
    atto = np.asarray(inputs["att_outer"], f32)
    sh = {
        "watt2": np.ascontiguousarray(np.stack([att[:P], att[P:]], axis=1)),
        "winner": np.ascontiguousarray(np.asarray(inputs["W_inner"], f32)),
        "binner": np.asarray(inputs["b_inner"], f32).reshape(P, 1).copy(),
        "wattm": np.ascontiguousarray(
            np.stack([atto[0:128], atto[256:384]], axis=1)),
        "wattx": np.ascontiguousarray(
            np.stack([atto[128:256], atto[384:512]], axis=1)),
        "wout1a": np.ascontiguousarray(np.asarray(inputs["W_out1"], f32)[:P]),
        "wout1b": np.ascontiguousarray(np.asarray(inputs["W_out1"], f32)[P:]),
        "bout1": np.asarray(inputs["b_out1"], f32).reshape(P, 1).copy(),
        "wout2": np.ascontiguousarray(np.asarray(inputs["W_out2"], f32)),
        "bout2": np.asarray(inputs["b_out2"], f32).reshape(P, 1).copy(),
        "clsmW": np.ascontiguousarray(np.asarray(inputs["clsm_W"], f32)),
        "clsmb": np.asarray(inputs["clsm_b"], f32).reshape(10, 1).copy(),
        "clsvW": np.ascontiguousarray(np.asarray(inputs["clsv_W"], f32)),
        "clsvb": np.asarray(inputs["clsv_b"], f32).reshape(10, 1).copy(),
    }
    for pfx in ("m", "v"):
        w1 = np.asarray(inputs[f"fc1{pfx}_W"], f32)
        b1 = np.asarray(inputs[f"fc1{pfx}_b"], f32)
        w2 = np.asarray(inputs[f"fc2{pfx}_W"], f32)
        for j in range(2):
            sh[f"f1{pfx}0{j}"] = np.ascontiguousarray(
                w1[0:P, j * P:(j + 1) * P])
            sh[f"f1{pfx}1{j}"] = np.ascontiguousarray(
                w1[P:2 * P, j * P:(j + 1) * P])
            sh[f"b1{pfx}{j}"] = b1[j * P:(j + 1) * P].reshape(P, 1).copy()
        sh[f"f2{pfx}0"] = np.ascontiguousarray(w2[0:P])
        sh[f"f2{pfx}1"] = np.ascontiguousarray(w2[P:2 * P])
        sh[f"b2{pfx}"] = np.asarray(
            inputs[f"fc2{pfx}_b"], f32).reshape(P, 1).copy()
    return sh


SIGMA = np.concatenate([np.arange(r, P, H) for r in range(H)])  # [128]
SIGMA_INV = np.argsort(SIGMA)


def make_in_maps(inputs):
    x = np.asarray(inputs["x"], np.float32)
    col = np.asarray(inputs["edge_index"], np.int32)[1]
    sh = prepare_shared(inputs)
    in_maps = []
    for c in range(NCORES):
        xs = x[c * NN:(c + 1) * NN].reshape(G, P, -1)[:, SIGMA, :]
        xT = np.ascontiguousarray(xs.reshape(NN, -1).T)
        cs = col[c * NN * K:(c + 1) * NN * K].reshape(G, P, K)
        cs = SIGMA_INV[cs % P][:, SIGMA, :]        # relabel + reorder rows
        colr = np.ascontiguousarray(
            cs.transpose(1, 0, 2).reshape(P, G * K)).astype(np.int32)
        in_maps.append({"xT": xT, "colr": colr, **sh})
    return in_maps


_NC = None


def _ensure_ntff_hook():
    """Register the axon NTFF profiling hook if the image's antenv lacks
    the axon_hooks module (needed for trace=True exec-time capture)."""
    import sys, types
    try:
        from antenv.axon_hooks import get_axon_ntff_profile_hook  # noqa
        return
    except ImportError:
        pass
    try:
        import antenv
        from trn_agent_boot.trn_boot import _ntff_profile_via_ctypes
        mod = types.ModuleType("antenv.axon_hooks")
        hook = _ntff_profile_via_ctypes("/opt/axon/libaxon_pjrt.so")
        mod._hook = hook
        mod.set_axon_ntff_profile_hook = lambda h: setattr(mod, "_hook", h)
        mod.get_axon_ntff_profile_hook = lambda: mod._hook
        sys.modules["antenv.axon_hooks"] = mod
        antenv.axon_hooks = mod
    except Exception as e:  # pragma: no cover
        print(f"ntff hook setup failed: {e}")


def kernel(**inputs):
    global _NC
    if _NC is None:
        _NC = build_nc()
    in_maps = make_in_maps(inputs)
    trace = os.environ.get("BHGNN_TRACE", "") not in ("", "0")
    if trace:
        _ensure_ntff_hook()
    res = run_bass_kernel_spmd(_NC, in_maps, core_ids=list(range(NCORES)),
                               trace=trace)
    full = np.concatenate([res.results[c]["out"] for c in range(NCORES)],
                          axis=0)
    if trace and res.exec_time_ns is not None:
        print(f"HW exec time: {res.exec_time_ns} ns")
    return (np.ascontiguousarray(full[:, 0:10]),
            np.ascontiguousarray(full[:, 10:20]),
            np.ascontiguousarray(full[:, 20:148]))
